# revision 1
# baseline (speedup 1.0000x reference)
"""Trainium2 Bass kernel for 4-layer cross-stencil CNN.

Per-core: one image [6,256,256] (batch dim sharded across 8 cores).
conv(cross-5-stencil) = 5 channel-matmuls with spatially shifted rhs APs,
accumulated in PSUM. Channels on partitions, spatial (rows x cols) on the
free dim. fp32r matmuls (full PE rate at N>=256).

Strips of R output rows with overlap-compute for the halos; all four
layers fused in SBUF (no DRAM intermediates).

L1 packs the 5 taps into K=30 via a 5-group pre-shifted input buffer
(one matmul per chunk). L4 computes the 4 shifted taps as one M=128
matmul whose output slabs sit at partitions 0/32/64/96 (legal engine
bases), the center tap as an M=6 matmul, and the shifted tap-sum runs on
DVE in bf16; the tap-sum for strip k-1 is emitted interleaved into strip
k's L2 phase so the DVE FIFO never blocks the next strip's L1 copies.
"""

import sys

sys.path.insert(0, "/opt/trn_rl_repo")

import ml_dtypes
import numpy as np

import concourse.bacc as bacc
import concourse.mybir as mybir
from concourse.tile import TileContext
from concourse import bass_utils

IN_C, HID_C, OUT_C = 6, 128, 6
B, H, W = 8, 256, 256
WP = W + 2  # padded width
R = 24  # output rows per strip
N_CORES = 8

f32 = mybir.dt.float32
f32r = mybir.dt.float32r
bf16 = mybir.dt.bfloat16
Add = mybir.AluOpType.add
Max = mybir.AluOpType.max
Relu = mybir.ActivationFunctionType.Relu
Ident = mybir.ActivationFunctionType.Identity

# tap order matches reference: 0=center, 1=up(x[h-1]), 2=down(x[h+1]),
# 3=left(x[w-1]), 4=right(x[w+1])


def _build(repeat=1, stages=6, interleave=True):
    nc = bacc.Bacc("TRN2", target_bir_lowering=False)

    x_d = nc.dram_tensor("x", [IN_C, H, W], f32, kind="ExternalInput")
    w1_d = nc.dram_tensor("w1p", [5 * IN_C, HID_C], f32, kind="ExternalInput")
    w2_d = nc.dram_tensor("w2p", [HID_C, 5, HID_C], f32, kind="ExternalInput")
    w3_d = nc.dram_tensor("w3p", [HID_C, 5, HID_C], f32, kind="ExternalInput")
    # w4a: all 5 taps as M=128 slabs: up@0-5, center@6-11, down@32-37,
    # left@64-69, right@96-101; zero elsewhere
    w4a_d = nc.dram_tensor("w4a", [HID_C, HID_C], f32, kind="ExternalInput")
    # s6: bf16 selector summing the 5 (pre-shifted) slabs of t5s
    s6_d = nc.dram_tensor("s6", [HID_C, OUT_C], bf16, kind="ExternalInput")
    b1_d = nc.dram_tensor("b1", [HID_C], f32, kind="ExternalInput")
    b2_d = nc.dram_tensor("b2", [HID_C], f32, kind="ExternalInput")
    b3_d = nc.dram_tensor("b3", [HID_C], f32, kind="ExternalInput")
    b4_d = nc.dram_tensor("b4", [OUT_C], f32, kind="ExternalInput")
    y_d = nc.dram_tensor("y", [OUT_C, H, W], f32, kind="ExternalOutput")

    with TileContext(nc) as tc:
        with (
            tc.tile_pool(name="const", bufs=1) as cpool,
            tc.tile_pool(name="bufs", bufs=1) as bpool,
            tc.tile_pool(name="io", bufs=4) as iopool,
            tc.tile_pool(name="psmain", bufs=7, space="PSUM") as pmain,
        ):
            # --- weights / biases (resident) ---
            w1_sb = cpool.tile([5 * IN_C, HID_C], f32r)
            nc.sync.dma_start(out=w1_sb, in_=w1_d[:, :].bitcast(f32r))
            w2_sb = cpool.tile([HID_C, 5, HID_C], f32r)
            nc.sync.dma_start(out=w2_sb, in_=w2_d[:, :, :].bitcast(f32r))
            w3_sb = cpool.tile([HID_C, 5, HID_C], f32r)
            nc.sync.dma_start(out=w3_sb, in_=w3_d[:, :, :].bitcast(f32r))
            w4a_sb = cpool.tile([HID_C, HID_C], f32r)
            nc.sync.dma_start(out=w4a_sb, in_=w4a_d[:, :].bitcast(f32r))
            s6_sb = cpool.tile([HID_C, OUT_C], bf16)
            nc.sync.dma_start(out=s6_sb, in_=s6_d[:, :])
            b1_sb = cpool.tile([HID_C, 1], f32)
            nc.sync.dma_start(out=b1_sb, in_=b1_d[:, None])
            b2_sb = cpool.tile([HID_C, 1], f32)
            nc.sync.dma_start(out=b2_sb, in_=b2_d[:, None])
            b3_sb = cpool.tile([HID_C, 1], f32)
            nc.sync.dma_start(out=b3_sb, in_=b3_d[:, None])
            b4_sb = cpool.tile([OUT_C, 1], f32)
            nc.sync.dma_start(out=b4_sb, in_=b4_d[:, None])

            # --- persistent strip buffers (bufs=1; pads zeroed once) ---
            # x30: 5 tap-groups x 6ch, pre-shifted by DMA placement.
            # group g partitions [6g,6g+6); center x(h,w) -> (slot h-a+5, col w+1)
            x30 = bpool.tile([5 * IN_C, R + 10, WP], f32r)
            h1 = bpool.tile([HID_C, R + 6, WP], f32r)  # L1 rows [a-3,b+3)
            h2 = bpool.tile([HID_C, R + 4, WP], f32r)  # L2 rows [a-2,b+2)
            h3 = bpool.tile([HID_C, R + 2, WP], f32r)  # L3 rows [a-1,b+1)
            # t5: tap partials (slabs up@0,cen@6,dn@32,lf@64,rt@96), bf16
            t5 = bpool.tile([HID_C, R + 2, WP], bf16)
            # t5s: DMA-gathered pre-shifted taps; slot d = output row a+d
            t5s = bpool.tile([HID_C, R, WP], bf16)

            # zero only cells that are read but never written (all base-0 APs)
            nc.vector.memset(x30[:, 0:6, :].bitcast(f32), 0.0)
            nc.vector.memset(x30[:, :, 1:2].bitcast(f32), 0.0)
            nc.vector.memset(x30[:, :, 256:257].bitcast(f32), 0.0)
            for _t, _topz in ((h1, 3), (h2, 2), (h3, 1)):
                nc.vector.memset(_t[:, :, 0:1].bitcast(f32), 0.0)
                nc.vector.memset(_t[:, :, 257:258].bitcast(f32), 0.0)
                nc.vector.memset(_t[:, 0:_topz, :].bitcast(f32), 0.0)
            nc.vector.memset(t5[:, :, 0:1], 0.0)
            nc.vector.memset(t5[:, :, 257:258], 0.0)
            nc.vector.memset(t5[:, 0:1, :], 0.0)
            # t5s garbage partitions are never gathered; selector rows are
            # zero there but 0*garbage must not be NaN -> zero once
            nc.vector.memset(t5s[:, :, :], 0.0)

            def conv_chunk(ps, w_sb, src, s, n):
                """5 accumulating matmuls; center is src[:, s:s+n, 1:1+W]."""
                nc.tensor.matmul(
                    ps, w_sb[:, 0, :], src[:, s : s + n, 1 : 1 + W],
                    start=True, stop=False,
                )
                nc.tensor.matmul(
                    ps, w_sb[:, 1, :], src[:, s - 1 : s - 1 + n, 1 : 1 + W],
                    start=False, stop=False,
                )
                nc.tensor.matmul(
                    ps, w_sb[:, 2, :], src[:, s + 1 : s + 1 + n, 1 : 1 + W],
                    start=False, stop=False,
                )
                nc.tensor.matmul(
                    ps, w_sb[:, 3, :], src[:, s : s + n, 0:W],
                    start=False, stop=False,
                )
                nc.tensor.matmul(
                    ps, w_sb[:, 4, :], src[:, s : s + n, 2 : 2 + W],
                    start=False, stop=True,
                )

            def l4b_chunks(a0, b0):
                """Deferred emitters: one K=128 selector matmul sums the 5
                pre-shifted slabs of t5s; bias on DVE; DMA out."""
                out = []
                rr = a0
                while rr < b0:
                    n = min(2, b0 - rr)

                    def emit(rr=rr, n=n, a0=a0):
                        d = rr - a0
                        ps = pmain.tile([OUT_C, n, W], f32, tag="ps")
                        nc.tensor.matmul(
                            ps, s6_sb[:, :], t5s[:, d : d + n, 1 : 1 + W],
                            start=True, stop=True,
                        )
                        yt = iopool.tile([OUT_C, n, W], f32, tag="yt")
                        nc.vector.tensor_scalar_add(yt, ps, b4_sb)
                        nc.scalar.dma_start(out=y_d[:, rr : rr + n, :], in_=yt)

                    out.append(emit)
                    rr += n
                return out

            pending = []  # tap-sum emitters from the previous strip
            for rep in range(repeat):
             for a in range(0, H, R):
                b = min(a + R, H)
                last = b == H
                lo_x, hi_x = max(0, a - 4), min(H, b + 4)

                if last:
                    # re-zero stale tail slots (bufs=1 reuse) before writes
                    nc.vector.memset(
                        x30[:, hi_x - a + 4 : R + 10, :].bitcast(f32), 0.0)
                    nc.vector.memset(
                        h1[:, 256 - (a - 3) : R + 6, :].bitcast(f32), 0.0)
                    nc.vector.memset(
                        h2[:, 256 - (a - 2) : R + 4, :].bitcast(f32), 0.0)
                    nc.vector.memset(
                        h3[:, 256 - (a - 1) : R + 2, :].bitcast(f32), 0.0)

                # --- load x strip, 5 shifted placements ---
                src = x_d[:, lo_x:hi_x, :].bitcast(f32r)
                o = lo_x - a
                nc.sync.dma_start(out=x30[0:6, o + 5 : hi_x - a + 5, 1 : 1 + W], in_=src)
                nc.sync.dma_start(out=x30[6:12, o + 6 : hi_x - a + 6, 1 : 1 + W], in_=src)
                nc.sync.dma_start(out=x30[12:18, o + 4 : hi_x - a + 4, 1 : 1 + W], in_=src)
                nc.sync.dma_start(out=x30[18:24, o + 5 : hi_x - a + 5, 2 : 2 + W], in_=src)
                nc.sync.dma_start(out=x30[24:30, o + 5 : hi_x - a + 5, 0:W], in_=src)

                # --- L1: rows [a-3, b+3) -> h1; copies alternate DVE/ACT ---
                rr = max(0, a - 3)
                hi = min(H, b + 3)
                ci = 0
                while rr < hi:
                    n = min(2, hi - rr)
                    s = rr - a + 5
                    ps = pmain.tile([HID_C, n, W], f32, tag="ps")
                    nc.tensor.matmul(
                        ps, w1_sb[:, :], x30[:, s : s + n, 1 : 1 + W],
                        start=True, stop=True,
                    )
                    d = rr - (a - 3)
                    if ci % 2 == 0:
                        nc.vector.tensor_scalar(
                            h1[:, d : d + n, 1 : 1 + W], ps, b1_sb, 0.0, Add, Max
                        )
                    else:
                        nc.scalar.activation(
                            h1[:, d : d + n, 1 : 1 + W], ps, Relu, bias=b1_sb
                        )
                    ci += 1
                    rr += n

                # --- L2: rows [a-2, b+2), reads h1; interleave prev tap-sum ---
                rr = max(0, a - 2) if stages >= 2 else hi
                hi = min(H, b + 2)
                while rr < hi:
                    n = min(2, hi - rr)
                    s = rr - a + 3  # h1 slot of center
                    ps = pmain.tile([HID_C, n, W], f32, tag="ps")
                    conv_chunk(ps, w2_sb, h1, s, n)
                    d = rr - (a - 2)
                    nc.scalar.activation(
                        h2[:, d : d + n, 1 : 1 + W], ps, Relu, bias=b2_sb
                    )
                    if pending and interleave:
                        pending.pop(0)()
                    rr += n
                while pending and interleave:
                    pending.pop(0)()

                # --- L3: rows [a-1, b+1), reads h2; mmA interleaved so
                # t5 fills (and the gather can start) as h3 rows land ---
                lo4, hi4 = max(0, a - 1), min(H, b + 1)
                mma_q = []
                rr = lo4
                ci4 = 0
                while rr < hi4:
                    n = min(2, hi4 - rr)
                    mma_q.append((rr, n))
                    rr += n

                def emit_mma(rr, n, ci):
                    s = rr - a + 1  # h3 slot of center
                    ps = pmain.tile([HID_C, n, W], f32, tag="ps")
                    nc.tensor.matmul(
                        ps, w4a_sb[:, :], h3[:, s : s + n, 1 : 1 + W],
                        start=True, stop=True,
                    )
                    d = rr - (a - 1)  # t5 slot
                    if ci % 2 == 0:
                        nc.vector.tensor_copy(t5[:, d : d + n, 1 : 1 + W], ps)
                    else:
                        nc.scalar.activation(
                            t5[:, d : d + n, 1 : 1 + W], ps, Ident)

                rr = max(0, a - 1) if stages >= 3 else H
                hi = min(H, b + 1)
                while rr < hi:
                    n = min(2, hi - rr)
                    s = rr - a + 2  # h2 slot of center
                    ps = pmain.tile([HID_C, n, W], f32, tag="ps")
                    conv_chunk(ps, w3_sb, h2, s, n)
                    d = rr - (a - 1)
                    nc.scalar.activation(
                        h3[:, d : d + n, 1 : 1 + W], ps, Relu, bias=b3_sb
                    )
                    if mma_q and mma_q[0][0] + 2 < rr:
                        r0, n0 = mma_q.pop(0)
                        emit_mma(r0, n0, ci4)
                        ci4 += 1
                    rr += n

                # --- L4a: drain remaining tap-partial chunks ---
                if stages < 4:
                    continue
                # tail slots beyond the written range must be zero (gathers
                # read them); t5 cells disjoint from mmA writes
                if hi4 - (a - 1) < R + 2:
                    nc.vector.memset(t5[:, hi4 - (a - 1) : R + 2, :], 0.0)
                while mma_q:
                    r0, n0 = mma_q.pop(0)
                    emit_mma(r0, n0, ci4)
                    ci4 += 1

                if stages < 5:
                    continue
                # --- gather: build pre-shifted t5s via SBUF->SBUF DMA ---
                # Full-padded-row flat copies: one contiguous run per
                # partition. t5 slot st = row-(a-1); t5s slot d = row-a.
                nr = b - a
                t5f = t5.rearrange("p r c -> p (r c)")
                t5sf = t5s.rearrange("p r c -> p (r c)")
                # two half-strip gathers: the first half fires before the
                # last tap-partial copies land, unblocking the first
                # deferred sum-matmuls earlier in the next strip
                for h0, h1r in ((0, nr // 2), (nr // 2, nr)):
                    o0, o1 = h0 * WP, h1r * WP
                    nc.scalar.dma_start(
                        out=t5sf[0:6, o0:o1], in_=t5f[0:6, o0:o1])
                    nc.scalar.dma_start(
                        out=t5sf[6:12, o0:o1], in_=t5f[6:12, WP + o0 : WP + o1])
                    nc.scalar.dma_start(
                        out=t5sf[32:38, o0:o1],
                        in_=t5f[32:38, 2 * WP + o0 : 2 * WP + o1])
                    nc.scalar.dma_start(
                        out=t5sf[64:70, o0 + 1 : o1],
                        in_=t5f[64:70, WP + o0 : WP + o1 - 1])
                    nc.scalar.dma_start(
                        out=t5sf[96:102, o0 : o1 - 1],
                        in_=t5f[96:102, WP + o0 + 1 : WP + o1])

                if stages < 6:
                    continue
                if not interleave:
                    while pending:
                        pending.pop(0)()
                pending = l4b_chunks(a, b)

            # flush the final strip's tap-sum
            while pending:
                pending.pop(0)()

    nc.finalize()
    return nc


_NC_CACHE = {}


def _pack_inputs(x, w1, b1, w2, b2, w3, b3, w4, b4):
    x = np.ascontiguousarray(np.asarray(x, dtype=np.float32))
    w1 = np.asarray(w1, dtype=np.float32)
    w2 = np.asarray(w2, dtype=np.float32)
    w3 = np.asarray(w3, dtype=np.float32)
    w4 = np.asarray(w4, dtype=np.float32)
    # w4a slabs: up@0-5, center@6-11, down@32-37, left@64-69, right@96-101
    w4a = np.zeros((HID_C, HID_C), np.float32)
    slabs = ((0, 1), (6, 2 - 2), (32, 2), (64, 3), (96, 4))
    w4a[:, 0:OUT_C] = w4[:, :, 1].T          # up
    w4a[:, 6 : 6 + OUT_C] = w4[:, :, 0].T    # center
    w4a[:, 32 : 32 + OUT_C] = w4[:, :, 2].T  # down
    w4a[:, 64 : 64 + OUT_C] = w4[:, :, 3].T  # left
    w4a[:, 96 : 96 + OUT_C] = w4[:, :, 4].T  # right
    s6 = np.zeros((HID_C, OUT_C), np.float32)
    for base in (0, 6, 32, 64, 96):
        s6[base + np.arange(OUT_C), np.arange(OUT_C)] = 1.0
    s6 = s6.astype(ml_dtypes.bfloat16)
    common = {
        # w1p[t*6+ic, oc] = w1[oc, ic, t]
        "w1p": np.ascontiguousarray(w1.transpose(2, 1, 0).reshape(5 * IN_C, HID_C)),
        # w2p[ic, t, oc] = w2[oc, ic, t]
        "w2p": np.ascontiguousarray(w2.transpose(1, 2, 0)),
        "w3p": np.ascontiguousarray(w3.transpose(1, 2, 0)),
        "w4a": w4a,
        "s6": s6,
        "b1": np.asarray(b1, np.float32),
        "b2": np.asarray(b2, np.float32),
        "b3": np.asarray(b3, np.float32),
        "b4": np.asarray(b4, np.float32),
    }
    return x, common


def kernel(x, w1, b1, w2, b2, w3, b3, w4, b4):
    x, common = _pack_inputs(x, w1, b1, w2, b2, w3, b3, w4, b4)
    if "nc" not in _NC_CACHE:
        _NC_CACHE["nc"] = _build()
    nc = _NC_CACHE["nc"]
    in_maps = [dict(common, x=x[i]) for i in range(N_CORES)]
    res = bass_utils.run_bass_kernel_spmd(nc, in_maps, core_ids=list(range(N_CORES)))
    out = np.stack([res.results[i]["y"] for i in range(N_CORES)], axis=0)
    return out



# revision 5
# speedup vs baseline: 1.0918x; 1.0918x over previous
"""Trainium2 Bass kernel for 4-layer cross-stencil CNN.

Per-core: one image [6,256,256] (batch dim sharded across 8 cores).
conv(cross-5-stencil) = 5 channel-matmuls with spatially shifted rhs APs,
accumulated in PSUM. Channels on partitions, spatial (rows x cols) on the
free dim. fp32r matmuls (full PE rate at N>=256).

Rolling-retention pipeline: intermediate buffers h1/h2/h3/t5 are circular
row-slot buffers (S=32 slots, slot = row % 32). Each layer computes every
image row exactly once — no halo recompute. Per strip of R=24 rows the
layer blocks are skewed (L1 rows [a,a+24), L2 [a-2,a+22), L3 [a-4,a+20),
mmA [a-5,a+19), L4b [a-7,a+17)) so all reads hit rows already produced
this strip or retained from the previous one. Reads that cross the
circular wrap are split into <=2 accumulating matmuls; chunk boundaries
are chosen so writes never wrap. Image-boundary taps are handled by
narrowing the tap matmul to valid rows (no zero-pad rows needed).

L1 packs the 5 taps into K=30 via a 5-group pre-shifted x staging buffer
(one matmul per chunk), double-buffered across strips. L4 computes all 5
taps as one M=128 matmul (slabs at partitions 0/6/32/64/96), gathers the
shifted slabs into t5s by SBUF->SBUF DMA, and sums them with one K=128
selector matmul; the strip's tap-sum drains interleaved into the next
strip's L2 phase.
"""

import sys

sys.path.insert(0, "/opt/trn_rl_repo")

import ml_dtypes
import numpy as np

import concourse.bacc as bacc
import concourse.mybir as mybir
from concourse.tile import TileContext
from concourse import bass_utils

IN_C, HID_C, OUT_C = 6, 128, 6
B, H, W = 8, 256, 256
WP = W + 2  # padded width
R = 24  # rows per strip
S = 32  # circular row slots in h1/h2/h3/t5
XS = 28  # row slots in the x staging buffers
N_CORES = 8

f32 = mybir.dt.float32
f32r = mybir.dt.float32r
bf16 = mybir.dt.bfloat16
Add = mybir.AluOpType.add
Max = mybir.AluOpType.max
Relu = mybir.ActivationFunctionType.Relu
Ident = mybir.ActivationFunctionType.Identity

# tap order matches reference: 0=center, 1=up(x[h-1]), 2=down(x[h+1]),
# 3=left(x[w-1]), 4=right(x[w+1])


def _chunks(r0, r1):
    """2-row chunks of [r0,r1), never straddling a multiple of S."""
    r = r0
    while r < r1:
        n = min(2, r1 - r, (r // S + 1) * S - r)
        yield r, n
        r += n


def _runs(r0, r1):
    """Split [r0,r1) into runs of consecutive slots (break at S multiples)."""
    r = r0
    while r < r1:
        e = min(r1, (r // S + 1) * S)
        yield r, e
        r = e


def _build():
    nc = bacc.Bacc("TRN2", target_bir_lowering=False)

    x_d = nc.dram_tensor("x", [IN_C, H, W], f32, kind="ExternalInput")
    w1_d = nc.dram_tensor("w1p", [5 * IN_C, HID_C], f32, kind="ExternalInput")
    w2_d = nc.dram_tensor("w2p", [HID_C, 5, HID_C], f32, kind="ExternalInput")
    w3_d = nc.dram_tensor("w3p", [HID_C, 5, HID_C], f32, kind="ExternalInput")
    # w4a: all 5 taps as M=128 slabs: up@0-5, center@6-11, down@32-37,
    # left@64-69, right@96-101; zero elsewhere
    w4a_d = nc.dram_tensor("w4a", [HID_C, HID_C], f32, kind="ExternalInput")
    # s6: bf16 selector summing the 5 (pre-shifted) slabs of t5s
    s6_d = nc.dram_tensor("s6", [HID_C, OUT_C], bf16, kind="ExternalInput")
    b1_d = nc.dram_tensor("b1", [HID_C], f32, kind="ExternalInput")
    b2_d = nc.dram_tensor("b2", [HID_C], f32, kind="ExternalInput")
    b3_d = nc.dram_tensor("b3", [HID_C], f32, kind="ExternalInput")
    b4_d = nc.dram_tensor("b4", [OUT_C], f32, kind="ExternalInput")
    y_d = nc.dram_tensor("y", [OUT_C, H, W], f32, kind="ExternalOutput")

    strips = list(range(0, H, R))  # block anchors a = 24k

    with TileContext(nc) as tc:
        with (
            tc.tile_pool(name="const", bufs=1) as cpool,
            tc.tile_pool(name="bufs", bufs=1) as bpool,
            tc.tile_pool(name="io", bufs=4) as iopool,
            tc.tile_pool(name="psmain", bufs=7, space="PSUM") as pmain,
        ):
            # --- weights / biases (resident); w1 + strip-0 x first so L1
            # can start ~1.5us in, the rest split across two DMA queues ---
            w1_sb = cpool.tile([5 * IN_C, HID_C], f32r)
            nc.sync.dma_start(out=w1_sb, in_=w1_d[:, :].bitcast(f32r))

            # --- persistent buffers ---
            # xst: 2 staging buffers, 5 tap-groups x 6ch, pre-shifted by DMA
            # placement. Strip k window: x rows [24k-2, 24k+26) at slot
            # r - 24k + 2 (center group); reading slot(r) in group g yields
            # x row r + dr_g at col w + dc_g.
            xst = [
                bpool.tile([5 * IN_C, XS, WP], f32r, name=f"xst{i}")
                for i in range(2)
            ]
            h1 = bpool.tile([HID_C, S, WP], f32r)
            h2 = bpool.tile([HID_C, S, WP], f32r)
            h3 = bpool.tile([HID_C, S, WP], f32r)
            # t5: tap partials (slabs up@0,cen@6,dn@32,lf@64,rt@96), bf16
            t5 = bpool.tile([HID_C, S, WP], bf16)
            # t5s: gathered pre-shifted taps; slot d = output row a5+d
            t5s = bpool.tile([HID_C, R, WP], bf16)

            # x staging pads: col 1 zero for the left group (parts 18:24,
            # placed at cols [2,258)), col 256 zero for the right group
            # (parts 24:30, cols [0,256)); memset all parts once, DMAs of
            # the other groups overwrite their copies each strip.
            for xb in xst:
                nc.vector.memset(xb[:, :, 1:2].bitcast(f32), 0.0)
                nc.vector.memset(xb[:, :, 256:257].bitcast(f32), 0.0)
                # strip-0 rows < 0 (slots 0..2 cover x rows -3..-1 in the
                # most-shifted group); also col 0 / 257 never DMA'd
                nc.vector.memset(xb[:, 0:3, :].bitcast(f32), 0.0)
                nc.vector.memset(xb[:, :, 0:1].bitcast(f32), 0.0)
                nc.vector.memset(xb[:, :, 257:258].bitcast(f32), 0.0)

            def x_dma(a, buf):
                """Load x rows [a-1, a+R+1) into staging buffer (5 shifted
                placements); center group slot = r - a + 2."""
                lo, hi = max(0, a - 1), min(H, a + R + 1)
                src = x_d[:, lo:hi, :].bitcast(f32r)
                o = lo - a  # >= -1
                # (partitions, slot offset of x row m = m - a + ofs, cols)
                for p0, ofs, c0 in (
                    (0, 2, 1),   # center
                    (6, 3, 1),   # up: read slot(r) -> x row r-1
                    (12, 1, 1),  # down
                    (18, 2, 2),  # left: read col w+1 -> x col w-1
                    (24, 2, 0),  # right
                ):
                    nc.sync.dma_start(
                        out=xst[buf][p0 : p0 + 6, o + ofs : hi - a + ofs, c0 : c0 + W],
                        in_=src,
                    )

            x_dma(0, 0)

            w2_sb = cpool.tile([HID_C, 5, HID_C], f32r)
            nc.sync.dma_start(out=w2_sb, in_=w2_d[:, :, :].bitcast(f32r))
            b2_sb = cpool.tile([HID_C, 1], f32)
            nc.sync.dma_start(out=b2_sb, in_=b2_d[:, None])
            w3_sb = cpool.tile([HID_C, 5, HID_C], f32r)
            nc.scalar.dma_start(out=w3_sb, in_=w3_d[:, :, :].bitcast(f32r))
            b3_sb = cpool.tile([HID_C, 1], f32)
            nc.scalar.dma_start(out=b3_sb, in_=b3_d[:, None])
            w4a_sb = cpool.tile([HID_C, HID_C], f32r)
            nc.scalar.dma_start(out=w4a_sb, in_=w4a_d[:, :].bitcast(f32r))
            s6_sb = cpool.tile([HID_C, OUT_C], bf16)
            nc.scalar.dma_start(out=s6_sb, in_=s6_d[:, :])
            b4_sb = cpool.tile([OUT_C, 1], f32)
            nc.scalar.dma_start(out=b4_sb, in_=b4_d[:, None])

            # h/t5 pads: cols 0 and 257 are read (left/right taps) but
            # never written; zero once — rolling writes only touch [1,257).
            for _t in (h1, h2, h3):
                nc.vector.memset(_t[:, :, 0:1].bitcast(f32), 0.0)
                nc.vector.memset(_t[:, :, 257:258].bitcast(f32), 0.0)
            nc.vector.memset(t5[:, :, 0:1], 0.0)
            nc.vector.memset(t5[:, :, 257:258], 0.0)
            # t5s non-slab partitions are read by the selector matmul
            # (0 * garbage must not be NaN) — zero everything once
            nc.vector.memset(t5s[:, :, :], 0.0)

            def conv_chunk(ps, w_sb, src, r0, n):
                """Cross-stencil accumulation for output rows [r0, r0+n).
                Center tap first (covers the whole psum tile -> start=True
                zeroes it); up/down taps clamp to [0,H) and split at the
                circular wrap."""
                parts = []  # (tap, psum_off, rows, slot)
                for tap, dr in ((1, -1), (2, 1), (3, 0), (4, 0)):
                    lo = max(0, r0 + dr)
                    hi = min(H, r0 + n + dr)
                    for ra, rb in _runs(lo, hi):
                        parts.append((tap, ra - dr - r0, rb - ra, ra % S))
                cols = {0: (1, 1 + W), 1: (1, 1 + W), 2: (1, 1 + W),
                        3: (0, W), 4: (2, 2 + W)}
                nc.tensor.matmul(
                    ps, w_sb[:, 0, :], src[:, r0 % S : r0 % S + n, 1 : 1 + W],
                    start=True, stop=False,
                )
                for i, (tap, off, m, sl) in enumerate(parts):
                    c0, c1 = cols[tap]
                    nc.tensor.matmul(
                        ps[:, off : off + m, :],
                        w_sb[:, tap, :],
                        src[:, sl : sl + m, c0:c1],
                        start=False, stop=(i == len(parts) - 1),
                    )

            def l4b_chunks(a5, b5):
                """Deferred emitters: one K=128 selector matmul sums the 5
                pre-shifted slabs of t5s; bias on DVE; DMA out."""
                out = []
                for rr, n in list(_chunks(a5, b5)):

                    def emit(rr=rr, n=n, a5=a5):
                        d = rr - a5
                        ps = pmain.tile([OUT_C, n, W], f32, tag="ps")
                        nc.tensor.matmul(
                            ps, s6_sb[:, :], t5s[:, d : d + n, 1 : 1 + W],
                            start=True, stop=True,
                        )
                        yt = iopool.tile([OUT_C, n, W], f32, tag="yt")
                        nc.vector.tensor_scalar_add(yt, ps, b4_sb)
                        nc.scalar.dma_start(out=y_d[:, rr : rr + n, :], in_=yt)

                    out.append(emit)
                return out

            pending = []  # tap-sum emitters from the previous strip
            for k, a in enumerate(strips):
                last = k == len(strips) - 1
                # skewed layer blocks for this strip
                a1, b1r = a, min(H, a + R)
                a2, b2r = max(0, a - 2), min(H, a + R - 2)
                a3, b3r = max(0, a - 4), min(H, a + R - 4)
                a4, b4r = max(0, a - 5), min(H, a + R - 5)
                a5, b5r = max(0, a - 7), min(H, a + R - 7)
                if last:
                    b2r = b3r = b4r = b5r = H

                # prefetch next strip's x into the other staging buffer
                if not last:
                    a_n = a + R
                    if a_n + R >= H:
                        # final strip: slots past the image tail keep stale
                        # data from two strips ago — zero them before the DMA
                        tail0 = min(H, a_n + R + 1) - a_n + 1
                        nc.vector.memset(
                            xst[(k + 1) % 2][:, tail0:XS, :].bitcast(f32), 0.0)
                    x_dma(a_n, (k + 1) % 2)

                # --- L1: one K=30 matmul per chunk; copies alternate
                # DVE/ACT ---
                xb = xst[k % 2]
                ci = 0
                for rr, n in _chunks(a1, b1r):
                    s0 = rr - a + 2
                    ps = pmain.tile([HID_C, n, W], f32, tag="ps")
                    nc.tensor.matmul(
                        ps, w1_sb[:, :], xb[:, s0 : s0 + n, 1 : 1 + W],
                        start=True, stop=True,
                    )
                    sl = rr % S
                    if ci % 2 == 0:
                        nc.vector.tensor_scalar(
                            h1[:, sl : sl + n, 1 : 1 + W], ps, b1_sb, 0.0, Add, Max
                        )
                    else:
                        nc.scalar.activation(
                            h1[:, sl : sl + n, 1 : 1 + W], ps, Relu, bias=b1_sb
                        )
                    ci += 1

                # --- L2: reads h1; interleave prev strip's tap-sum ---
                for rr, n in _chunks(a2, b2r):
                    ps = pmain.tile([HID_C, n, W], f32, tag="ps")
                    conv_chunk(ps, w2_sb, h1, rr, n)
                    sl = rr % S
                    nc.scalar.activation(
                        h2[:, sl : sl + n, 1 : 1 + W], ps, Relu, bias=b2_sb
                    )
                    if pending:
                        pending.pop(0)()
                while pending:
                    pending.pop(0)()

                # --- L3: reads h2; mmA interleaved so t5 fills (and the
                # gather can start) as h3 rows land ---
                mma_q = list(_chunks(a4, b4r))

                def emit_mma(rr, n, ci):
                    ps = pmain.tile([HID_C, n, W], f32, tag="ps")
                    sl = rr % S
                    nc.tensor.matmul(
                        ps, w4a_sb[:, :], h3[:, sl : sl + n, 1 : 1 + W],
                        start=True, stop=True,
                    )
                    if ci % 2 == 0:
                        nc.vector.tensor_copy(t5[:, sl : sl + n, 1 : 1 + W], ps)
                    else:
                        nc.scalar.activation(
                            t5[:, sl : sl + n, 1 : 1 + W], ps, Ident)

                ci4 = 0
                for rr, n in _chunks(a3, b3r):
                    ps = pmain.tile([HID_C, n, W], f32, tag="ps")
                    conv_chunk(ps, w3_sb, h2, rr, n)
                    sl = rr % S
                    nc.scalar.activation(
                        h3[:, sl : sl + n, 1 : 1 + W], ps, Relu, bias=b3_sb
                    )
                    rr_next = rr + n
                    while mma_q and mma_q[0][0] + mma_q[0][1] + 2 <= rr_next:
                        m0, n0 = mma_q.pop(0)
                        emit_mma(m0, n0, ci4)
                        ci4 += 1
                while mma_q:
                    m0, n0 = mma_q.pop(0)
                    emit_mma(m0, n0, ci4)
                    ci4 += 1

                # --- gather: build pre-shifted t5s via SBUF->SBUF DMA ---
                # t5s slot d = output row a5+d; two half-strip batches so
                # the first deferred sum-matmuls unblock earlier next strip.
                # slab parts / tap row-offset / col offsets (dst, src):
                gat = ((0, -1, 1, 1), (6, 0, 1, 1), (32, 1, 1, 1),
                       (64, 0, 2, 1), (96, 0, 1, 2))
                if k == 0:
                    # row 0 up-slab source is row -1: zero that dest
                    nc.vector.memset(t5s[0:6, 0:1, :], 0.0)
                if last:
                    # row 255 down-slab source is row 256
                    nc.vector.memset(t5s[32:38, b5r - 1 - a5 : b5r - a5, :], 0.0)
                nr = b5r - a5
                for d0, d1 in ((0, nr // 2), (nr // 2, nr)):
                    for p0, dr, cd, cs in gat:
                        lo = max(0, a5 + d0 + dr)
                        hi = min(H, a5 + d1 + dr)
                        for ra, rb in _runs(lo, hi):
                            dd = ra - dr - a5
                            m = rb - ra
                            nc.scalar.dma_start(
                                out=t5s[p0 : p0 + 6, dd : dd + m, cd : cd + W],
                                in_=t5[p0 : p0 + 6, ra % S : ra % S + m, cs : cs + W],
                            )

                pending = l4b_chunks(a5, b5r)

            # flush the final strip's tap-sum
            while pending:
                pending.pop(0)()

    nc.finalize()
    return nc


_NC_CACHE = {}


def _pack_inputs(x, w1, b1, w2, b2, w3, b3, w4, b4):
    x = np.ascontiguousarray(np.asarray(x, dtype=np.float32))
    w1 = np.asarray(w1, dtype=np.float32)
    w2 = np.asarray(w2, dtype=np.float32)
    w3 = np.asarray(w3, dtype=np.float32)
    w4 = np.asarray(w4, dtype=np.float32)
    # w4a slabs: up@0-5, center@6-11, down@32-37, left@64-69, right@96-101
    w4a = np.zeros((HID_C, HID_C), np.float32)
    w4a[:, 0:OUT_C] = w4[:, :, 1].T          # up
    w4a[:, 6 : 6 + OUT_C] = w4[:, :, 0].T    # center
    w4a[:, 32 : 32 + OUT_C] = w4[:, :, 2].T  # down
    w4a[:, 64 : 64 + OUT_C] = w4[:, :, 3].T  # left
    w4a[:, 96 : 96 + OUT_C] = w4[:, :, 4].T  # right
    s6 = np.zeros((HID_C, OUT_C), np.float32)
    for base in (0, 6, 32, 64, 96):
        s6[base + np.arange(OUT_C), np.arange(OUT_C)] = 1.0
    s6 = s6.astype(ml_dtypes.bfloat16)
    common = {
        # w1p[t*6+ic, oc] = w1[oc, ic, t]
        "w1p": np.ascontiguousarray(w1.transpose(2, 1, 0).reshape(5 * IN_C, HID_C)),
        # w2p[ic, t, oc] = w2[oc, ic, t]
        "w2p": np.ascontiguousarray(w2.transpose(1, 2, 0)),
        "w3p": np.ascontiguousarray(w3.transpose(1, 2, 0)),
        "w4a": w4a,
        "s6": s6,
        "b1": np.asarray(b1, np.float32),
        "b2": np.asarray(b2, np.float32),
        "b3": np.asarray(b3, np.float32),
        "b4": np.asarray(b4, np.float32),
    }
    return x, common


def kernel(x, w1, b1, w2, b2, w3, b3, w4, b4):
    x, common = _pack_inputs(x, w1, b1, w2, b2, w3, b3, w4, b4)
    if "nc" not in _NC_CACHE:
        _NC_CACHE["nc"] = _build()
    nc = _NC_CACHE["nc"]
    in_maps = [dict(common, x=x[i]) for i in range(N_CORES)]
    res = bass_utils.run_bass_kernel_spmd(nc, in_maps, core_ids=list(range(N_CORES)))
    out = np.stack([res.results[i]["y"] for i in range(N_CORES)], axis=0)
    return out


# revision 25
# speedup vs baseline: 1.2133x; 1.1113x over previous
"""Trainium2 Bass kernel for 4-layer cross-stencil CNN.

Per-core: one image [6,256,256] (batch dim sharded across 8 cores).
conv(cross-5-stencil) = 5 channel-matmuls with spatially shifted rhs APs,
accumulated in PSUM. Channels on partitions, spatial (rows x cols) on the
free dim. fp32r matmuls (full PE rate at N>=256).

Rolling-retention pipeline: intermediate buffers h1/h2/h3/t5 are circular
row-slot buffers (S=32 slots, slot = row % 32). Each layer computes every
image row exactly once — no halo recompute. Per strip of R=24 rows the
layer blocks are skewed (L1 rows [a,a+24), L2 [a-2,a+22), L3 [a-4,a+20),
mmA [a-5,a+19), L4b [a-7,a+17)) so all reads hit rows already produced
this strip or retained from the previous one. Reads that cross the
circular wrap are split into <=2 accumulating matmuls; chunk boundaries
are chosen so writes never wrap. Image-boundary taps are handled by
narrowing the tap matmul to valid rows (no zero-pad rows needed).

L1 packs the 5 taps into K=30 via a 5-group pre-shifted x staging buffer
(one matmul per chunk), double-buffered across strips. L4 computes all 5
taps as one M=128 matmul (slabs at partitions 0/6/32/64/96), gathers the
shifted slabs into t5s by SBUF->SBUF DMA, and sums them with one K=128
selector matmul; the strip's tap-sum drains interleaved into the next
strip's L2 phase.
"""

import sys

sys.path.insert(0, "/opt/trn_rl_repo")

import ml_dtypes
import numpy as np

import concourse.bacc as bacc
import concourse.mybir as mybir
from concourse.tile import TileContext
from concourse import bass_utils

IN_C, HID_C, OUT_C = 6, 128, 6
B, H, W = 8, 256, 256
WP = W + 2  # padded width
R = 24  # rows per strip
S = 28  # circular row slots in h1/h2/h3/t5 (live windows are <= 27 rows)
XS = 28  # row slots in the x staging buffers
N_CORES = 8

f32 = mybir.dt.float32
f32r = mybir.dt.float32r
bf16 = mybir.dt.bfloat16
Add = mybir.AluOpType.add
Max = mybir.AluOpType.max
Relu = mybir.ActivationFunctionType.Relu
Ident = mybir.ActivationFunctionType.Identity

# tap order matches reference: 0=center, 1=up(x[h-1]), 2=down(x[h+1]),
# 3=left(x[w-1]), 4=right(x[w+1])


def _chunks(r0, r1):
    """2-row chunks of [r0,r1), never straddling a multiple of S."""
    r = r0
    while r < r1:
        n = min(2, r1 - r, (r // S + 1) * S - r)
        yield r, n
        r += n


def _runs(r0, r1):
    """Split [r0,r1) into runs of consecutive slots (break at S multiples)."""
    r = r0
    while r < r1:
        e = min(r1, (r // S + 1) * S)
        yield r, e
        r = e


def _build():
    nc = bacc.Bacc("TRN2", target_bir_lowering=False)

    x_d = nc.dram_tensor("x", [IN_C, H, W], f32, kind="ExternalInput")
    w1_d = nc.dram_tensor("w1p", [5 * IN_C, HID_C], f32, kind="ExternalInput")
    w2_d = nc.dram_tensor("w2p", [HID_C, 5, HID_C], f32, kind="ExternalInput")
    w3_d = nc.dram_tensor("w3p", [HID_C, 5, HID_C], f32, kind="ExternalInput")
    # w4a: all 5 taps as M=128 slabs: up@0-5, center@6-11, down@32-37,
    # left@64-69, right@96-101; zero elsewhere
    w4a_d = nc.dram_tensor("w4a", [HID_C, HID_C], f32, kind="ExternalInput")
    # s6: bf16 selector summing the 5 (pre-shifted) slabs of t5s
    s6_d = nc.dram_tensor("s6", [HID_C, OUT_C], bf16, kind="ExternalInput")
    b1_d = nc.dram_tensor("b1", [HID_C], f32, kind="ExternalInput")
    b2_d = nc.dram_tensor("b2", [HID_C], f32, kind="ExternalInput")
    b3_d = nc.dram_tensor("b3", [HID_C], f32, kind="ExternalInput")
    b4_d = nc.dram_tensor("b4", [OUT_C], f32, kind="ExternalInput")
    y_d = nc.dram_tensor("y", [OUT_C, H, W], f32, kind="ExternalOutput")

    strips = list(range(0, H, R))  # block anchors a = 24k

    with TileContext(nc) as tc:
        with (
            tc.tile_pool(name="const", bufs=1) as cpool,
            tc.tile_pool(name="bufs", bufs=1) as bpool,
            tc.tile_pool(name="io", bufs=2) as iopool,
            tc.tile_pool(name="psmain", bufs=7, space="PSUM") as pmain,
        ):
            # --- weights / biases (resident); w1 + strip-0 x first so L1
            # can start ~1.5us in, the rest split across two DMA queues ---
            w1_sb = cpool.tile([5 * IN_C, HID_C], f32r)
            nc.sync.dma_start(out=w1_sb, in_=w1_d[:, :].bitcast(f32r))

            # --- persistent buffers ---
            # xst: 2 staging buffers, 5 tap-groups x 6ch, pre-shifted by DMA
            # placement. Strip k window: x rows [24k-2, 24k+26) at slot
            # r - 24k + 2 (center group); reading slot(r) in group g yields
            # x row r + dr_g at col w + dc_g.
            xst = [
                bpool.tile([5 * IN_C, XS, WP], f32r, name=f"xst{i}")
                for i in range(2)
            ]
            h1 = bpool.tile([HID_C, S, WP], f32r)
            h2 = bpool.tile([HID_C, S, WP], f32r)
            h3 = bpool.tile([HID_C, S, WP], f32r)
            # t5: tap partials (slabs up@0,cen@6,dn@32,lf@64,rt@96), bf16
            t5 = bpool.tile([HID_C, S, WP], bf16)
            # t5s: gathered pre-shifted taps; slot d = output row a5+d
            t5s = bpool.tile([HID_C, R, WP], bf16)

            # x staging pads: col 1 zero for the left group (parts 18:24,
            # placed at cols [2,258)), col 256 zero for the right group
            # (parts 24:30, cols [0,256)); memset all parts once, DMAs of
            # the other groups overwrite their copies each strip.
            for xb in xst:
                nc.gpsimd.memset(xb[:, :, 1:2].bitcast(f32), 0.0)
                nc.gpsimd.memset(xb[:, :, 256:257].bitcast(f32), 0.0)
                # strip-0 rows < 0 (slots 0..2 cover x rows -3..-1 in the
                # most-shifted group); also col 0 / 257 never DMA'd
                nc.gpsimd.memset(xb[:, 0:3, :].bitcast(f32), 0.0)
                nc.gpsimd.memset(xb[:, :, 0:1].bitcast(f32), 0.0)
                nc.gpsimd.memset(xb[:, :, 257:258].bitcast(f32), 0.0)

            def x_dma(a, buf):
                """Load x rows [a-1, a+R+1) into staging buffer (5 shifted
                placements); center group slot = r - a + 2."""
                lo, hi = max(0, a - 1), min(H, a + R + 1)
                src = x_d[:, lo:hi, :].bitcast(f32r)
                o = lo - a  # >= -1
                # (partitions, slot offset of x row m = m - a + ofs, cols)
                for p0, ofs, c0 in (
                    (0, 2, 1),   # center
                    (6, 3, 1),   # up: read slot(r) -> x row r-1
                    (12, 1, 1),  # down
                    (18, 2, 2),  # left: read col w+1 -> x col w-1
                    (24, 2, 0),  # right
                ):
                    nc.sync.dma_start(
                        out=xst[buf][p0 : p0 + 6, o + ofs : hi - a + ofs, c0 : c0 + W],
                        in_=src,
                    )

            x_dma(0, 0)

            b1_sb = cpool.tile([HID_C, 1], f32)
            nc.sync.dma_start(out=b1_sb, in_=b1_d[:, None])
            w2_sb = cpool.tile([HID_C, 5, HID_C], f32r)
            nc.sync.dma_start(out=w2_sb, in_=w2_d[:, :, :].bitcast(f32r))
            b2_sb = cpool.tile([HID_C, 1], f32)
            nc.sync.dma_start(out=b2_sb, in_=b2_d[:, None])
            w3_sb = cpool.tile([HID_C, 5, HID_C], f32r)
            nc.sync.dma_start(out=w3_sb, in_=w3_d[:, :, :].bitcast(f32r))
            b3_sb = cpool.tile([HID_C, 1], f32)
            nc.sync.dma_start(out=b3_sb, in_=b3_d[:, None])
            w4a_sb = cpool.tile([HID_C, HID_C], f32r)
            nc.sync.dma_start(out=w4a_sb, in_=w4a_d[:, :].bitcast(f32r))
            s6_sb = cpool.tile([HID_C, OUT_C], bf16)
            nc.sync.dma_start(out=s6_sb, in_=s6_d[:, :])
            b4_sb = cpool.tile([OUT_C, 1], f32)
            nc.sync.dma_start(out=b4_sb, in_=b4_d[:, None])

            # h/t5 pads: cols 0 and 257 are read (left/right taps) but
            # never written; zero once — rolling writes only touch [1,257).
            for _t in (h1, h2, h3):
                nc.gpsimd.memset(_t[:, :, 0:1].bitcast(f32), 0.0)
                nc.gpsimd.memset(_t[:, :, 257:258].bitcast(f32), 0.0)
            nc.gpsimd.memset(t5[:, :, 0:1], 0.0)
            nc.gpsimd.memset(t5[:, :, 257:258], 0.0)
            # t5s non-slab partitions are read by the selector matmul
            # (0 * garbage must not be NaN) — zero everything once
            nc.gpsimd.memset(t5s[:, :, :], 0.0)

            def conv_chunk(ps, w_sb, src, r0, n):
                """Cross-stencil accumulation for output rows [r0, r0+n).
                Center tap first (covers the whole psum tile -> start=True
                zeroes it); up/down taps clamp to [0,H) and split at the
                circular wrap."""
                parts = []  # (tap, psum_off, rows, slot)
                for tap, dr in ((1, -1), (2, 1), (3, 0), (4, 0)):
                    lo = max(0, r0 + dr)
                    hi = min(H, r0 + n + dr)
                    for ra, rb in _runs(lo, hi):
                        parts.append((tap, ra - dr - r0, rb - ra, ra % S))
                cols = {0: (1, 1 + W), 1: (1, 1 + W), 2: (1, 1 + W),
                        3: (0, W), 4: (2, 2 + W)}
                nc.tensor.matmul(
                    ps, w_sb[:, 0, :], src[:, r0 % S : r0 % S + n, 1 : 1 + W],
                    start=True, stop=False,
                )
                for i, (tap, off, m, sl) in enumerate(parts):
                    c0, c1 = cols[tap]
                    nc.tensor.matmul(
                        ps[:, off : off + m, :],
                        w_sb[:, tap, :],
                        src[:, sl : sl + m, c0:c1],
                        start=False, stop=(i == len(parts) - 1),
                    )

            # slab parts / tap row-offset / col offsets (dst, src)
            GAT = ((0, -1, 1, 1), (6, 0, 1, 1), (32, 1, 1, 1),
                   (64, 0, 2, 1), (96, 0, 1, 2))

            def emit_gather(a5, d0, d1, split=False):
                """Gather t5 slab rows into pre-shifted t5s slots [d0,d1).
                With split=True alternate slabs between the HWDGE (sync) and
                SWDGE (gpsimd) paths so the two DMA front-ends overlap."""
                for gi, (p0, dr, cd, cs) in enumerate(GAT):
                    eng = nc.gpsimd if split and gi % 2 else nc.sync
                    lo = max(0, a5 + d0 + dr)
                    hi = min(H, a5 + d1 + dr)
                    for ra, rb in _runs(lo, hi):
                        dd = ra - dr - a5
                        m = rb - ra
                        eng.dma_start(
                            out=t5s[p0 : p0 + 6, dd : dd + m, cd : cd + W],
                            in_=t5[p0 : p0 + 6, ra % S : ra % S + m, cs : cs + W],
                        )

            def l4b_chunks(a5, b5):
                """Deferred emitters: one K=128 selector matmul per chunk
                sums the 5 pre-shifted slabs of t5s; bias on DVE into a
                strip-wide staging tile; two half-strip DMAs out."""
                nr = b5 - a5
                chunks = list(_chunks(a5, b5))
                # split into flush groups, each with its own staging tile
                nparts = 4 if last else 2
                halves = []
                prev_i = 0
                for j in range(1, nparts + 1):
                    tgt = nr * j // nparts
                    i = next((i for i, (rr, n) in enumerate(chunks)
                              if rr + n - a5 >= tgt), len(chunks) - 1)
                    if i + 1 > prev_i:
                        halves.append(chunks[prev_i : i + 1])
                        prev_i = i + 1
                out = []  # (needed t5s rows, emitter) pairs
                for half in halves:
                    if not half:
                        continue
                    lo = half[0][0] - a5
                    hi = half[-1][0] + half[-1][1] - a5
                    yst = iopool.tile([OUT_C, hi - lo, W], f32, tag="yst")
                    for rr, n in half:

                        def emit(rr=rr, n=n, a5=a5, yst=yst, lo=lo):
                            d = rr - a5
                            ps = pmain.tile([OUT_C, n, W], f32, tag="ps")
                            nc.tensor.matmul(
                                ps, s6_sb[:, :], t5s[:, d : d + n, 1 : 1 + W],
                                start=True, stop=True,
                            )
                            nc.vector.tensor_scalar_add(
                                yst[:, d - lo : d - lo + n, :], ps, b4_sb)

                        out.append((rr + n - a5, emit))

                    def flush(lo=lo, hi=hi, a5=a5, yst=yst):
                        nc.sync.dma_start(
                            out=y_d[:, a5 + lo : a5 + hi, :],
                            in_=yst[:, :, :])

                    out.append((hi, flush))
                return out

            pending = []  # tap-sum emitters from the previous strip
            mma_carry = []  # end-of-strip mmA chunks deferred into next L1
            gather_pend = None  # deferred gather + l4b creation
            for k, a in enumerate(strips):
                last = k == len(strips) - 1
                # skewed layer blocks for this strip
                a1, b1r = a, min(H, a + R)
                a2, b2r = max(0, a - 2), min(H, a + R - 2)
                a3, b3r = max(0, a - 4), min(H, a + R - 4)
                a4, b4r = max(0, a - 5), min(H, a + R - 5)
                a5, b5r = max(0, a - 7), min(H, a + R - 7)
                if last:
                    b2r = b3r = b4r = b5r = H

                # prefetch next strip's x into the other staging buffer
                if not last:
                    a_n = a + R
                    if a_n + R >= H:
                        # final strip: slots past the image tail keep stale
                        # data from two strips ago — zero them before the DMA
                        tail0 = min(H, a_n + R + 1) - a_n + 1
                        nc.gpsimd.memset(
                            xst[(k + 1) % 2][:, tail0:XS, :].bitcast(f32), 0.0)
                    x_dma(a_n, (k + 1) % 2)

                # --- L1: one K=30 matmul per chunk; copies alternate
                # DVE/ACT ---
                xb = xst[k % 2]
                ci = 0
                for rr, n in _chunks(a1, b1r):
                    s0 = rr - a + 2
                    ps = pmain.tile([HID_C, n, W], f32, tag="ps")
                    nc.tensor.matmul(
                        ps, w1_sb[:, :], xb[:, s0 : s0 + n, 1 : 1 + W],
                        start=True, stop=True,
                    )
                    sl = rr % S
                    if ci % 2 == 0:
                        nc.vector.tensor_scalar(
                            h1[:, sl : sl + n, 1 : 1 + W], ps, b1_sb, 0.0, Add, Max
                        )
                    else:
                        nc.scalar.activation(
                            h1[:, sl : sl + n, 1 : 1 + W], ps, Relu, bias=b1_sb
                        )
                    ci += 1
                    if mma_carry:
                        mma_carry.pop(0)()
                while mma_carry:
                    mma_carry.pop(0)()
                if gather_pend is not None:
                    pending = gather_pend()
                    gather_pend = None

                # --- L2: reads h1; interleave prev strip's tap-sum ---
                for rr, n in _chunks(a2, b2r):
                    ps = pmain.tile([HID_C, n, W], f32, tag="ps")
                    conv_chunk(ps, w2_sb, h1, rr, n)
                    sl = rr % S
                    nc.scalar.activation(
                        h2[:, sl : sl + n, 1 : 1 + W], ps, Relu, bias=b2_sb
                    )
                    if pending:
                        pending.pop(0)[1]()
                while pending:
                    pending.pop(0)[1]()

                # --- L3: reads h2; mmA interleaved so t5 fills (and the
                # gather can start) as h3 rows land ---
                mma_q = list(_chunks(a4, b4r))

                def emit_mma(rr, n, ci):
                    ps = pmain.tile([HID_C, n, W], f32, tag="ps")
                    sl = rr % S
                    nc.tensor.matmul(
                        ps, w4a_sb[:, :], h3[:, sl : sl + n, 1 : 1 + W],
                        start=True, stop=True,
                    )
                    if ci % 2 == 0:
                        nc.vector.tensor_copy(t5[:, sl : sl + n, 1 : 1 + W], ps)
                    else:
                        nc.scalar.activation(
                            t5[:, sl : sl + n, 1 : 1 + W], ps, Ident)

                if k == 0:
                    # row 0 up-slab source is row -1: zero that dest
                    nc.gpsimd.memset(t5s[0:6, 0:1, :], 0.0)
                if last:
                    # row 255 down-slab source is row 256
                    nc.gpsimd.memset(t5s[32:38, b5r - 1 - a5 : b5r - a5, :], 0.0)

                if last:
                    # final strip: interleave gather / tap-sum into the L3
                    # drain so the tail doesn't serialize; y flushes (whose
                    # deps resolve last) go at the very end. Selector pops
                    # lag one gather batch so they never block the in-order
                    # PE queue while their gather DMAs are still in flight.
                    allp = l4b_chunks(a5, b5r)
                    chunks_f = [p for p in allp if p[1].__name__ == "emit"]
                    flushes = [p for p in allp if p[1].__name__ == "flush"]
                    gathered = 0
                    gprev = 0
                    nrl = b5r - a5

                def tail_step(final=False):
                    nonlocal gathered, gprev
                    ready = (nrl if not mma_q else
                             max(0, min(nrl, mma_q[0][0] - 1 - a5)))
                    if ready - gathered >= 8 or (ready == nrl and ready > gathered
                                                 and not mma_q):
                        gprev = gathered
                        emit_gather(a5, gathered, ready, split=True)
                        gathered = ready
                    lim = gathered if final and gathered == nrl else gprev
                    while chunks_f and chunks_f[0][0] <= lim:
                        chunks_f.pop(0)[1]()

                ci4 = 0
                for rr, n in _chunks(a3, b3r):
                    ps = pmain.tile([HID_C, n, W], f32, tag="ps")
                    conv_chunk(ps, w3_sb, h2, rr, n)
                    sl = rr % S
                    nc.scalar.activation(
                        h3[:, sl : sl + n, 1 : 1 + W], ps, Relu, bias=b3_sb
                    )
                    rr_next = rr + n
                    while mma_q and mma_q[0][0] + mma_q[0][1] + 2 <= rr_next:
                        m0, n0 = mma_q.pop(0)
                        emit_mma(m0, n0, ci4)
                        ci4 += 1
                    if last:
                        tail_step()

                while last and mma_q:
                    m0, n0 = mma_q.pop(0)
                    emit_mma(m0, n0, ci4)
                    ci4 += 1
                    tail_step()

                if not last:
                    # defer the mmA stragglers into the next strip's L1
                    # phase (their ACT/DVE copy deps get ~5us of slack) and
                    # the gather + tap-sum creation right after them
                    for m0, n0 in mma_q:
                        mma_carry.append(
                            lambda m0=m0, n0=n0, c=ci4: emit_mma(m0, n0, c))
                        ci4 += 1
                    mma_q.clear()

                    def gather_pend(a5=a5, b5=b5r):
                        nrg = b5 - a5
                        emit_gather(a5, 0, nrg // 2)
                        emit_gather(a5, nrg // 2, nrg)
                        return l4b_chunks(a5, b5)
                else:
                    while chunks_f or gathered < nrl:
                        tail_step(final=True)
                    pending = flushes

            # flush any remaining tap-sum emitters
            while pending:
                pending.pop(0)[1]()

    nc.finalize()
    return nc


_NC_CACHE = {}


def _pack_inputs(x, w1, b1, w2, b2, w3, b3, w4, b4):
    x = np.ascontiguousarray(np.asarray(x, dtype=np.float32))
    w1 = np.asarray(w1, dtype=np.float32)
    w2 = np.asarray(w2, dtype=np.float32)
    w3 = np.asarray(w3, dtype=np.float32)
    w4 = np.asarray(w4, dtype=np.float32)
    # w4a slabs: up@0-5, center@6-11, down@32-37, left@64-69, right@96-101
    w4a = np.zeros((HID_C, HID_C), np.float32)
    w4a[:, 0:OUT_C] = w4[:, :, 1].T          # up
    w4a[:, 6 : 6 + OUT_C] = w4[:, :, 0].T    # center
    w4a[:, 32 : 32 + OUT_C] = w4[:, :, 2].T  # down
    w4a[:, 64 : 64 + OUT_C] = w4[:, :, 3].T  # left
    w4a[:, 96 : 96 + OUT_C] = w4[:, :, 4].T  # right
    s6 = np.zeros((HID_C, OUT_C), np.float32)
    for base in (0, 6, 32, 64, 96):
        s6[base + np.arange(OUT_C), np.arange(OUT_C)] = 1.0
    s6 = s6.astype(ml_dtypes.bfloat16)
    common = {
        # w1p[t*6+ic, oc] = w1[oc, ic, t]
        "w1p": np.ascontiguousarray(w1.transpose(2, 1, 0).reshape(5 * IN_C, HID_C)),
        # w2p[ic, t, oc] = w2[oc, ic, t]
        "w2p": np.ascontiguousarray(w2.transpose(1, 2, 0)),
        "w3p": np.ascontiguousarray(w3.transpose(1, 2, 0)),
        "w4a": w4a,
        "s6": s6,
        "b1": np.asarray(b1, np.float32),
        "b2": np.asarray(b2, np.float32),
        "b3": np.asarray(b3, np.float32),
        "b4": np.asarray(b4, np.float32),
    }
    return x, common


def kernel(x, w1, b1, w2, b2, w3, b3, w4, b4):
    x, common = _pack_inputs(x, w1, b1, w2, b2, w3, b3, w4, b4)
    if "nc" not in _NC_CACHE:
        _NC_CACHE["nc"] = _build()
    nc = _NC_CACHE["nc"]
    in_maps = [dict(common, x=x[i]) for i in range(N_CORES)]
    res = bass_utils.run_bass_kernel_spmd(nc, in_maps, core_ids=list(range(N_CORES)))
    out = np.stack([res.results[i]["y"] for i in range(N_CORES)], axis=0)
    return out


# revision 34
# speedup vs baseline: 1.2198x; 1.0053x over previous
"""Trainium2 Bass kernel for 4-layer cross-stencil CNN.

Per-core: one image [6,256,256] (batch dim sharded across 8 cores).
conv(cross-5-stencil) = 5 channel-matmuls with spatially shifted rhs APs,
accumulated in PSUM. Channels on partitions, spatial (rows x cols) on the
free dim. fp32r matmuls (full PE rate at N>=256).

Rolling-retention pipeline: intermediate buffers h1/h2/h3/t5 are circular
row-slot buffers (S=32 slots, slot = row % 32). Each layer computes every
image row exactly once — no halo recompute. Per strip of R=24 rows the
layer blocks are skewed (L1 rows [a,a+24), L2 [a-2,a+22), L3 [a-4,a+20),
mmA [a-5,a+19), L4b [a-7,a+17)) so all reads hit rows already produced
this strip or retained from the previous one. Reads that cross the
circular wrap are split into <=2 accumulating matmuls; chunk boundaries
are chosen so writes never wrap. Image-boundary taps are handled by
narrowing the tap matmul to valid rows (no zero-pad rows needed).

L1 packs the 5 taps into K=30 via a 5-group pre-shifted x staging buffer
(one matmul per chunk), double-buffered across strips. L4 computes all 5
taps as one M=128 matmul (slabs at partitions 0/6/32/64/96), gathers the
shifted slabs into t5s by SBUF->SBUF DMA, and sums them with one K=128
selector matmul; the strip's tap-sum drains interleaved into the next
strip's L2 phase.
"""

import sys

sys.path.insert(0, "/opt/trn_rl_repo")

import ml_dtypes
import numpy as np

import concourse.bacc as bacc
import concourse.mybir as mybir
from concourse.tile import TileContext
from concourse import bass_utils

IN_C, HID_C, OUT_C = 6, 128, 6
B, H, W = 8, 256, 256
WP = W + 2  # padded width
R = 24  # rows per strip
S = 28  # circular row slots in h1/h2/h3/t5 (live windows are <= 27 rows)
XS = 28  # row slots in the x staging buffers
N_CORES = 8

f32 = mybir.dt.float32
f32r = mybir.dt.float32r
bf16 = mybir.dt.bfloat16
Add = mybir.AluOpType.add
Max = mybir.AluOpType.max
Relu = mybir.ActivationFunctionType.Relu
Ident = mybir.ActivationFunctionType.Identity

# tap order matches reference: 0=center, 1=up(x[h-1]), 2=down(x[h+1]),
# 3=left(x[w-1]), 4=right(x[w+1])


def _chunks(r0, r1):
    """2-row chunks of [r0,r1), never straddling a multiple of S."""
    r = r0
    while r < r1:
        n = min(2, r1 - r, (r // S + 1) * S - r)
        yield r, n
        r += n


def _runs(r0, r1):
    """Split [r0,r1) into runs of consecutive slots (break at S multiples)."""
    r = r0
    while r < r1:
        e = min(r1, (r // S + 1) * S)
        yield r, e
        r = e


def _build():
    nc = bacc.Bacc("TRN2", target_bir_lowering=False)

    x_d = nc.dram_tensor("x", [IN_C, H, W], f32, kind="ExternalInput")
    w1_d = nc.dram_tensor("w1p", [5 * IN_C, HID_C], f32, kind="ExternalInput")
    w23_d = nc.dram_tensor("w23p", [HID_C, 10, HID_C], f32, kind="ExternalInput")
    # w4a: all 5 taps as M=128 slabs: up@0-5, center@6-11, down@32-37,
    # left@64-69, right@96-101; zero elsewhere
    w4a_d = nc.dram_tensor("w4a", [HID_C, HID_C], f32, kind="ExternalInput")
    # s6: bf16 selector summing the 5 (pre-shifted) slabs of t5s
    s6_d = nc.dram_tensor("s6", [HID_C, OUT_C], bf16, kind="ExternalInput")
    bias_d = nc.dram_tensor("bias", [HID_C, 4], f32, kind="ExternalInput")
    y_d = nc.dram_tensor("y", [OUT_C, H, W], f32, kind="ExternalOutput")

    strips = list(range(0, H, R))  # block anchors a = 24k

    with TileContext(nc) as tc:
        with (
            tc.tile_pool(name="const", bufs=1) as cpool,
            tc.tile_pool(name="bufs", bufs=1) as bpool,
            tc.tile_pool(name="io", bufs=2) as iopool,
            tc.tile_pool(name="psmain", bufs=7, space="PSUM") as pmain,
        ):
            # --- weights / biases (resident); w1 + strip-0 x first so L1
            # can start ~1.5us in, the rest split across two DMA queues ---
            w1_sb = cpool.tile([5 * IN_C, HID_C], f32r)
            nc.sync.dma_start(out=w1_sb, in_=w1_d[:, :].bitcast(f32r))

            # --- persistent buffers ---
            # xst: 2 staging buffers, 5 tap-groups x 6ch, pre-shifted by DMA
            # placement. Strip k window: x rows [24k-2, 24k+26) at slot
            # r - 24k + 2 (center group); reading slot(r) in group g yields
            # x row r + dr_g at col w + dc_g.
            xst = [
                bpool.tile([5 * IN_C, XS, WP], f32r, name=f"xst{i}")
                for i in range(2)
            ]
            h1 = bpool.tile([HID_C, S, WP], f32r)
            h2 = bpool.tile([HID_C, S, WP], f32r)
            h3 = bpool.tile([HID_C, S, WP], f32r)
            # t5: tap partials (slabs up@0,cen@6,dn@32,lf@64,rt@96), bf16
            t5 = bpool.tile([HID_C, S, WP], bf16)
            # t5s: gathered pre-shifted taps; slot d = output row a5+d
            t5s = bpool.tile([HID_C, R, WP], bf16)

            # x staging pads: col 1 zero for the left group (parts 18:24,
            # placed at cols [2,258)), col 256 zero for the right group
            # (parts 24:30, cols [0,256)); memset all parts once, DMAs of
            # the other groups overwrite their copies each strip.
            for xb in xst:
                nc.gpsimd.memset(xb[:, :, 1:2].bitcast(f32), 0.0)
                nc.gpsimd.memset(xb[:, :, 256:257].bitcast(f32), 0.0)
                # strip-0 rows < 0 (slots 0..2 cover x rows -3..-1 in the
                # most-shifted group); also col 0 / 257 never DMA'd
                nc.gpsimd.memset(xb[:, 0:3, :].bitcast(f32), 0.0)
                nc.gpsimd.memset(xb[:, :, 0:1].bitcast(f32), 0.0)
                nc.gpsimd.memset(xb[:, :, 257:258].bitcast(f32), 0.0)

            def x_dma(a, buf):
                """Load x rows [a-1, a+R+1) into staging buffer (5 shifted
                placements); center group slot = r - a + 2."""
                lo, hi = max(0, a - 1), min(H, a + R + 1)
                src = x_d[:, lo:hi, :].bitcast(f32r)
                o = lo - a  # >= -1
                # (partitions, slot offset of x row m = m - a + ofs, cols)
                for p0, ofs, c0 in (
                    (0, 2, 1),   # center
                    (6, 3, 1),   # up: read slot(r) -> x row r-1
                    (12, 1, 1),  # down
                    (18, 2, 2),  # left: read col w+1 -> x col w-1
                    (24, 2, 0),  # right
                ):
                    nc.sync.dma_start(
                        out=xst[buf][p0 : p0 + 6, o + ofs : hi - a + ofs, c0 : c0 + W],
                        in_=src,
                    )

            x_dma(0, 0)

            bias_sb = cpool.tile([HID_C, 4], f32)
            nc.sync.dma_start(out=bias_sb, in_=bias_d[:, :])
            b1_sb = bias_sb[:, 0:1]
            b2_sb = bias_sb[:, 1:2]
            b3_sb = bias_sb[:, 2:3]
            b4_sb = bias_sb[0:OUT_C, 3:4]
            w23_sb = cpool.tile([HID_C, 10, HID_C], f32r)
            nc.sync.dma_start(out=w23_sb, in_=w23_d[:, :, :].bitcast(f32r))
            w2_sb = w23_sb[:, 0:5, :]
            w3_sb = w23_sb[:, 5:10, :]
            w4a_sb = cpool.tile([HID_C, HID_C], f32r)
            nc.sync.dma_start(out=w4a_sb, in_=w4a_d[:, :].bitcast(f32r))
            s6_sb = cpool.tile([HID_C, OUT_C], bf16)
            nc.sync.dma_start(out=s6_sb, in_=s6_d[:, :])

            # h/t5 pads: cols 0 and 257 are read (left/right taps) but
            # never written; zero once — rolling writes only touch [1,257).
            for _t in (h1, h2, h3):
                nc.vector.memset(_t[:, :, 0:1].bitcast(f32), 0.0)
                nc.vector.memset(_t[:, :, 257:258].bitcast(f32), 0.0)
            nc.vector.memset(t5[:, :, 0:1], 0.0)
            nc.vector.memset(t5[:, :, 257:258], 0.0)
            # t5s non-slab partitions are read by the selector matmul
            # (0 * garbage must not be NaN) — zero everything once
            nc.vector.memset(t5s[:, :, :], 0.0)

            def conv_chunk(ps, w_sb, src, r0, n):
                """Cross-stencil accumulation for output rows [r0, r0+n).
                Center tap first (covers the whole psum tile -> start=True
                zeroes it); up/down taps clamp to [0,H) and split at the
                circular wrap."""
                parts = []  # (tap, psum_off, rows, slot)
                for tap, dr in ((1, -1), (2, 1), (3, 0), (4, 0)):
                    lo = max(0, r0 + dr)
                    hi = min(H, r0 + n + dr)
                    for ra, rb in _runs(lo, hi):
                        parts.append((tap, ra - dr - r0, rb - ra, ra % S))
                cols = {0: (1, 1 + W), 1: (1, 1 + W), 2: (1, 1 + W),
                        3: (0, W), 4: (2, 2 + W)}
                nc.tensor.matmul(
                    ps, w_sb[:, 0, :], src[:, r0 % S : r0 % S + n, 1 : 1 + W],
                    start=True, stop=False,
                )
                for i, (tap, off, m, sl) in enumerate(parts):
                    c0, c1 = cols[tap]
                    nc.tensor.matmul(
                        ps[:, off : off + m, :],
                        w_sb[:, tap, :],
                        src[:, sl : sl + m, c0:c1],
                        start=False, stop=(i == len(parts) - 1),
                    )

            # slab parts / tap row-offset / col offsets (dst, src)
            GAT = ((0, -1, 1, 1), (6, 0, 1, 1), (32, 1, 1, 1),
                   (64, 0, 2, 1), (96, 0, 1, 2))

            def emit_gather(a5, d0, d1, split=False):
                """Gather t5 slab rows into pre-shifted t5s slots [d0,d1).
                With split=True alternate slabs between the HWDGE (sync) and
                SWDGE (gpsimd) paths so the two DMA front-ends overlap."""
                for gi, (p0, dr, cd, cs) in enumerate(GAT):
                    eng = nc.gpsimd if split and gi % 2 else nc.sync
                    lo = max(0, a5 + d0 + dr)
                    hi = min(H, a5 + d1 + dr)
                    for ra, rb in _runs(lo, hi):
                        dd = ra - dr - a5
                        m = rb - ra
                        eng.dma_start(
                            out=t5s[p0 : p0 + 6, dd : dd + m, cd : cd + W],
                            in_=t5[p0 : p0 + 6, ra % S : ra % S + m, cs : cs + W],
                        )

            def l4b_chunks(a5, b5):
                """Deferred emitters: one K=128 selector matmul per chunk
                sums the 5 pre-shifted slabs of t5s; bias on DVE into a
                strip-wide staging tile; two half-strip DMAs out."""
                nr = b5 - a5
                chunks = list(_chunks(a5, b5))
                # split into flush groups, each with its own staging tile
                nparts = 4 if last else 2
                halves = []
                prev_i = 0
                for j in range(1, nparts + 1):
                    tgt = nr * j // nparts
                    i = next((i for i, (rr, n) in enumerate(chunks)
                              if rr + n - a5 >= tgt), len(chunks) - 1)
                    if i + 1 > prev_i:
                        halves.append(chunks[prev_i : i + 1])
                        prev_i = i + 1
                out = []  # (needed t5s rows, emitter) pairs
                for half in halves:
                    if not half:
                        continue
                    lo = half[0][0] - a5
                    hi = half[-1][0] + half[-1][1] - a5
                    yst = iopool.tile([OUT_C, hi - lo, W], f32, tag="yst")
                    for ei, (rr, n) in enumerate(half):

                        def emit(rr=rr, n=n, a5=a5, yst=yst, lo=lo, ei=ei):
                            d = rr - a5
                            ps = pmain.tile([OUT_C, n, W], f32, tag="ps")
                            nc.tensor.matmul(
                                ps, s6_sb[:, :], t5s[:, d : d + n, 1 : 1 + W],
                                start=True, stop=True,
                            )
                            nc.vector.tensor_scalar_add(
                                yst[:, d - lo : d - lo + n, :], ps, b4_sb)

                        out.append((rr + n - a5, emit))

                    def flush(lo=lo, hi=hi, a5=a5, yst=yst):
                        nc.sync.dma_start(
                            out=y_d[:, a5 + lo : a5 + hi, :],
                            in_=yst[:, :, :])

                    out.append((hi, flush))
                return out

            pending = []  # tap-sum emitters from the previous strip
            mma_carry = []  # end-of-strip mmA chunks deferred into next L1
            gather_pend = None  # deferred gather + l4b creation
            for k, a in enumerate(strips):
                last = k == len(strips) - 1
                # skewed layer blocks for this strip
                a1, b1r = a, min(H, a + R)
                a2, b2r = max(0, a - 2), min(H, a + R - 2)
                a3, b3r = max(0, a - 4), min(H, a + R - 4)
                a4, b4r = max(0, a - 5), min(H, a + R - 5)
                a5, b5r = max(0, a - 7), min(H, a + R - 7)
                if last:
                    b2r = b3r = b4r = b5r = H

                # prefetch next strip's x into the other staging buffer
                if not last:
                    a_n = a + R
                    if a_n + R >= H:
                        # final strip: slots past the image tail keep stale
                        # data from two strips ago — zero them before the DMA
                        tail0 = min(H, a_n + R + 1) - a_n + 1
                        nc.gpsimd.memset(
                            xst[(k + 1) % 2][:, tail0:XS, :].bitcast(f32), 0.0)
                    x_dma(a_n, (k + 1) % 2)

                # --- L1: one K=30 matmul per chunk; copies alternate
                # DVE/ACT ---
                xb = xst[k % 2]
                ci = 0
                for rr, n in _chunks(a1, b1r):
                    ps = pmain.tile([HID_C, n, W], f32, tag="ps")
                    s0 = rr - a + 2
                    nc.tensor.matmul(
                        ps, w1_sb[:, :], xb[:, s0 : s0 + n, 1 : 1 + W],
                        start=True, stop=True,
                    )
                    sl = rr % S
                    if ci % 2 == 0:
                        nc.vector.tensor_scalar(
                            h1[:, sl : sl + n, 1 : 1 + W], ps, b1_sb, 0.0, Add, Max
                        )
                    else:
                        nc.scalar.activation(
                            h1[:, sl : sl + n, 1 : 1 + W], ps, Relu, bias=b1_sb
                        )
                    ci += 1
                    if mma_carry:
                        mma_carry.pop(0)()
                while mma_carry:
                    mma_carry.pop(0)()
                if gather_pend is not None:
                    pending = gather_pend()
                    gather_pend = None

                # --- L2: reads h1; interleave prev strip's tap-sum ---
                for rr, n in _chunks(a2, b2r):
                    ps = pmain.tile([HID_C, n, W], f32, tag="ps")
                    conv_chunk(ps, w2_sb, h1, rr, n)
                    sl = rr % S
                    nc.scalar.activation(
                        h2[:, sl : sl + n, 1 : 1 + W], ps, Relu, bias=b2_sb
                    )
                    if pending:
                        pending.pop(0)[1]()
                while pending:
                    pending.pop(0)[1]()

                # --- L3: reads h2; mmA interleaved so t5 fills (and the
                # gather can start) as h3 rows land ---
                mma_q = list(_chunks(a4, b4r))

                def emit_mma(rr, n, ci):
                    ps = pmain.tile([HID_C, n, W], f32, tag="ps")
                    sl = rr % S
                    nc.tensor.matmul(
                        ps, w4a_sb[:, :], h3[:, sl : sl + n, 1 : 1 + W],
                        start=True, stop=True,
                    )
                    if ci % 2 == 0:
                        nc.vector.tensor_copy(t5[:, sl : sl + n, 1 : 1 + W], ps)
                    else:
                        nc.scalar.activation(
                            t5[:, sl : sl + n, 1 : 1 + W], ps, Ident)

                if k == 0:
                    # row 0 up-slab source is row -1: zero that dest
                    nc.gpsimd.memset(t5s[0:6, 0:1, :], 0.0)
                if last:
                    # row 255 down-slab source is row 256
                    nc.gpsimd.memset(t5s[32:38, b5r - 1 - a5 : b5r - a5, :], 0.0)

                if last:
                    # final strip: interleave gather / tap-sum into the L3
                    # drain so the tail doesn't serialize; y flushes (whose
                    # deps resolve last) go at the very end. Selector pops
                    # lag one gather batch so they never block the in-order
                    # PE queue while their gather DMAs are still in flight.
                    allp = l4b_chunks(a5, b5r)
                    chunks_f = [p for p in allp if p[1].__name__ == "emit"]
                    flushes = [p for p in allp if p[1].__name__ == "flush"]
                    gathered = 0
                    gprev = 0
                    nrl = b5r - a5

                def tail_step(final=False):
                    nonlocal gathered, gprev
                    ready = (nrl if not mma_q else
                             max(0, min(nrl, mma_q[0][0] - 1 - a5)))
                    if ready - gathered >= 8 or (ready == nrl and ready > gathered
                                                 and not mma_q):
                        gprev = gathered
                        emit_gather(a5, gathered, ready, split=True)
                        gathered = ready
                    lim = gathered if final and gathered == nrl else gprev
                    while chunks_f and chunks_f[0][0] <= lim:
                        chunks_f.pop(0)[1]()

                ci4 = 0
                for rr, n in _chunks(a3, b3r):
                    ps = pmain.tile([HID_C, n, W], f32, tag="ps")
                    conv_chunk(ps, w3_sb, h2, rr, n)
                    sl = rr % S
                    nc.scalar.activation(
                        h3[:, sl : sl + n, 1 : 1 + W], ps, Relu, bias=b3_sb
                    )
                    rr_next = rr + n
                    while mma_q and mma_q[0][0] + mma_q[0][1] + 2 <= rr_next:
                        m0, n0 = mma_q.pop(0)
                        emit_mma(m0, n0, ci4)
                        ci4 += 1
                    if last:
                        tail_step()

                while last and mma_q:
                    m0, n0 = mma_q.pop(0)
                    emit_mma(m0, n0, ci4)
                    ci4 += 1
                    tail_step()

                if not last:
                    # defer the mmA stragglers into the next strip's L1
                    # phase (their ACT/DVE copy deps get ~5us of slack) and
                    # the gather + tap-sum creation right after them
                    for m0, n0 in mma_q:
                        mma_carry.append(
                            lambda m0=m0, n0=n0, c=ci4: emit_mma(m0, n0, c))
                        ci4 += 1
                    mma_q.clear()

                    def gather_pend(a5=a5, b5=b5r):
                        nrg = b5 - a5
                        emit_gather(a5, 0, nrg // 2)
                        emit_gather(a5, nrg // 2, nrg)
                        return l4b_chunks(a5, b5)
                else:
                    while chunks_f or gathered < nrl:
                        tail_step(final=True)
                    pending = flushes

            # flush any remaining tap-sum emitters
            while pending:
                pending.pop(0)[1]()

    nc.finalize()
    return nc


_NC_CACHE = {}


def _pack_inputs(x, w1, b1, w2, b2, w3, b3, w4, b4):
    x = np.ascontiguousarray(np.asarray(x, dtype=np.float32))
    w1 = np.asarray(w1, dtype=np.float32)
    w2 = np.asarray(w2, dtype=np.float32)
    w3 = np.asarray(w3, dtype=np.float32)
    w4 = np.asarray(w4, dtype=np.float32)
    # w4a slabs: up@0-5, center@6-11, down@32-37, left@64-69, right@96-101
    w4a = np.zeros((HID_C, HID_C), np.float32)
    w4a[:, 0:OUT_C] = w4[:, :, 1].T          # up
    w4a[:, 6 : 6 + OUT_C] = w4[:, :, 0].T    # center
    w4a[:, 32 : 32 + OUT_C] = w4[:, :, 2].T  # down
    w4a[:, 64 : 64 + OUT_C] = w4[:, :, 3].T  # left
    w4a[:, 96 : 96 + OUT_C] = w4[:, :, 4].T  # right
    s6 = np.zeros((HID_C, OUT_C), np.float32)
    for base in (0, 6, 32, 64, 96):
        s6[base + np.arange(OUT_C), np.arange(OUT_C)] = 1.0
    s6 = s6.astype(ml_dtypes.bfloat16)
    bias = np.zeros((HID_C, 4), np.float32)
    bias[:, 0] = np.asarray(b1, np.float32)
    bias[:, 1] = np.asarray(b2, np.float32)
    bias[:, 2] = np.asarray(b3, np.float32)
    bias[:OUT_C, 3] = np.asarray(b4, np.float32)
    common = {
        # w1p[t*6+ic, oc] = w1[oc, ic, t]
        "w1p": np.ascontiguousarray(w1.transpose(2, 1, 0).reshape(5 * IN_C, HID_C)),
        # w23p[ic, t, oc] = w2[oc, ic, t] (t<5) / w3[oc, ic, t-5]
        "w23p": np.ascontiguousarray(np.concatenate(
            [w2.transpose(1, 2, 0), w3.transpose(1, 2, 0)], axis=1)),
        "w4a": w4a,
        "s6": s6,
        "bias": bias,
    }
    return x, common


def kernel(x, w1, b1, w2, b2, w3, b3, w4, b4):
    x, common = _pack_inputs(x, w1, b1, w2, b2, w3, b3, w4, b4)
    if "nc" not in _NC_CACHE:
        _NC_CACHE["nc"] = _build()
    nc = _NC_CACHE["nc"]
    in_maps = [dict(common, x=x[i]) for i in range(N_CORES)]
    res = bass_utils.run_bass_kernel_spmd(nc, in_maps, core_ids=list(range(N_CORES)))
    out = np.stack([res.results[i]["y"] for i in range(N_CORES)], axis=0)
    return out


# revision 53
# speedup vs baseline: 1.2308x; 1.0090x over previous
"""Trainium2 Bass kernel for 4-layer cross-stencil CNN.

Per-core: one image [6,256,256] (batch dim sharded across 8 cores).
conv(cross-5-stencil) = 5 channel-matmuls with spatially shifted rhs APs,
accumulated in PSUM. Channels on partitions, spatial (rows x cols) on the
free dim. fp32r matmuls (full PE rate at N>=256).

Rolling-retention pipeline: intermediate buffers h1/h2/h3/t5 are circular
row-slot buffers (S=28 slots, slot = row % 28). Each layer computes every
image row exactly once — no halo recompute. Per strip of R=24 rows the
layer blocks are skewed (L1 rows [a,a+24), L2 [a-2,a+22), L3 [a-4,a+20),
mmA [a-5,a+19), L4b [a-7,a+17)) so all reads hit rows already produced
this strip or retained from the previous one. Reads that cross the
circular wrap are split into <=2 accumulating matmuls; chunk boundaries
are chosen so writes never wrap. Image-boundary taps are handled by
narrowing the tap matmul to valid rows (no zero-pad rows needed).

L1 packs the 5 taps into K=30 via a 5-group pre-shifted x staging buffer
(one matmul per chunk), double-buffered across strips. L4 computes all 5
taps as one M=128 matmul (slabs at partitions 0/6/32/64/96), gathers the
shifted slabs into t5s by SBUF->SBUF DMA, and sums them with one K=128
selector matmul; the strip's tap-sum drains interleaved into the next
strip's L2 phase.
"""

import sys

sys.path.insert(0, "/opt/trn_rl_repo")

import ml_dtypes
import numpy as np

import concourse.bacc as bacc
import concourse.mybir as mybir
from concourse.tile import TileContext
from concourse import bass_utils

IN_C, HID_C, OUT_C = 6, 128, 6
B, H, W = 8, 256, 256
WP = W + 2  # padded width
R = 24  # rows per strip
DIRECT_TAIL = 4  # final rows computed gather-free from t5
S = 28  # circular row slots in h1/h2/h3/t5 (live windows are <= 27 rows)
XS = 28  # row slots in the x staging buffers
N_CORES = 8

f32 = mybir.dt.float32
f32r = mybir.dt.float32r
bf16 = mybir.dt.bfloat16
Add = mybir.AluOpType.add
Max = mybir.AluOpType.max
Relu = mybir.ActivationFunctionType.Relu
Ident = mybir.ActivationFunctionType.Identity

# tap order matches reference: 0=center, 1=up(x[h-1]), 2=down(x[h+1]),
# 3=left(x[w-1]), 4=right(x[w+1])


def _chunks(r0, r1):
    """2-row chunks of [r0,r1), never straddling a multiple of S."""
    r = r0
    while r < r1:
        n = min(2, r1 - r, (r // S + 1) * S - r)
        yield r, n
        r += n


def _runs(r0, r1):
    """Split [r0,r1) into runs of consecutive slots (break at S multiples)."""
    r = r0
    while r < r1:
        e = min(r1, (r // S + 1) * S)
        yield r, e
        r = e


def _build():
    nc = bacc.Bacc("TRN2", target_bir_lowering=False)

    x_d = nc.dram_tensor("x", [IN_C, H, W], f32, kind="ExternalInput")
    w1_d = nc.dram_tensor("w1p", [5 * IN_C, HID_C], f32, kind="ExternalInput")
    w23_d = nc.dram_tensor("w23p", [HID_C, 10, HID_C], f32, kind="ExternalInput")
    # w4a: all 5 taps as M=128 slabs: up@0-5, center@6-11, down@32-37,
    # left@64-69, right@96-101; zero elsewhere
    w4a_d = nc.dram_tensor("w4a", [HID_C, HID_C], f32, kind="ExternalInput")
    # s6: bf16 selector summing the 5 (pre-shifted) slabs of t5s
    s6_d = nc.dram_tensor("s6", [HID_C, OUT_C], bf16, kind="ExternalInput")
    # sd: per-slab selectors for the direct (gather-free) tail chunks
    sd_d = nc.dram_tensor("sd", [HID_C, 5, OUT_C], bf16, kind="ExternalInput")
    bias_d = nc.dram_tensor("bias", [HID_C, 4], f32, kind="ExternalInput")
    y_d = nc.dram_tensor("y", [OUT_C, H, W], f32, kind="ExternalOutput")

    strips = list(range(0, H, R))  # block anchors a = 24k

    with TileContext(nc) as tc:
        with (
            tc.tile_pool(name="const", bufs=1) as cpool,
            tc.tile_pool(name="bufs", bufs=1) as bpool,
            tc.tile_pool(name="io", bufs=2) as iopool,
            tc.tile_pool(name="psmain", bufs=7, space="PSUM") as pmain,
        ):
            # --- weights / biases (resident). HWDGE costs ~625ns per DMA
            # instruction and is a single serial device, so: w1 + strip-0 x
            # first (these gate the first matmul), everything else after,
            # with biases packed into one [128,4] tensor and w2+w3 into one
            # [128,10,128] tensor to cut the instruction count ---
            w1_sb = cpool.tile([5 * IN_C, HID_C], f32r)

            # --- persistent buffers ---
            # xst: 2 staging buffers, 5 tap-groups x 6ch, pre-shifted by DMA
            # placement. Strip k window: x rows [24k-2, 24k+26) at slot
            # r - 24k + 2 (center group); reading slot(r) in group g yields
            # x row r + dr_g at col w + dc_g.
            xst = [
                bpool.tile([5 * IN_C, XS, WP], f32r, name=f"xst{i}")
                for i in range(2)
            ]
            h1 = bpool.tile([HID_C, S, WP], f32r)
            h2 = bpool.tile([HID_C, S, WP], f32r)
            h3 = bpool.tile([HID_C, S, WP], f32r)
            # t5: tap partials (slabs up@0,cen@6,dn@32,lf@64,rt@96), bf16
            t5 = bpool.tile([HID_C, S, WP], bf16)
            # t5s: gathered pre-shifted taps; slot d = output row a5+d
            t5s = bpool.tile([HID_C, R, WP], bf16)

            # x staging pads, narrowed to exactly the cells that are read
            # but never DMA'd (disjoint from every placement, so the strip-0
            # DMAs carry no memset dependency): col 1 for the left group
            # (placed at cols [2,258)), col 256 for the right group (cols
            # [0,256)), and the up-group's slot 2 (x row -1) in buffer 0.
            # Cols 0/257 and other slots are never read.
            # (engine ops need partition base 0/32/64/96, so the ranges
            # start at 0; the extra cells are DMA-overwritten every strip)
            for xb in xst:
                nc.gpsimd.memset(xb[0:24, :, 1:2].bitcast(f32), 0.0)
                nc.gpsimd.memset(xb[0:30, :, 256:257].bitcast(f32), 0.0)
            nc.gpsimd.memset(xst[0][0:12, 2:3, :].bitcast(f32), 0.0)

            def x_dma(a, buf, split=False):
                """Load x rows [a-1, a+R+1) into staging buffer (5 shifted
                placements); center group slot = r - a + 2."""
                lo, hi = max(0, a - 1), min(H, a + R + 1)
                src = x_d[:, lo:hi, :].bitcast(f32r)
                o = lo - a  # >= -1
                # (partitions, slot offset of x row m = m - a + ofs, cols)
                for gi, (p0, ofs, c0) in enumerate((
                    (0, 2, 1),   # center
                    (6, 3, 1),   # up: read slot(r) -> x row r-1
                    (12, 1, 1),  # down
                    (18, 2, 2),  # left: read col w+1 -> x col w-1
                    (24, 2, 0),  # right
                )):
                    # split across the two DMA front-ends: HWDGE (sync) and
                    # SWDGE (gpsimd) process their ~625/1050ns-per-DMA fixed
                    # costs in parallel
                    eng = nc.gpsimd if split and gi >= 3 else nc.sync
                    eng.dma_start(
                        out=xst[buf][p0 : p0 + 6, o + ofs : hi - a + ofs, c0 : c0 + W],
                        in_=src,
                    )

            x_dma(0, 0)

            # w1 after the strip-0 placements: its 43ns transfer hides in
            # the HWDGE pipeline while the placements' 427ns transfers
            # overlap later HWDGE processing
            nc.sync.dma_start(out=w1_sb, in_=w1_d[:, :].bitcast(f32r))

            bias_sb = cpool.tile([HID_C, 4], f32)
            nc.sync.dma_start(out=bias_sb, in_=bias_d[:, :])
            b1_sb = bias_sb[:, 0:1]
            b2_sb = bias_sb[:, 1:2]
            b3_sb = bias_sb[:, 2:3]
            b4_sb = bias_sb[0:OUT_C, 3:4]
            w23_sb = cpool.tile([HID_C, 10, HID_C], f32r)
            nc.sync.dma_start(out=w23_sb, in_=w23_d[:, :, :].bitcast(f32r))
            w2_sb = w23_sb[:, 0:5, :]
            w3_sb = w23_sb[:, 5:10, :]
            w4a_sb = cpool.tile([HID_C, HID_C], f32r)
            nc.sync.dma_start(out=w4a_sb, in_=w4a_d[:, :].bitcast(f32r))
            s6_sb = cpool.tile([HID_C, OUT_C], bf16)
            nc.sync.dma_start(out=s6_sb, in_=s6_d[:, :])
            sd_sb = cpool.tile([HID_C, 5, OUT_C], bf16)
            nc.sync.dma_start(out=sd_sb, in_=sd_d[:, :, :])

            # h/t5 pads: cols 0 and 257 are read (left/right taps) but
            # never written; zero once — rolling writes only touch [1,257).
            for _t in (h1, h2, h3):
                nc.vector.memset(_t[:, :, 0:1].bitcast(f32), 0.0)
                nc.vector.memset(_t[:, :, 257:258].bitcast(f32), 0.0)
            nc.vector.memset(t5[:, :, 0:1], 0.0)
            nc.vector.memset(t5[:, :, 257:258], 0.0)
            # t5s non-slab partitions are read by the selector matmul
            # (0 * garbage must not be NaN) — zero everything once
            nc.vector.memset(t5s[:, :, :], 0.0)

            def conv_chunk(ps, w_sb, src, r0, n):
                """Cross-stencil accumulation for output rows [r0, r0+n).
                Center tap first (covers the whole psum tile -> start=True
                zeroes it); up/down taps clamp to [0,H) and split at the
                circular wrap."""
                parts = []  # (tap, psum_off, rows, slot)
                for tap, dr in ((1, -1), (2, 1), (3, 0), (4, 0)):
                    lo = max(0, r0 + dr)
                    hi = min(H, r0 + n + dr)
                    for ra, rb in _runs(lo, hi):
                        parts.append((tap, ra - dr - r0, rb - ra, ra % S))
                cols = {0: (1, 1 + W), 1: (1, 1 + W), 2: (1, 1 + W),
                        3: (0, W), 4: (2, 2 + W)}
                nc.tensor.matmul(
                    ps, w_sb[:, 0, :], src[:, r0 % S : r0 % S + n, 1 : 1 + W],
                    start=True, stop=False,
                )
                for i, (tap, off, m, sl) in enumerate(parts):
                    c0, c1 = cols[tap]
                    nc.tensor.matmul(
                        ps[:, off : off + m, :],
                        w_sb[:, tap, :],
                        src[:, sl : sl + m, c0:c1],
                        start=False, stop=(i == len(parts) - 1),
                    )

            # slab parts / tap row-offset / col offsets (dst, src)
            GAT = ((0, -1, 1, 1), (6, 0, 1, 1), (32, 1, 1, 1),
                   (64, 0, 2, 1), (96, 0, 1, 2))

            def emit_gather(a5, d0, d1, split=False):
                """Gather t5 slab rows into pre-shifted t5s slots [d0,d1).
                With split=True alternate slabs between the HWDGE (sync) and
                SWDGE (gpsimd) paths so the two DMA front-ends overlap."""
                for gi, (p0, dr, cd, cs) in enumerate(GAT):
                    eng = nc.gpsimd if split and gi % 2 else nc.sync
                    lo = max(0, a5 + d0 + dr)
                    hi = min(H, a5 + d1 + dr)
                    for ra, rb in _runs(lo, hi):
                        dd = ra - dr - a5
                        m = rb - ra
                        eng.dma_start(
                            out=t5s[p0 : p0 + 6, dd : dd + m, cd : cd + W],
                            in_=t5[p0 : p0 + 6, ra % S : ra % S + m, cs : cs + W],
                        )

            def l4b_chunks(a5, b5, tail=False):
                """Deferred emitters: one K=128 selector matmul per chunk
                sums the 5 pre-shifted slabs of t5s; bias on DVE into a
                strip-wide staging tile; two half-strip DMAs out."""
                nr = b5 - a5
                chunks = list(_chunks(a5, b5))
                # split into flush groups, each with its own staging tile
                nparts = 4 if tail else 2
                halves = []
                prev_i = 0
                for j in range(1, nparts + 1):
                    tgt = nr * j // nparts
                    i = next((i for i, (rr, n) in enumerate(chunks)
                              if rr + n - a5 >= tgt), len(chunks) - 1)
                    if i + 1 > prev_i:
                        halves.append(chunks[prev_i : i + 1])
                        prev_i = i + 1
                out = []  # (needed t5s rows, emitter) pairs
                dt0 = b5 - DIRECT_TAIL if tail else b5
                for half in halves:
                    if not half:
                        continue
                    lo = half[0][0] - a5
                    hi = half[-1][0] + half[-1][1] - a5
                    yst = iopool.tile([OUT_C, hi - lo, W], f32, tag="yst")
                    for ei, (rr, n) in enumerate(half):

                        def emit(rr=rr, n=n, a5=a5, yst=yst, lo=lo, ei=ei,
                                 tail=tail):
                            d = rr - a5
                            ps = pmain.tile([OUT_C, n, W], f32, tag="ps")
                            if tail and rr >= a5 + (dt0 - a5):
                                # gather-free: per-slab selectors read t5
                                # directly with the slab's row/col shift
                                parts = []
                                for tp, dr, c0 in ((0, -1, 1), (2, 1, 1),
                                                   (3, 0, 0), (4, 0, 2)):
                                    pl = max(0, rr + dr)
                                    ph = min(H, rr + n + dr)
                                    for ra, rb in _runs(pl, ph):
                                        parts.append(
                                            (tp, ra - dr - rr, rb - ra,
                                             ra % S, c0))
                                nc.tensor.matmul(
                                    ps, sd_sb[:, 1, :],
                                    t5[:, rr % S : rr % S + n, 1 : 1 + W],
                                    start=True, stop=False,
                                )
                                for i, (tp, off, m, sl, c0) in enumerate(parts):
                                    nc.tensor.matmul(
                                        ps[:, off : off + m, :], sd_sb[:, tp, :],
                                        t5[:, sl : sl + m, c0 : c0 + W],
                                        start=False, stop=(i == len(parts) - 1),
                                    )
                            else:
                                nc.tensor.matmul(
                                    ps, s6_sb[:, :], t5s[:, d : d + n, 1 : 1 + W],
                                    start=True, stop=True,
                                )
                            if tail and ei % 2:
                                # ACT is idle at the drain; alternating
                                # halves the serialized add chain
                                nc.scalar.activation(
                                    yst[:, d - lo : d - lo + n, :], ps,
                                    Ident, bias=b4_sb)
                            else:
                                nc.vector.tensor_scalar_add(
                                    yst[:, d - lo : d - lo + n, :], ps, b4_sb)

                        need = (min(rr + n - a5, dt0 - a5) if tail
                                else rr + n - a5)
                        out.append((need, emit))

                    def flush(lo=lo, hi=hi, a5=a5, yst=yst):
                        nc.sync.dma_start(
                            out=y_d[:, a5 + lo : a5 + hi, :],
                            in_=yst[:, :, :])

                    out.append((hi, flush))
                return out

            pending = []  # tap-sum emitters from the previous strip
            mma_carry = []  # end-of-strip mmA chunks deferred into next L1
            gather_pend = None  # deferred gather + l4b creation
            for k, a in enumerate(strips):
                last = k == len(strips) - 1
                # skewed layer blocks for this strip
                a1, b1r = a, min(H, a + R)
                a2, b2r = max(0, a - 2), min(H, a + R - 2)
                a3, b3r = max(0, a - 4), min(H, a + R - 4)
                a4, b4r = max(0, a - 5), min(H, a + R - 5)
                a5, b5r = max(0, a - 7), min(H, a + R - 7)
                if last:
                    b2r = b3r = b4r = b5r = H

                # prefetch next strip's x into the other staging buffer
                if not last:
                    a_n = a + R
                    if a_n + R >= H:
                        # final strip: slots past the image tail keep stale
                        # data from two strips ago — zero them before the DMA
                        tail0 = min(H, a_n + R + 1) - a_n + 1
                        nc.gpsimd.memset(
                            xst[(k + 1) % 2][:, tail0:XS, :].bitcast(f32), 0.0)
                    x_dma(a_n, (k + 1) % 2)

                # --- L1: one K=30 matmul per chunk; copies alternate
                # DVE/ACT ---
                xb = xst[k % 2]
                ci = 0
                for rr, n in _chunks(a1, b1r):
                    ps = pmain.tile([HID_C, n, W], f32, tag="ps")
                    s0 = rr - a + 2
                    nc.tensor.matmul(
                        ps, w1_sb[:, :], xb[:, s0 : s0 + n, 1 : 1 + W],
                        start=True, stop=True,
                    )
                    sl = rr % S
                    if ci % 2 == 0:
                        nc.vector.tensor_scalar(
                            h1[:, sl : sl + n, 1 : 1 + W], ps, b1_sb, 0.0, Add, Max
                        )
                    else:
                        nc.scalar.activation(
                            h1[:, sl : sl + n, 1 : 1 + W], ps, Relu, bias=b1_sb
                        )
                    ci += 1
                    if mma_carry:
                        mma_carry.pop(0)()
                while mma_carry:
                    mma_carry.pop(0)()
                if gather_pend is not None:
                    pending = gather_pend()
                    gather_pend = None

                # --- L2: reads h1; interleave prev strip's tap-sum ---
                for ci2, (rr, n) in enumerate(_chunks(a2, b2r)):
                    ps = pmain.tile([HID_C, n, W], f32, tag="ps")
                    conv_chunk(ps, w2_sb, h1, rr, n)
                    sl = rr % S
                    nc.scalar.activation(
                        h2[:, sl : sl + n, 1 : 1 + W], ps, Relu, bias=b2_sb
                    )
                    # skip the first chunks: the previous strip's gather DMAs
                    # may still be in flight, and a waiting selector at the
                    # head of the in-order PE queue blocks the L2 convs
                    if pending and ci2 >= 2:
                        pending.pop(0)[1]()
                while pending:
                    pending.pop(0)[1]()

                # --- L3: reads h2; mmA interleaved so t5 fills (and the
                # gather can start) as h3 rows land ---
                mma_q = list(_chunks(a4, b4r))

                def emit_mma(rr, n, ci):
                    ps = pmain.tile([HID_C, n, W], f32, tag="ps")
                    sl = rr % S
                    nc.tensor.matmul(
                        ps, w4a_sb[:, :], h3[:, sl : sl + n, 1 : 1 + W],
                        start=True, stop=True,
                    )
                    if ci % 2 == 0:
                        nc.vector.tensor_copy(t5[:, sl : sl + n, 1 : 1 + W], ps)
                    else:
                        nc.scalar.activation(
                            t5[:, sl : sl + n, 1 : 1 + W], ps, Ident)

                if k == 0:
                    # row 0 up-slab source is row -1: zero that dest
                    nc.gpsimd.memset(t5s[0:6, 0:1, :], 0.0)
                if last:
                    # row 255 down-slab source is row 256
                    nc.gpsimd.memset(t5s[32:38, b5r - 1 - a5 : b5r - a5, :], 0.0)

                if last:
                    # final strip: interleave gather / tap-sum into the L3
                    # drain so the tail doesn't serialize; y flushes (whose
                    # deps resolve last) go at the very end. Selector pops
                    # lag one gather batch so they never block the in-order
                    # PE queue while their gather DMAs are still in flight.
                    allp = l4b_chunks(a5, b5r, tail=True)
                    chunks_f = [p for p in allp if p[1].__name__ == "emit"]
                    flushes = [p for p in allp if p[1].__name__ == "flush"]
                    gathered = 0
                    gprev = 0
                    gage = 0  # tail_steps since the last gather batch
                    nrl = b5r - a5

                def tail_step(final=False):
                    nonlocal gathered, gprev, gage
                    nrg = nrl - DIRECT_TAIL
                    ready = (nrg if not mma_q else
                             max(0, min(nrg, mma_q[0][0] - 1 - a5)))
                    if ready - gathered >= 12 or (ready == nrg and ready > gathered
                                                  and not mma_q):
                        gprev = gathered
                        emit_gather(a5, gathered, ready, split=True)
                        gathered = ready
                        gage = 0
                    gage += 1
                    # a gather's DMAs are safely in flight after ~2 more L3/
                    # mmA chunks of PE work; then its selectors won't block
                    # the in-order PE queue
                    lim = gathered if (final and gathered == nrl - DIRECT_TAIL) \
                        or gage > 5 else gprev
                    while chunks_f and chunks_f[0][0] <= lim:
                        chunks_f.pop(0)[1]()

                ci4 = 0
                for rr, n in _chunks(a3, b3r):
                    ps = pmain.tile([HID_C, n, W], f32, tag="ps")
                    conv_chunk(ps, w3_sb, h2, rr, n)
                    sl = rr % S
                    nc.scalar.activation(
                        h3[:, sl : sl + n, 1 : 1 + W], ps, Relu, bias=b3_sb
                    )
                    rr_next = rr + n
                    while mma_q and mma_q[0][0] + mma_q[0][1] + 2 <= rr_next:
                        m0, n0 = mma_q.pop(0)
                        emit_mma(m0, n0, ci4)
                        ci4 += 1
                    if last:
                        tail_step()

                while last and mma_q:
                    m0, n0 = mma_q.pop(0)
                    emit_mma(m0, n0, ci4)
                    ci4 += 1
                    tail_step()

                if not last:
                    # defer the mmA stragglers into the next strip's L1
                    # phase (their ACT/DVE copy deps get ~5us of slack) and
                    # the gather + tap-sum creation right after them
                    for m0, n0 in mma_q:
                        mma_carry.append(
                            lambda m0=m0, n0=n0, c=ci4: emit_mma(m0, n0, c))
                        ci4 += 1
                    mma_q.clear()

                    def gather_pend(a5=a5, b5=b5r):
                        nrg = b5 - a5
                        emit_gather(a5, 0, nrg // 2)
                        emit_gather(a5, nrg // 2, nrg)
                        return l4b_chunks(a5, b5)
                else:
                    while chunks_f or gathered < nrl - DIRECT_TAIL:
                        tail_step(final=True)
                    pending = flushes

            # flush any remaining tap-sum emitters
            while pending:
                pending.pop(0)[1]()

    nc.finalize()
    return nc


_NC_CACHE = {}


def _pack_inputs(x, w1, b1, w2, b2, w3, b3, w4, b4):
    x = np.ascontiguousarray(np.asarray(x, dtype=np.float32))
    w1 = np.asarray(w1, dtype=np.float32)
    w2 = np.asarray(w2, dtype=np.float32)
    w3 = np.asarray(w3, dtype=np.float32)
    w4 = np.asarray(w4, dtype=np.float32)
    # w4a slabs: up@0-5, center@6-11, down@32-37, left@64-69, right@96-101
    w4a = np.zeros((HID_C, HID_C), np.float32)
    w4a[:, 0:OUT_C] = w4[:, :, 1].T          # up
    w4a[:, 6 : 6 + OUT_C] = w4[:, :, 0].T    # center
    w4a[:, 32 : 32 + OUT_C] = w4[:, :, 2].T  # down
    w4a[:, 64 : 64 + OUT_C] = w4[:, :, 3].T  # left
    w4a[:, 96 : 96 + OUT_C] = w4[:, :, 4].T  # right
    s6 = np.zeros((HID_C, OUT_C), np.float32)
    for base in (0, 6, 32, 64, 96):
        s6[base + np.arange(OUT_C), np.arange(OUT_C)] = 1.0
    s6 = s6.astype(ml_dtypes.bfloat16)
    sd = np.zeros((HID_C, 5, OUT_C), np.float32)
    for t, base in enumerate((0, 6, 32, 64, 96)):  # up,cen,dn,lf,rt slabs
        sd[base + np.arange(OUT_C), t, np.arange(OUT_C)] = 1.0
    sd = sd.astype(ml_dtypes.bfloat16)
    bias = np.zeros((HID_C, 4), np.float32)
    bias[:, 0] = np.asarray(b1, np.float32)
    bias[:, 1] = np.asarray(b2, np.float32)
    bias[:, 2] = np.asarray(b3, np.float32)
    bias[:OUT_C, 3] = np.asarray(b4, np.float32)
    common = {
        # w1p[t*6+ic, oc] = w1[oc, ic, t]
        "w1p": np.ascontiguousarray(w1.transpose(2, 1, 0).reshape(5 * IN_C, HID_C)),
        # w23p[ic, t, oc] = w2[oc, ic, t] (t<5) / w3[oc, ic, t-5]
        "w23p": np.ascontiguousarray(np.concatenate(
            [w2.transpose(1, 2, 0), w3.transpose(1, 2, 0)], axis=1)),
        "w4a": w4a,
        "s6": s6,
        "sd": sd,
        "bias": bias,
    }
    return x, common


def kernel(x, w1, b1, w2, b2, w3, b3, w4, b4):
    x, common = _pack_inputs(x, w1, b1, w2, b2, w3, b3, w4, b4)
    if "nc" not in _NC_CACHE:
        _NC_CACHE["nc"] = _build()
    nc = _NC_CACHE["nc"]
    in_maps = [dict(common, x=x[i]) for i in range(N_CORES)]
    res = bass_utils.run_bass_kernel_spmd(nc, in_maps, core_ids=list(range(N_CORES)))
    out = np.stack([res.results[i]["y"] for i in range(N_CORES)], axis=0)
    return out


# revision 61
# speedup vs baseline: 1.2349x; 1.0033x over previous
"""Trainium2 Bass kernel for 4-layer cross-stencil CNN.

Per-core: one image [6,256,256] (batch dim sharded across 8 cores).
conv(cross-5-stencil) = 5 channel-matmuls with spatially shifted rhs APs,
accumulated in PSUM. Channels on partitions, spatial (rows x cols) on the
free dim. fp32r matmuls (full PE rate at N>=256).

Rolling-retention pipeline: intermediate buffers h1/h2/h3/t5 are circular
row-slot buffers (S=28 slots, slot = row % 28). Each layer computes every
image row exactly once — no halo recompute. Per strip of R=24 rows the
layer blocks are skewed (L1 rows [a,a+24), L2 [a-2,a+22), L3 [a-4,a+20),
mmA [a-5,a+19), L4b [a-7,a+17)) so all reads hit rows already produced
this strip or retained from the previous one. Reads that cross the
circular wrap are split into <=2 accumulating matmuls; chunk boundaries
are chosen so writes never wrap. Image-boundary taps are handled by
narrowing the tap matmul to valid rows (no zero-pad rows needed).

L1 packs the 5 taps into K=30 via a 5-group pre-shifted x staging buffer
(one matmul per chunk), double-buffered across strips. L4 computes all 5
taps as one M=128 matmul (slabs at partitions 0/6/32/64/96), gathers the
shifted slabs into t5s by SBUF->SBUF DMA, and sums them with one K=128
selector matmul; the strip's tap-sum drains interleaved into the next
strip's L2 phase.
"""

import sys

sys.path.insert(0, "/opt/trn_rl_repo")

import ml_dtypes
import numpy as np

import concourse.bacc as bacc
import concourse.mybir as mybir
from concourse.tile import TileContext
from concourse import bass_utils

IN_C, HID_C, OUT_C = 6, 128, 6
B, H, W = 8, 256, 256
WP = W + 2  # padded width
R = 24  # rows per strip
DIRECT_TAIL = 4  # final rows computed gather-free from t5
S = 28  # circular row slots in h1/h2/h3/t5 (live windows are <= 27 rows)
XS = 28  # row slots in the x staging buffers
N_CORES = 8

f32 = mybir.dt.float32
f32r = mybir.dt.float32r
bf16 = mybir.dt.bfloat16
Add = mybir.AluOpType.add
Max = mybir.AluOpType.max
Relu = mybir.ActivationFunctionType.Relu
Ident = mybir.ActivationFunctionType.Identity

# tap order matches reference: 0=center, 1=up(x[h-1]), 2=down(x[h+1]),
# 3=left(x[w-1]), 4=right(x[w+1])


def _chunks(r0, r1):
    """2-row chunks of [r0,r1), never straddling a multiple of S."""
    r = r0
    while r < r1:
        n = min(2, r1 - r, (r // S + 1) * S - r)
        yield r, n
        r += n


def _runs(r0, r1):
    """Split [r0,r1) into runs of consecutive slots (break at S multiples)."""
    r = r0
    while r < r1:
        e = min(r1, (r // S + 1) * S)
        yield r, e
        r = e


def _build():
    nc = bacc.Bacc("TRN2", target_bir_lowering=False)

    x_d = nc.dram_tensor("x", [IN_C, H, W], f32, kind="ExternalInput")
    w1_d = nc.dram_tensor("w1p", [5 * IN_C, HID_C], f32, kind="ExternalInput")
    w23_d = nc.dram_tensor("w23p", [HID_C, 10, HID_C], f32, kind="ExternalInput")
    # w4a: all 5 taps as M=128 slabs: up@0-5, center@6-11, down@32-37,
    # left@64-69, right@96-101; zero elsewhere
    w4a_d = nc.dram_tensor("w4a", [HID_C, HID_C], f32, kind="ExternalInput")
    # s6: bf16 selector summing the 5 (pre-shifted) slabs of t5s
    s6_d = nc.dram_tensor("s6", [HID_C, OUT_C], bf16, kind="ExternalInput")
    # sd: per-slab selectors for the direct (gather-free) tail chunks
    sd_d = nc.dram_tensor("sd", [HID_C, 5, OUT_C], bf16, kind="ExternalInput")
    bias_d = nc.dram_tensor("bias", [HID_C, 4], f32, kind="ExternalInput")
    y_d = nc.dram_tensor("y", [OUT_C, H, W], f32, kind="ExternalOutput")

    strips = list(range(0, H, R))  # block anchors a = 24k

    with TileContext(nc) as tc:
        with (
            tc.tile_pool(name="const", bufs=1) as cpool,
            tc.tile_pool(name="bufs", bufs=1) as bpool,
            tc.tile_pool(name="io", bufs=2) as iopool,
            tc.tile_pool(name="psmain", bufs=7, space="PSUM") as pmain,
        ):
            # --- weights / biases (resident). HWDGE costs ~625ns per DMA
            # instruction and is a single serial device, so: w1 + strip-0 x
            # first (these gate the first matmul), everything else after,
            # with biases packed into one [128,4] tensor and w2+w3 into one
            # [128,10,128] tensor to cut the instruction count ---
            w1_sb = cpool.tile([5 * IN_C, HID_C], f32r)

            # --- persistent buffers ---
            # xst: 2 staging buffers, 5 tap-groups x 6ch, pre-shifted by DMA
            # placement. Strip k window: x rows [24k-2, 24k+26) at slot
            # r - 24k + 2 (center group); reading slot(r) in group g yields
            # x row r + dr_g at col w + dc_g.
            xst = [
                bpool.tile([5 * IN_C, XS, WP], f32r, name=f"xst{i}")
                for i in range(2)
            ]
            h1 = bpool.tile([HID_C, S, WP], f32r)
            h2 = bpool.tile([HID_C, S, WP], f32r)
            h3 = bpool.tile([HID_C, S, WP], f32r)
            # t5: tap partials (slabs up@0,cen@6,dn@32,lf@64,rt@96), bf16
            t5 = bpool.tile([HID_C, S, WP], bf16)
            # t5s: gathered pre-shifted taps; slot d = output row a5+d
            t5s = bpool.tile([HID_C, R, WP], bf16)

            # x staging pads, narrowed to exactly the cells that are read
            # but never DMA'd (disjoint from every placement, so the strip-0
            # DMAs carry no memset dependency): col 1 for the left group
            # (placed at cols [2,258)), col 256 for the right group (cols
            # [0,256)), and the up-group's slot 2 (x row -1) in buffer 0.
            # Cols 0/257 and other slots are never read.
            # (engine ops need partition base 0/32/64/96, so the ranges
            # start at 0; the extra cells are DMA-overwritten every strip)
            for xb in xst:
                nc.gpsimd.memset(xb[0:24, :, 1:2].bitcast(f32), 0.0)
                nc.gpsimd.memset(xb[0:30, :, 256:257].bitcast(f32), 0.0)
            nc.gpsimd.memset(xst[0][0:12, 2:3, :].bitcast(f32), 0.0)

            def x_dma(a, buf, split=False):
                """Load x rows [a-1, a+R+1) into staging buffer (5 shifted
                placements); center group slot = r - a + 2."""
                lo, hi = max(0, a - 1), min(H, a + R + 1)
                src = x_d[:, lo:hi, :].bitcast(f32r)
                o = lo - a  # >= -1
                # (partitions, slot offset of x row m = m - a + ofs, cols)
                for gi, (p0, ofs, c0) in enumerate((
                    (0, 2, 1),   # center
                    (6, 3, 1),   # up: read slot(r) -> x row r-1
                    (12, 1, 1),  # down
                    (18, 2, 2),  # left: read col w+1 -> x col w-1
                    (24, 2, 0),  # right
                )):
                    # split across the two DMA front-ends: HWDGE (sync) and
                    # SWDGE (gpsimd) process their ~625/1050ns-per-DMA fixed
                    # costs in parallel
                    eng = nc.gpsimd if split and gi >= 3 else nc.sync
                    eng.dma_start(
                        out=xst[buf][p0 : p0 + 6, o + ofs : hi - a + ofs, c0 : c0 + W],
                        in_=src,
                    )

            x_dma(0, 0)

            # w1 after the strip-0 placements: its 43ns transfer hides in
            # the HWDGE pipeline while the placements' 427ns transfers
            # overlap later HWDGE processing
            nc.sync.dma_start(out=w1_sb, in_=w1_d[:, :].bitcast(f32r))

            bias_sb = cpool.tile([HID_C, 4], f32)
            nc.sync.dma_start(out=bias_sb, in_=bias_d[:, :])
            b1_sb = bias_sb[:, 0:1]
            b2_sb = bias_sb[:, 1:2]
            b3_sb = bias_sb[:, 2:3]
            b4_sb = bias_sb[0:OUT_C, 3:4]
            w23_sb = cpool.tile([HID_C, 10, HID_C], f32r)
            nc.sync.dma_start(out=w23_sb, in_=w23_d[:, :, :].bitcast(f32r))
            w2_sb = w23_sb[:, 0:5, :]
            w3_sb = w23_sb[:, 5:10, :]
            w4a_sb = cpool.tile([HID_C, HID_C], f32r)
            nc.sync.dma_start(out=w4a_sb, in_=w4a_d[:, :].bitcast(f32r))
            s6_sb = cpool.tile([HID_C, OUT_C], bf16)
            nc.sync.dma_start(out=s6_sb, in_=s6_d[:, :])
            sd_sb = cpool.tile([HID_C, 5, OUT_C], bf16)
            nc.sync.dma_start(out=sd_sb, in_=sd_d[:, :, :])

            # h/t5 pads: cols 0 and 257 are read (left/right taps) but
            # never written; zero once — rolling writes only touch [1,257).
            for _t in (h1, h2, h3):
                nc.vector.memset(_t[:, :, 0:1].bitcast(f32), 0.0)
                nc.vector.memset(_t[:, :, 257:258].bitcast(f32), 0.0)
            nc.vector.memset(t5[:, :, 0:1], 0.0)
            nc.vector.memset(t5[:, :, 257:258], 0.0)
            # t5s non-slab partitions are read by the selector matmul
            # (0 * garbage must not be NaN) — zero everything once
            nc.vector.memset(t5s[:, :, :], 0.0)

            def conv_chunk(ps, w_sb, src, r0, n):
                """Cross-stencil accumulation for output rows [r0, r0+n).
                Center tap first (covers the whole psum tile -> start=True
                zeroes it); up/down taps clamp to [0,H) and split at the
                circular wrap."""
                parts = []  # (tap, psum_off, rows, slot)
                for tap, dr in ((1, -1), (2, 1), (3, 0), (4, 0)):
                    lo = max(0, r0 + dr)
                    hi = min(H, r0 + n + dr)
                    for ra, rb in _runs(lo, hi):
                        parts.append((tap, ra - dr - r0, rb - ra, ra % S))
                cols = {0: (1, 1 + W), 1: (1, 1 + W), 2: (1, 1 + W),
                        3: (0, W), 4: (2, 2 + W)}
                nc.tensor.matmul(
                    ps, w_sb[:, 0, :], src[:, r0 % S : r0 % S + n, 1 : 1 + W],
                    start=True, stop=False,
                )
                for i, (tap, off, m, sl) in enumerate(parts):
                    c0, c1 = cols[tap]
                    nc.tensor.matmul(
                        ps[:, off : off + m, :],
                        w_sb[:, tap, :],
                        src[:, sl : sl + m, c0:c1],
                        start=False, stop=(i == len(parts) - 1),
                    )

            # slab parts / tap row-offset / col offsets (dst, src)
            GAT = ((0, -1, 1, 1), (6, 0, 1, 1), (32, 1, 1, 1),
                   (64, 0, 2, 1), (96, 0, 1, 2))

            def emit_gather(a5, d0, d1, split=False):
                """Gather t5 slab rows into pre-shifted t5s slots [d0,d1).
                With split=True alternate slabs between the HWDGE (sync) and
                SWDGE (gpsimd) paths so the two DMA front-ends overlap."""
                for gi, (p0, dr, cd, cs) in enumerate(GAT):
                    eng = nc.gpsimd if split and gi % 2 else nc.sync
                    lo = max(0, a5 + d0 + dr)
                    hi = min(H, a5 + d1 + dr)
                    for ra, rb in _runs(lo, hi):
                        dd = ra - dr - a5
                        m = rb - ra
                        eng.dma_start(
                            out=t5s[p0 : p0 + 6, dd : dd + m, cd : cd + W],
                            in_=t5[p0 : p0 + 6, ra % S : ra % S + m, cs : cs + W],
                        )

            def l4b_chunks(a5, b5, tail=False):
                """Deferred emitters: one K=128 selector matmul per chunk
                sums the 5 pre-shifted slabs of t5s; bias on DVE into a
                strip-wide staging tile; two half-strip DMAs out."""
                nr = b5 - a5
                chunks = list(_chunks(a5, b5))
                # split into flush groups, each with its own staging tile
                nparts = 4 if tail else 2
                halves = []
                prev_i = 0
                for j in range(1, nparts + 1):
                    tgt = nr * j // nparts
                    i = next((i for i, (rr, n) in enumerate(chunks)
                              if rr + n - a5 >= tgt), len(chunks) - 1)
                    if i + 1 > prev_i:
                        halves.append(chunks[prev_i : i + 1])
                        prev_i = i + 1
                out = []  # (needed t5s rows, emitter) pairs
                dt0 = b5 - DIRECT_TAIL if tail else b5
                for half in halves:
                    if not half:
                        continue
                    lo = half[0][0] - a5
                    hi = half[-1][0] + half[-1][1] - a5
                    yst = iopool.tile([OUT_C, hi - lo, W], f32, tag="yst")
                    for ei, (rr, n) in enumerate(half):

                        def emit(rr=rr, n=n, a5=a5, yst=yst, lo=lo, ei=ei,
                                 tail=tail):
                            d = rr - a5
                            ps = pmain.tile([OUT_C, n, W], f32, tag="ps")
                            if tail and rr >= a5 + (dt0 - a5):
                                # gather-free: per-slab selectors read t5
                                # directly with the slab's row/col shift
                                parts = []
                                for tp, dr, c0 in ((0, -1, 1), (2, 1, 1),
                                                   (3, 0, 0), (4, 0, 2)):
                                    pl = max(0, rr + dr)
                                    ph = min(H, rr + n + dr)
                                    for ra, rb in _runs(pl, ph):
                                        parts.append(
                                            (tp, ra - dr - rr, rb - ra,
                                             ra % S, c0))
                                nc.tensor.matmul(
                                    ps, sd_sb[:, 1, :],
                                    t5[:, rr % S : rr % S + n, 1 : 1 + W],
                                    start=True, stop=False,
                                )
                                for i, (tp, off, m, sl, c0) in enumerate(parts):
                                    nc.tensor.matmul(
                                        ps[:, off : off + m, :], sd_sb[:, tp, :],
                                        t5[:, sl : sl + m, c0 : c0 + W],
                                        start=False, stop=(i == len(parts) - 1),
                                    )
                            else:
                                nc.tensor.matmul(
                                    ps, s6_sb[:, :], t5s[:, d : d + n, 1 : 1 + W],
                                    start=True, stop=True,
                                )
                            if tail and ei % 2:
                                # ACT is idle at the drain; alternating
                                # halves the serialized add chain
                                nc.scalar.activation(
                                    yst[:, d - lo : d - lo + n, :], ps,
                                    Ident, bias=b4_sb)
                            else:
                                nc.vector.tensor_scalar_add(
                                    yst[:, d - lo : d - lo + n, :], ps, b4_sb)

                        need = (min(rr + n - a5, dt0 - a5) if tail
                                else rr + n - a5)
                        out.append((need, emit))

                    def flush(lo=lo, hi=hi, a5=a5, yst=yst):
                        nc.sync.dma_start(
                            out=y_d[:, a5 + lo : a5 + hi, :],
                            in_=yst[:, :, :])

                    out.append((hi, flush))
                return out

            pending = []  # tap-sum emitters from the previous strip
            mma_carry = []  # end-of-strip mmA chunks deferred into next L1
            gather_pend = None  # deferred gather + l4b creation
            for k, a in enumerate(strips):
                last = k == len(strips) - 1
                # skewed layer blocks for this strip
                a1, b1r = a, min(H, a + R)
                a2, b2r = max(0, a - 2), min(H, a + R - 2)
                a3, b3r = max(0, a - 4), min(H, a + R - 4)
                a4, b4r = max(0, a - 5), min(H, a + R - 5)
                a5, b5r = max(0, a - 7), min(H, a + R - 7)
                if last:
                    b2r = b3r = b4r = b5r = H

                # prefetch next strip's x into the other staging buffer
                if not last:
                    a_n = a + R
                    if a_n + R >= H:
                        # final strip: slots past the image tail keep stale
                        # data from two strips ago — zero them before the DMA
                        tail0 = min(H, a_n + R + 1) - a_n + 1
                        nc.gpsimd.memset(
                            xst[(k + 1) % 2][:, tail0:XS, :].bitcast(f32), 0.0)
                    x_dma(a_n, (k + 1) % 2)

                # --- L1: one K=30 matmul per chunk; copies alternate
                # DVE/ACT ---
                xb = xst[k % 2]
                ci = 0
                for rr, n in _chunks(a1, b1r):
                    ps = pmain.tile([HID_C, n, W], f32, tag="ps")
                    s0 = rr - a + 2
                    nc.tensor.matmul(
                        ps, w1_sb[:, :], xb[:, s0 : s0 + n, 1 : 1 + W],
                        start=True, stop=True,
                    )
                    sl = rr % S
                    if ci % 2 == 0:
                        nc.vector.tensor_scalar(
                            h1[:, sl : sl + n, 1 : 1 + W], ps, b1_sb, 0.0, Add, Max
                        )
                    else:
                        nc.scalar.activation(
                            h1[:, sl : sl + n, 1 : 1 + W], ps, Relu, bias=b1_sb
                        )
                    ci += 1
                    if mma_carry and ci >= 3:
                        mma_carry.pop(0)()
                while mma_carry:
                    mma_carry.pop(0)()
                if gather_pend is not None:
                    pending = gather_pend()
                    gather_pend = None

                # --- L2: reads h1; interleave prev strip's tap-sum ---
                for ci2, (rr, n) in enumerate(_chunks(a2, b2r)):
                    ps = pmain.tile([HID_C, n, W], f32, tag="ps")
                    conv_chunk(ps, w2_sb, h1, rr, n)
                    sl = rr % S
                    nc.scalar.activation(
                        h2[:, sl : sl + n, 1 : 1 + W], ps, Relu, bias=b2_sb
                    )
                    # skip the first chunks: the previous strip's gather DMAs
                    # may still be in flight, and a waiting selector at the
                    # head of the in-order PE queue blocks the L2 convs
                    if pending and ci2 >= 0:
                        pending.pop(0)[1]()
                while pending:
                    pending.pop(0)[1]()

                # --- L3: reads h2; mmA interleaved so t5 fills (and the
                # gather can start) as h3 rows land ---
                mma_q = list(_chunks(a4, b4r))

                def emit_mma(rr, n, ci):
                    ps = pmain.tile([HID_C, n, W], f32, tag="ps")
                    sl = rr % S
                    nc.tensor.matmul(
                        ps, w4a_sb[:, :], h3[:, sl : sl + n, 1 : 1 + W],
                        start=True, stop=True,
                    )
                    if ci % 2 == 0:
                        nc.vector.tensor_copy(t5[:, sl : sl + n, 1 : 1 + W], ps)
                    else:
                        nc.scalar.activation(
                            t5[:, sl : sl + n, 1 : 1 + W], ps, Ident)

                if k == 0:
                    # row 0 up-slab source is row -1: zero that dest
                    nc.gpsimd.memset(t5s[0:6, 0:1, :], 0.0)
                if last:
                    # row 255 down-slab source is row 256
                    nc.gpsimd.memset(t5s[32:38, b5r - 1 - a5 : b5r - a5, :], 0.0)

                if last:
                    # final strip: interleave gather / tap-sum into the L3
                    # drain so the tail doesn't serialize; y flushes (whose
                    # deps resolve last) go at the very end. Selector pops
                    # lag one gather batch so they never block the in-order
                    # PE queue while their gather DMAs are still in flight.
                    allp = l4b_chunks(a5, b5r, tail=True)
                    chunks_f = [p for p in allp if p[1].__name__ == "emit"]
                    flushes = [p for p in allp if p[1].__name__ == "flush"]
                    gathered = 0
                    gprev = 0
                    gage = 0  # tail_steps since the last gather batch
                    nrl = b5r - a5

                def tail_step(final=False):
                    nonlocal gathered, gprev, gage
                    nrg = nrl - DIRECT_TAIL
                    ready = (nrg if not mma_q else
                             max(0, min(nrg, mma_q[0][0] - 1 - a5)))
                    if ready - gathered >= 12 or (ready == nrg and ready > gathered
                                                  and not mma_q):
                        gprev = gathered
                        emit_gather(a5, gathered, ready, split=True)
                        gathered = ready
                        gage = 0
                    gage += 1
                    # a gather's DMAs are safely in flight after ~2 more L3/
                    # mmA chunks of PE work; then its selectors won't block
                    # the in-order PE queue
                    lim = gathered if (final and gathered == nrl - DIRECT_TAIL) \
                        or gage > 5 else gprev
                    while chunks_f and chunks_f[0][0] <= lim:
                        chunks_f.pop(0)[1]()

                ci4 = 0
                for rr, n in _chunks(a3, b3r):
                    ps = pmain.tile([HID_C, n, W], f32, tag="ps")
                    conv_chunk(ps, w3_sb, h2, rr, n)
                    sl = rr % S
                    nc.scalar.activation(
                        h3[:, sl : sl + n, 1 : 1 + W], ps, Relu, bias=b3_sb
                    )
                    rr_next = rr + n
                    while mma_q and mma_q[0][0] + mma_q[0][1] + 2 <= rr_next:
                        m0, n0 = mma_q.pop(0)
                        emit_mma(m0, n0, ci4)
                        ci4 += 1
                    if last:
                        tail_step()

                while last and mma_q:
                    m0, n0 = mma_q.pop(0)
                    emit_mma(m0, n0, ci4)
                    ci4 += 1
                    tail_step()

                if not last:
                    # defer the mmA stragglers into the next strip's L1
                    # phase (their ACT/DVE copy deps get ~5us of slack) and
                    # the gather + tap-sum creation right after them
                    for m0, n0 in mma_q:
                        mma_carry.append(
                            lambda m0=m0, n0=n0, c=ci4: emit_mma(m0, n0, c))
                        ci4 += 1
                    mma_q.clear()

                    def gather_pend(a5=a5, b5=b5r):
                        nrg = b5 - a5
                        emit_gather(a5, 0, nrg // 2)
                        emit_gather(a5, nrg // 2, nrg)
                        return l4b_chunks(a5, b5)
                else:
                    while chunks_f or gathered < nrl - DIRECT_TAIL:
                        tail_step(final=True)
                    pending = flushes

            # flush any remaining tap-sum emitters
            while pending:
                pending.pop(0)[1]()

    nc.finalize()
    return nc


_NC_CACHE = {}


def _pack_inputs(x, w1, b1, w2, b2, w3, b3, w4, b4):
    x = np.ascontiguousarray(np.asarray(x, dtype=np.float32))
    w1 = np.asarray(w1, dtype=np.float32)
    w2 = np.asarray(w2, dtype=np.float32)
    w3 = np.asarray(w3, dtype=np.float32)
    w4 = np.asarray(w4, dtype=np.float32)
    # w4a slabs: up@0-5, center@6-11, down@32-37, left@64-69, right@96-101
    w4a = np.zeros((HID_C, HID_C), np.float32)
    w4a[:, 0:OUT_C] = w4[:, :, 1].T          # up
    w4a[:, 6 : 6 + OUT_C] = w4[:, :, 0].T    # center
    w4a[:, 32 : 32 + OUT_C] = w4[:, :, 2].T  # down
    w4a[:, 64 : 64 + OUT_C] = w4[:, :, 3].T  # left
    w4a[:, 96 : 96 + OUT_C] = w4[:, :, 4].T  # right
    s6 = np.zeros((HID_C, OUT_C), np.float32)
    for base in (0, 6, 32, 64, 96):
        s6[base + np.arange(OUT_C), np.arange(OUT_C)] = 1.0
    s6 = s6.astype(ml_dtypes.bfloat16)
    sd = np.zeros((HID_C, 5, OUT_C), np.float32)
    for t, base in enumerate((0, 6, 32, 64, 96)):  # up,cen,dn,lf,rt slabs
        sd[base + np.arange(OUT_C), t, np.arange(OUT_C)] = 1.0
    sd = sd.astype(ml_dtypes.bfloat16)
    bias = np.zeros((HID_C, 4), np.float32)
    bias[:, 0] = np.asarray(b1, np.float32)
    bias[:, 1] = np.asarray(b2, np.float32)
    bias[:, 2] = np.asarray(b3, np.float32)
    bias[:OUT_C, 3] = np.asarray(b4, np.float32)
    common = {
        # w1p[t*6+ic, oc] = w1[oc, ic, t]
        "w1p": np.ascontiguousarray(w1.transpose(2, 1, 0).reshape(5 * IN_C, HID_C)),
        # w23p[ic, t, oc] = w2[oc, ic, t] (t<5) / w3[oc, ic, t-5]
        "w23p": np.ascontiguousarray(np.concatenate(
            [w2.transpose(1, 2, 0), w3.transpose(1, 2, 0)], axis=1)),
        "w4a": w4a,
        "s6": s6,
        "sd": sd,
        "bias": bias,
    }
    return x, common


def kernel(x, w1, b1, w2, b2, w3, b3, w4, b4):
    x, common = _pack_inputs(x, w1, b1, w2, b2, w3, b3, w4, b4)
    if "nc" not in _NC_CACHE:
        _NC_CACHE["nc"] = _build()
    nc = _NC_CACHE["nc"]
    in_maps = [dict(common, x=x[i]) for i in range(N_CORES)]
    res = bass_utils.run_bass_kernel_spmd(nc, in_maps, core_ids=list(range(N_CORES)))
    out = np.stack([res.results[i]["y"] for i in range(N_CORES)], axis=0)
    return out


# revision 69
# speedup vs baseline: 1.2350x; 1.0001x over previous
"""Trainium2 Bass kernel for 4-layer cross-stencil CNN.

Per-core: one image [6,256,256] (batch dim sharded across 8 cores).
conv(cross-5-stencil) = 5 channel-matmuls with spatially shifted rhs APs,
accumulated in PSUM. Channels on partitions, spatial (rows x cols) on the
free dim. fp32r matmuls (full PE rate at N>=256).

Rolling-retention pipeline: intermediate buffers h1/h2/h3/t5 are circular
row-slot buffers (S=28 slots, slot = row % 28). Each layer computes every
image row exactly once — no halo recompute. Per strip of R=24 rows the
layer blocks are skewed (L1 rows [a,a+24), L2 [a-2,a+22), L3 [a-4,a+20),
mmA [a-5,a+19), L4b [a-7,a+17)) so all reads hit rows already produced
this strip or retained from the previous one. Reads that cross the
circular wrap are split into <=2 accumulating matmuls; chunk boundaries
are chosen so writes never wrap. Image-boundary taps are handled by
narrowing the tap matmul to valid rows (no zero-pad rows needed).

L1 packs the 5 taps into K=30 via a 5-group pre-shifted x staging buffer
(one matmul per chunk), double-buffered across strips. L4 computes all 5
taps as one M=128 matmul (slabs at partitions 0/6/32/64/96), gathers the
shifted slabs into t5s by SBUF->SBUF DMA, and sums them with one K=128
selector matmul; the strip's tap-sum drains interleaved into the next
strip's L2 phase.
"""

import sys

sys.path.insert(0, "/opt/trn_rl_repo")

import ml_dtypes
import numpy as np

import concourse.bacc as bacc
import concourse.mybir as mybir
from concourse.tile import TileContext
from concourse import bass_utils

IN_C, HID_C, OUT_C = 6, 128, 6
B, H, W = 8, 256, 256
WP = W + 2  # padded width
R = 24  # rows per strip
DIRECT_TAIL = 4  # final rows computed gather-free from t5
S = 28  # circular row slots in h1/h2/h3/t5 (live windows are <= 27 rows)
XS = 28  # row slots in the x staging buffers
N_CORES = 8

f32 = mybir.dt.float32
f32r = mybir.dt.float32r
bf16 = mybir.dt.bfloat16
Add = mybir.AluOpType.add
Max = mybir.AluOpType.max
Relu = mybir.ActivationFunctionType.Relu
Ident = mybir.ActivationFunctionType.Identity

# tap order matches reference: 0=center, 1=up(x[h-1]), 2=down(x[h+1]),
# 3=left(x[w-1]), 4=right(x[w+1])


def _chunks(r0, r1):
    """2-row chunks of [r0,r1), never straddling a multiple of S."""
    r = r0
    while r < r1:
        n = min(2, r1 - r, (r // S + 1) * S - r)
        yield r, n
        r += n


def _runs(r0, r1):
    """Split [r0,r1) into runs of consecutive slots (break at S multiples)."""
    r = r0
    while r < r1:
        e = min(r1, (r // S + 1) * S)
        yield r, e
        r = e


def _build():
    nc = bacc.Bacc("TRN2", target_bir_lowering=False)

    x_d = nc.dram_tensor("x", [IN_C, H, W], f32, kind="ExternalInput")
    w1_d = nc.dram_tensor("w1p", [5 * IN_C, HID_C], f32, kind="ExternalInput")
    w23_d = nc.dram_tensor("w23p", [HID_C, 10, HID_C], f32, kind="ExternalInput")
    # w4a: all 5 taps as M=128 slabs: up@0-5, center@6-11, down@32-37,
    # left@64-69, right@96-101; zero elsewhere
    w4a_d = nc.dram_tensor("w4a", [HID_C, HID_C], f32, kind="ExternalInput")
    # s6: bf16 selector summing the 5 (pre-shifted) slabs of t5s
    s6_d = nc.dram_tensor("s6", [HID_C, OUT_C], bf16, kind="ExternalInput")
    # sd: per-slab selectors for the direct (gather-free) tail chunks
    sd_d = nc.dram_tensor("sd", [HID_C, 5, OUT_C], bf16, kind="ExternalInput")
    bias_d = nc.dram_tensor("bias", [HID_C, 4], f32, kind="ExternalInput")
    y_d = nc.dram_tensor("y", [OUT_C, H, W], f32, kind="ExternalOutput")

    strips = list(range(0, H, R))  # block anchors a = 24k

    with TileContext(nc) as tc:
        with (
            tc.tile_pool(name="const", bufs=1) as cpool,
            tc.tile_pool(name="bufs", bufs=1) as bpool,
            tc.tile_pool(name="io", bufs=2) as iopool,
            tc.tile_pool(name="psmain", bufs=7, space="PSUM") as pmain,
        ):
            # --- weights / biases (resident). HWDGE costs ~625ns per DMA
            # instruction and is a single serial device, so: w1 + strip-0 x
            # first (these gate the first matmul), everything else after,
            # with biases packed into one [128,4] tensor and w2+w3 into one
            # [128,10,128] tensor to cut the instruction count ---
            w1_sb = cpool.tile([5 * IN_C, HID_C], f32r)

            # --- persistent buffers ---
            # xst: 2 staging buffers, 5 tap-groups x 6ch, pre-shifted by DMA
            # placement. Strip k window: x rows [24k-2, 24k+26) at slot
            # r - 24k + 2 (center group); reading slot(r) in group g yields
            # x row r + dr_g at col w + dc_g.
            xst = [
                bpool.tile([5 * IN_C, XS, WP], f32r, name=f"xst{i}")
                for i in range(2)
            ]
            h1 = bpool.tile([HID_C, S, WP], f32r)
            h2 = bpool.tile([HID_C, S, WP], f32r)
            h3 = bpool.tile([HID_C, S, WP], f32r)
            # t5: tap partials (slabs up@0,cen@6,dn@32,lf@64,rt@96), bf16
            t5 = bpool.tile([HID_C, S, WP], bf16)
            # t5s: gathered pre-shifted taps; slot d = output row a5+d
            t5s = bpool.tile([HID_C, R, WP], bf16)

            # x staging pads, narrowed to exactly the cells that are read
            # but never DMA'd (disjoint from every placement, so the strip-0
            # DMAs carry no memset dependency): col 1 for the left group
            # (placed at cols [2,258)), col 256 for the right group (cols
            # [0,256)), and the up-group's slot 2 (x row -1) in buffer 0.
            # Cols 0/257 and other slots are never read.
            # (engine ops need partition base 0/32/64/96, so the ranges
            # start at 0; the extra cells are DMA-overwritten every strip)
            for xb in xst:
                nc.gpsimd.memset(xb[0:24, :, 1:2].bitcast(f32), 0.0)
                nc.gpsimd.memset(xb[0:30, :, 256:257].bitcast(f32), 0.0)
            nc.gpsimd.memset(xst[0][0:12, 2:3, :].bitcast(f32), 0.0)

            def x_dma(a, buf, split=False):
                """Load x rows [a-1, a+R+1) into staging buffer (5 shifted
                placements); center group slot = r - a + 2."""
                lo, hi = max(0, a - 1), min(H, a + R + 1)
                src = x_d[:, lo:hi, :].bitcast(f32r)
                o = lo - a  # >= -1
                # (partitions, slot offset of x row m = m - a + ofs, cols)
                for gi, (p0, ofs, c0) in enumerate((
                    (0, 2, 1),   # center
                    (6, 3, 1),   # up: read slot(r) -> x row r-1
                    (12, 1, 1),  # down
                    (18, 2, 2),  # left: read col w+1 -> x col w-1
                    (24, 2, 0),  # right
                )):
                    # split across the two DMA front-ends: HWDGE (sync) and
                    # SWDGE (gpsimd) process their ~625/1050ns-per-DMA fixed
                    # costs in parallel
                    eng = nc.gpsimd if split and gi >= 3 else nc.sync
                    eng.dma_start(
                        out=xst[buf][p0 : p0 + 6, o + ofs : hi - a + ofs, c0 : c0 + W],
                        in_=src,
                    )

            x_dma(0, 0)

            # w1 after the strip-0 placements: its 43ns transfer hides in
            # the HWDGE pipeline while the placements' 427ns transfers
            # overlap later HWDGE processing
            nc.sync.dma_start(out=w1_sb, in_=w1_d[:, :].bitcast(f32r))

            bias_sb = cpool.tile([HID_C, 4], f32)
            nc.sync.dma_start(out=bias_sb, in_=bias_d[:, :])
            b1_sb = bias_sb[:, 0:1]
            b2_sb = bias_sb[:, 1:2]
            b3_sb = bias_sb[:, 2:3]
            b4_sb = bias_sb[0:OUT_C, 3:4]
            w23_sb = cpool.tile([HID_C, 10, HID_C], f32r)
            nc.sync.dma_start(out=w23_sb, in_=w23_d[:, :, :].bitcast(f32r))
            w2_sb = w23_sb[:, 0:5, :]
            w3_sb = w23_sb[:, 5:10, :]
            w4a_sb = cpool.tile([HID_C, HID_C], f32r)
            nc.sync.dma_start(out=w4a_sb, in_=w4a_d[:, :].bitcast(f32r))
            s6_sb = cpool.tile([HID_C, OUT_C], bf16)
            nc.sync.dma_start(out=s6_sb, in_=s6_d[:, :])
            sd_sb = cpool.tile([HID_C, 5, OUT_C], bf16)
            nc.sync.dma_start(out=sd_sb, in_=sd_d[:, :, :])

            # h/t5 pads: cols 0 and 257 are read (left/right taps) but
            # never written; zero once — rolling writes only touch [1,257).
            for _t in (h1, h2, h3):
                nc.vector.memset(_t[:, :, 0:1].bitcast(f32), 0.0)
                nc.vector.memset(_t[:, :, 257:258].bitcast(f32), 0.0)
            nc.vector.memset(t5[:, :, 0:1], 0.0)
            nc.vector.memset(t5[:, :, 257:258], 0.0)
            # t5s non-slab partitions are read by the selector matmul
            # (0 * garbage must not be NaN) — zero everything once
            nc.vector.memset(t5s[:, :, :], 0.0)

            def conv_chunk(ps, w_sb, src, r0, n):
                """Cross-stencil accumulation for output rows [r0, r0+n).
                Center tap first (covers the whole psum tile -> start=True
                zeroes it); up/down taps clamp to [0,H) and split at the
                circular wrap."""
                parts = []  # (tap, psum_off, rows, slot)
                for tap, dr in ((1, -1), (2, 1), (3, 0), (4, 0)):
                    lo = max(0, r0 + dr)
                    hi = min(H, r0 + n + dr)
                    for ra, rb in _runs(lo, hi):
                        parts.append((tap, ra - dr - r0, rb - ra, ra % S))
                cols = {0: (1, 1 + W), 1: (1, 1 + W), 2: (1, 1 + W),
                        3: (0, W), 4: (2, 2 + W)}
                nc.tensor.matmul(
                    ps, w_sb[:, 0, :], src[:, r0 % S : r0 % S + n, 1 : 1 + W],
                    start=True, stop=False,
                )
                for i, (tap, off, m, sl) in enumerate(parts):
                    c0, c1 = cols[tap]
                    nc.tensor.matmul(
                        ps[:, off : off + m, :],
                        w_sb[:, tap, :],
                        src[:, sl : sl + m, c0:c1],
                        start=False, stop=(i == len(parts) - 1),
                    )

            # slab parts / tap row-offset / col offsets (dst, src)
            GAT = ((0, -1, 1, 1), (6, 0, 1, 1), (32, 1, 1, 1),
                   (64, 0, 2, 1), (96, 0, 1, 2))

            def emit_gather(a5, d0, d1, split=False):
                """Gather t5 slab rows into pre-shifted t5s slots [d0,d1).
                With split=True alternate slabs between the HWDGE (sync) and
                SWDGE (gpsimd) paths so the two DMA front-ends overlap."""
                for gi, (p0, dr, cd, cs) in enumerate(GAT):
                    eng = nc.gpsimd if split and gi % 2 else nc.sync
                    lo = max(0, a5 + d0 + dr)
                    hi = min(H, a5 + d1 + dr)
                    for ra, rb in _runs(lo, hi):
                        dd = ra - dr - a5
                        m = rb - ra
                        eng.dma_start(
                            out=t5s[p0 : p0 + 6, dd : dd + m, cd : cd + W],
                            in_=t5[p0 : p0 + 6, ra % S : ra % S + m, cs : cs + W],
                        )

            def l4b_chunks(a5, b5, tail=False):
                """Deferred emitters: one K=128 selector matmul per chunk
                sums the 5 pre-shifted slabs of t5s; bias on DVE into a
                strip-wide staging tile; two half-strip DMAs out."""
                nr = b5 - a5
                chunks = list(_chunks(a5, b5))
                # split into flush groups, each with its own staging tile
                nparts = 4 if tail else 2
                halves = []
                prev_i = 0
                for j in range(1, nparts + 1):
                    tgt = nr * j // nparts
                    i = next((i for i, (rr, n) in enumerate(chunks)
                              if rr + n - a5 >= tgt), len(chunks) - 1)
                    if i + 1 > prev_i:
                        halves.append(chunks[prev_i : i + 1])
                        prev_i = i + 1
                out = []  # (needed t5s rows, emitter) pairs
                dt0 = b5 - DIRECT_TAIL if tail else b5
                for half in halves:
                    if not half:
                        continue
                    lo = half[0][0] - a5
                    hi = half[-1][0] + half[-1][1] - a5
                    yst = iopool.tile([OUT_C, hi - lo, W], f32, tag="yst")
                    for ei, (rr, n) in enumerate(half):

                        def emit(rr=rr, n=n, a5=a5, yst=yst, lo=lo, ei=ei,
                                 tail=tail):
                            d = rr - a5
                            ps = pmain.tile([OUT_C, n, W], f32, tag="ps")
                            if tail and rr >= a5 + (dt0 - a5):
                                # gather-free: per-slab selectors read t5
                                # directly with the slab's row/col shift
                                parts = []
                                for tp, dr, c0 in ((0, -1, 1), (2, 1, 1),
                                                   (3, 0, 0), (4, 0, 2)):
                                    pl = max(0, rr + dr)
                                    ph = min(H, rr + n + dr)
                                    for ra, rb in _runs(pl, ph):
                                        parts.append(
                                            (tp, ra - dr - rr, rb - ra,
                                             ra % S, c0))
                                nc.tensor.matmul(
                                    ps, sd_sb[:, 1, :],
                                    t5[:, rr % S : rr % S + n, 1 : 1 + W],
                                    start=True, stop=False,
                                )
                                for i, (tp, off, m, sl, c0) in enumerate(parts):
                                    nc.tensor.matmul(
                                        ps[:, off : off + m, :], sd_sb[:, tp, :],
                                        t5[:, sl : sl + m, c0 : c0 + W],
                                        start=False, stop=(i == len(parts) - 1),
                                    )
                            else:
                                nc.tensor.matmul(
                                    ps, s6_sb[:, :], t5s[:, d : d + n, 1 : 1 + W],
                                    start=True, stop=True,
                                )
                            if tail and ei % 2:
                                # ACT is idle at the drain; alternating
                                # halves the serialized add chain
                                nc.scalar.activation(
                                    yst[:, d - lo : d - lo + n, :], ps,
                                    Ident, bias=b4_sb)
                            else:
                                nc.vector.tensor_scalar_add(
                                    yst[:, d - lo : d - lo + n, :], ps, b4_sb)

                        need = (min(rr + n - a5, dt0 - a5) if tail
                                else rr + n - a5)
                        out.append((need, emit))

                    def flush(lo=lo, hi=hi, a5=a5, yst=yst):
                        nc.sync.dma_start(
                            out=y_d[:, a5 + lo : a5 + hi, :],
                            in_=yst[:, :, :])

                    out.append((hi, flush))
                return out

            pending = []  # tap-sum emitters from the previous strip
            mma_carry = []  # end-of-strip mmA chunks deferred into next L1
            gather_pend = None  # deferred gather + l4b creation
            for k, a in enumerate(strips):
                last = k == len(strips) - 1
                # skewed layer blocks for this strip
                a1, b1r = a, min(H, a + R)
                a2, b2r = max(0, a - 2), min(H, a + R - 2)
                a3, b3r = max(0, a - 4), min(H, a + R - 4)
                a4, b4r = max(0, a - 5), min(H, a + R - 5)
                a5, b5r = max(0, a - 7), min(H, a + R - 7)
                if last:
                    b2r = b3r = b4r = b5r = H

                # prefetch next strip's x into the other staging buffer
                if not last:
                    a_n = a + R
                    if a_n + R >= H:
                        # final strip: slots past the image tail keep stale
                        # data from two strips ago — zero them before the DMA
                        tail0 = min(H, a_n + R + 1) - a_n + 1
                        nc.gpsimd.memset(
                            xst[(k + 1) % 2][:, tail0:XS, :].bitcast(f32), 0.0)
                    x_dma(a_n, (k + 1) % 2)

                # --- L1: one K=30 matmul per chunk; copies alternate
                # DVE/ACT ---
                xb = xst[k % 2]
                ci = 0
                for rr, n in _chunks(a1, b1r):
                    ps = pmain.tile([HID_C, n, W], f32, tag="ps")
                    s0 = rr - a + 2
                    nc.tensor.matmul(
                        ps, w1_sb[:, :], xb[:, s0 : s0 + n, 1 : 1 + W],
                        start=True, stop=True,
                    )
                    sl = rr % S
                    if ci % 2 == 0:
                        nc.vector.tensor_scalar(
                            h1[:, sl : sl + n, 1 : 1 + W], ps, b1_sb, 0.0, Add, Max
                        )
                    else:
                        nc.scalar.activation(
                            h1[:, sl : sl + n, 1 : 1 + W], ps, Relu, bias=b1_sb
                        )
                    ci += 1
                    if mma_carry and ci >= 4:
                        mma_carry.pop(0)()
                while mma_carry:
                    mma_carry.pop(0)()
                if gather_pend is not None:
                    pending = gather_pend()
                    gather_pend = None

                # --- L2: reads h1; interleave prev strip's tap-sum ---
                for rr, n in _chunks(a2, b2r):
                    ps = pmain.tile([HID_C, n, W], f32, tag="ps")
                    conv_chunk(ps, w2_sb, h1, rr, n)
                    sl = rr % S
                    nc.scalar.activation(
                        h2[:, sl : sl + n, 1 : 1 + W], ps, Relu, bias=b2_sb
                    )
                    if pending:
                        pending.pop(0)[1]()
                while pending:
                    pending.pop(0)[1]()

                # --- L3: reads h2; mmA interleaved so t5 fills (and the
                # gather can start) as h3 rows land ---
                mma_q = list(_chunks(a4, b4r))

                def emit_mma(rr, n, ci):
                    ps = pmain.tile([HID_C, n, W], f32, tag="ps")
                    sl = rr % S
                    nc.tensor.matmul(
                        ps, w4a_sb[:, :], h3[:, sl : sl + n, 1 : 1 + W],
                        start=True, stop=True,
                    )
                    if ci % 2 == 0:
                        nc.vector.tensor_copy(t5[:, sl : sl + n, 1 : 1 + W], ps)
                    else:
                        nc.scalar.activation(
                            t5[:, sl : sl + n, 1 : 1 + W], ps, Ident)

                if k == 0:
                    # row 0 up-slab source is row -1: zero that dest
                    nc.gpsimd.memset(t5s[0:6, 0:1, :], 0.0)
                if last:
                    # row 255 down-slab source is row 256
                    nc.gpsimd.memset(t5s[32:38, b5r - 1 - a5 : b5r - a5, :], 0.0)

                if last:
                    # final strip: interleave gather / tap-sum into the L3
                    # drain so the tail doesn't serialize; y flushes (whose
                    # deps resolve last) go at the very end. Selector pops
                    # lag one gather batch so they never block the in-order
                    # PE queue while their gather DMAs are still in flight.
                    allp = l4b_chunks(a5, b5r, tail=True)
                    chunks_f = [p for p in allp if p[1].__name__ == "emit"]
                    flushes = [p for p in allp if p[1].__name__ == "flush"]
                    gathered = 0
                    gprev = 0
                    gage = 0  # tail_steps since the last gather batch
                    nrl = b5r - a5

                def tail_step(final=False):
                    nonlocal gathered, gprev, gage
                    nrg = nrl - DIRECT_TAIL
                    ready = (nrg if not mma_q else
                             max(0, min(nrg, mma_q[0][0] - 1 - a5)))
                    if ready - gathered >= 12 or (ready == nrg and ready > gathered):
                        gprev = gathered
                        emit_gather(a5, gathered, ready, split=True)
                        gathered = ready
                        gage = 0
                    gage += 1
                    # a gather's DMAs are safely in flight after ~2 more L3/
                    # mmA chunks of PE work; then its selectors won't block
                    # the in-order PE queue
                    lim = gathered if (final and gathered == nrl - DIRECT_TAIL) \
                        or gage > 5 else gprev
                    while chunks_f and chunks_f[0][0] <= lim:
                        chunks_f.pop(0)[1]()

                ci4 = 0
                for rr, n in _chunks(a3, b3r):
                    ps = pmain.tile([HID_C, n, W], f32, tag="ps")
                    conv_chunk(ps, w3_sb, h2, rr, n)
                    sl = rr % S
                    nc.scalar.activation(
                        h3[:, sl : sl + n, 1 : 1 + W], ps, Relu, bias=b3_sb
                    )
                    rr_next = rr + n
                    while mma_q and mma_q[0][0] + mma_q[0][1] + 2 <= rr_next:
                        m0, n0 = mma_q.pop(0)
                        emit_mma(m0, n0, ci4)
                        ci4 += 1
                    if last:
                        tail_step()

                while last and mma_q:
                    m0, n0 = mma_q.pop(0)
                    emit_mma(m0, n0, ci4)
                    ci4 += 1
                    tail_step()

                if not last:
                    # defer the mmA stragglers into the next strip's L1
                    # phase (their ACT/DVE copy deps get ~5us of slack) and
                    # the gather + tap-sum creation right after them
                    for m0, n0 in mma_q:
                        mma_carry.append(
                            lambda m0=m0, n0=n0, c=ci4: emit_mma(m0, n0, c))
                        ci4 += 1
                    mma_q.clear()

                    def gather_pend(a5=a5, b5=b5r):
                        nrg = b5 - a5
                        emit_gather(a5, 0, nrg // 2)
                        emit_gather(a5, nrg // 2, nrg)
                        return l4b_chunks(a5, b5)
                else:
                    while chunks_f or gathered < nrl - DIRECT_TAIL:
                        tail_step(final=True)
                    pending = flushes

            # flush any remaining tap-sum emitters
            while pending:
                pending.pop(0)[1]()

    nc.finalize()
    return nc


_NC_CACHE = {}


def _pack_inputs(x, w1, b1, w2, b2, w3, b3, w4, b4):
    x = np.ascontiguousarray(np.asarray(x, dtype=np.float32))
    w1 = np.asarray(w1, dtype=np.float32)
    w2 = np.asarray(w2, dtype=np.float32)
    w3 = np.asarray(w3, dtype=np.float32)
    w4 = np.asarray(w4, dtype=np.float32)
    # w4a slabs: up@0-5, center@6-11, down@32-37, left@64-69, right@96-101
    w4a = np.zeros((HID_C, HID_C), np.float32)
    w4a[:, 0:OUT_C] = w4[:, :, 1].T          # up
    w4a[:, 6 : 6 + OUT_C] = w4[:, :, 0].T    # center
    w4a[:, 32 : 32 + OUT_C] = w4[:, :, 2].T  # down
    w4a[:, 64 : 64 + OUT_C] = w4[:, :, 3].T  # left
    w4a[:, 96 : 96 + OUT_C] = w4[:, :, 4].T  # right
    s6 = np.zeros((HID_C, OUT_C), np.float32)
    for base in (0, 6, 32, 64, 96):
        s6[base + np.arange(OUT_C), np.arange(OUT_C)] = 1.0
    s6 = s6.astype(ml_dtypes.bfloat16)
    sd = np.zeros((HID_C, 5, OUT_C), np.float32)
    for t, base in enumerate((0, 6, 32, 64, 96)):  # up,cen,dn,lf,rt slabs
        sd[base + np.arange(OUT_C), t, np.arange(OUT_C)] = 1.0
    sd = sd.astype(ml_dtypes.bfloat16)
    bias = np.zeros((HID_C, 4), np.float32)
    bias[:, 0] = np.asarray(b1, np.float32)
    bias[:, 1] = np.asarray(b2, np.float32)
    bias[:, 2] = np.asarray(b3, np.float32)
    bias[:OUT_C, 3] = np.asarray(b4, np.float32)
    common = {
        # w1p[t*6+ic, oc] = w1[oc, ic, t]
        "w1p": np.ascontiguousarray(w1.transpose(2, 1, 0).reshape(5 * IN_C, HID_C)),
        # w23p[ic, t, oc] = w2[oc, ic, t] (t<5) / w3[oc, ic, t-5]
        "w23p": np.ascontiguousarray(np.concatenate(
            [w2.transpose(1, 2, 0), w3.transpose(1, 2, 0)], axis=1)),
        "w4a": w4a,
        "s6": s6,
        "sd": sd,
        "bias": bias,
    }
    return x, common


def kernel(x, w1, b1, w2, b2, w3, b3, w4, b4):
    x, common = _pack_inputs(x, w1, b1, w2, b2, w3, b3, w4, b4)
    if "nc" not in _NC_CACHE:
        _NC_CACHE["nc"] = _build()
    nc = _NC_CACHE["nc"]
    in_maps = [dict(common, x=x[i]) for i in range(N_CORES)]
    res = bass_utils.run_bass_kernel_spmd(nc, in_maps, core_ids=list(range(N_CORES)))
    out = np.stack([res.results[i]["y"] for i in range(N_CORES)], axis=0)
    return out


# revision 75
# speedup vs baseline: 1.2372x; 1.0018x over previous
"""Trainium2 Bass kernel for 4-layer cross-stencil CNN.

Per-core: one image [6,256,256] (batch dim sharded across 8 cores).
conv(cross-5-stencil) = 5 channel-matmuls with spatially shifted rhs APs,
accumulated in PSUM. Channels on partitions, spatial (rows x cols) on the
free dim. fp32r matmuls (full PE rate at N>=256).

Rolling-retention pipeline: intermediate buffers h1/h2/h3/t5 are circular
row-slot buffers (S=28 slots, slot = row % 28). Each layer computes every
image row exactly once — no halo recompute. Per strip of R=24 rows the
layer blocks are skewed (L1 rows [a,a+24), L2 [a-2,a+22), L3 [a-4,a+20),
mmA [a-5,a+19), L4b [a-7,a+17)) so all reads hit rows already produced
this strip or retained from the previous one. Reads that cross the
circular wrap are split into <=2 accumulating matmuls; chunk boundaries
are chosen so writes never wrap. Image-boundary taps are handled by
narrowing the tap matmul to valid rows (no zero-pad rows needed).

L1 packs the 5 taps into K=30 via a 5-group pre-shifted x staging buffer
(one matmul per chunk), double-buffered across strips. L4 computes all 5
taps as one M=128 matmul (slabs at partitions 0/6/32/64/96), gathers the
shifted slabs into t5s by SBUF->SBUF DMA, and sums them with one K=128
selector matmul; the strip's tap-sum drains interleaved into the next
strip's L2 phase.
"""

import sys

sys.path.insert(0, "/opt/trn_rl_repo")

import ml_dtypes
import numpy as np

import concourse.bacc as bacc
import concourse.mybir as mybir
from concourse.tile import TileContext
from concourse import bass_utils

IN_C, HID_C, OUT_C = 6, 128, 6
B, H, W = 8, 256, 256
WP = W + 2  # padded width
R = 24  # rows per strip
DIRECT_TAIL = 4  # final rows computed gather-free from t5
S = 28  # circular row slots in h1/h2/h3/t5 (live windows are <= 27 rows)
XS = 28  # row slots in the x staging buffers
N_CORES = 8

f32 = mybir.dt.float32
f32r = mybir.dt.float32r
bf16 = mybir.dt.bfloat16
Add = mybir.AluOpType.add
Max = mybir.AluOpType.max
Relu = mybir.ActivationFunctionType.Relu
Ident = mybir.ActivationFunctionType.Identity

# tap order matches reference: 0=center, 1=up(x[h-1]), 2=down(x[h+1]),
# 3=left(x[w-1]), 4=right(x[w+1])


def _chunks(r0, r1):
    """2-row chunks of [r0,r1), never straddling a multiple of S."""
    r = r0
    while r < r1:
        n = min(2, r1 - r, (r // S + 1) * S - r)
        yield r, n
        r += n


def _runs(r0, r1):
    """Split [r0,r1) into runs of consecutive slots (break at S multiples)."""
    r = r0
    while r < r1:
        e = min(r1, (r // S + 1) * S)
        yield r, e
        r = e


def _build():
    nc = bacc.Bacc("TRN2", target_bir_lowering=False)

    x_d = nc.dram_tensor("x", [IN_C, H, W], f32, kind="ExternalInput")
    w1_d = nc.dram_tensor("w1p", [5 * IN_C, HID_C], f32, kind="ExternalInput")
    w23_d = nc.dram_tensor("w23p", [HID_C, 10, HID_C], f32, kind="ExternalInput")
    # w4a: all 5 taps as M=128 slabs: up@0-5, center@6-11, down@32-37,
    # left@64-69, right@96-101; zero elsewhere
    w4a_d = nc.dram_tensor("w4a", [HID_C, HID_C], f32, kind="ExternalInput")
    # s6: bf16 selector summing the 5 (pre-shifted) slabs of t5s
    s6_d = nc.dram_tensor("s6", [HID_C, OUT_C], bf16, kind="ExternalInput")
    # sd: per-slab selectors for the direct (gather-free) tail chunks
    sd_d = nc.dram_tensor("sd", [HID_C, 5, OUT_C], bf16, kind="ExternalInput")
    bias_d = nc.dram_tensor("bias", [HID_C, 4], f32, kind="ExternalInput")
    y_d = nc.dram_tensor("y", [OUT_C, H, W], f32, kind="ExternalOutput")

    strips = list(range(0, H, R))  # block anchors a = 24k

    with TileContext(nc) as tc:
        with (
            tc.tile_pool(name="const", bufs=1) as cpool,
            tc.tile_pool(name="bufs", bufs=1) as bpool,
            tc.tile_pool(name="io", bufs=2) as iopool,
            tc.tile_pool(name="psmain", bufs=7, space="PSUM") as pmain,
        ):
            # --- weights / biases (resident). HWDGE costs ~625ns per DMA
            # instruction and is a single serial device, so: w1 + strip-0 x
            # first (these gate the first matmul), everything else after,
            # with biases packed into one [128,4] tensor and w2+w3 into one
            # [128,10,128] tensor to cut the instruction count ---
            w1_sb = cpool.tile([5 * IN_C, HID_C], f32r)

            # --- persistent buffers ---
            # xst: 2 staging buffers, 5 tap-groups x 6ch, pre-shifted by DMA
            # placement. Strip k window: x rows [24k-2, 24k+26) at slot
            # r - 24k + 2 (center group); reading slot(r) in group g yields
            # x row r + dr_g at col w + dc_g.
            xst = [
                bpool.tile([5 * IN_C, XS, WP], f32r, name=f"xst{i}")
                for i in range(2)
            ]
            h1 = bpool.tile([HID_C, S, WP], f32r)
            h2 = bpool.tile([HID_C, S, WP], f32r)
            h3 = bpool.tile([HID_C, S, WP], f32r)
            # t5: tap partials (slabs up@0,cen@6,dn@32,lf@64,rt@96), bf16
            t5 = bpool.tile([HID_C, S, WP], bf16)
            # t5s: gathered pre-shifted taps; slot d = output row a5+d
            t5s = bpool.tile([HID_C, R, WP], bf16)

            # x staging pads, narrowed to exactly the cells that are read
            # but never DMA'd (disjoint from every placement, so the strip-0
            # DMAs carry no memset dependency): col 1 for the left group
            # (placed at cols [2,258)), col 256 for the right group (cols
            # [0,256)), and the up-group's slot 2 (x row -1) in buffer 0.
            # Cols 0/257 and other slots are never read.
            # (engine ops need partition base 0/32/64/96, so the ranges
            # start at 0; the extra cells are DMA-overwritten every strip)
            for xb in xst:
                nc.gpsimd.memset(xb[0:24, :, 1:2].bitcast(f32), 0.0)
                nc.gpsimd.memset(xb[0:30, :, 256:257].bitcast(f32), 0.0)
            nc.gpsimd.memset(xst[0][0:12, 2:3, :].bitcast(f32), 0.0)

            def x_dma(a, buf, split=False):
                """Load x rows [a-1, a+R+1) into staging buffer (5 shifted
                placements); center group slot = r - a + 2."""
                lo, hi = max(0, a - 1), min(H, a + R + 1)
                src = x_d[:, lo:hi, :].bitcast(f32r)
                o = lo - a  # >= -1
                # (partitions, slot offset of x row m = m - a + ofs, cols)
                for gi, (p0, ofs, c0) in enumerate((
                    (0, 2, 1),   # center
                    (6, 3, 1),   # up: read slot(r) -> x row r-1
                    (12, 1, 1),  # down
                    (18, 2, 2),  # left: read col w+1 -> x col w-1
                    (24, 2, 0),  # right
                )):
                    # split across the two DMA front-ends: HWDGE (sync) and
                    # SWDGE (gpsimd) process their ~625/1050ns-per-DMA fixed
                    # costs in parallel
                    eng = nc.gpsimd if split and gi >= 3 else nc.sync
                    eng.dma_start(
                        out=xst[buf][p0 : p0 + 6, o + ofs : hi - a + ofs, c0 : c0 + W],
                        in_=src,
                    )

            x_dma(0, 0)

            # p-state priming: the PE idles ~6us while the strip-0 x
            # placements stage, and its clock ramps 0.65->1.2->2.4GHz over
            # the first 3us of busy time. Dependency-free matmuls on a
            # never-written scratch tile fill the idle window so the real
            # work starts at full clock. Their psum is never read.
            dummy = bpool.tile([HID_C, 2, W], f32r)
            nc.gpsimd.memset(dummy[:, 0:1, :].bitcast(f32), 0.0)
            for _ in range(12):
                psd = pmain.tile([HID_C, 1, W], f32, tag="ps")
                nc.tensor.matmul(
                    psd, dummy[:, 0, 0:HID_C], dummy[:, 0:1, :],
                    start=True, stop=True,
                )

            # w1 after the strip-0 placements: its 43ns transfer hides in
            # the HWDGE pipeline while the placements' 427ns transfers
            # overlap later HWDGE processing
            nc.sync.dma_start(out=w1_sb, in_=w1_d[:, :].bitcast(f32r))

            bias_sb = cpool.tile([HID_C, 4], f32)
            nc.sync.dma_start(out=bias_sb, in_=bias_d[:, :])
            b1_sb = bias_sb[:, 0:1]
            b2_sb = bias_sb[:, 1:2]
            b3_sb = bias_sb[:, 2:3]
            b4_sb = bias_sb[0:OUT_C, 3:4]
            w23_sb = cpool.tile([HID_C, 10, HID_C], f32r)
            nc.sync.dma_start(out=w23_sb, in_=w23_d[:, :, :].bitcast(f32r))
            w2_sb = w23_sb[:, 0:5, :]
            w3_sb = w23_sb[:, 5:10, :]
            w4a_sb = cpool.tile([HID_C, HID_C], f32r)
            nc.sync.dma_start(out=w4a_sb, in_=w4a_d[:, :].bitcast(f32r))
            s6_sb = cpool.tile([HID_C, OUT_C], bf16)
            nc.sync.dma_start(out=s6_sb, in_=s6_d[:, :])
            sd_sb = cpool.tile([HID_C, 5, OUT_C], bf16)
            nc.sync.dma_start(out=sd_sb, in_=sd_d[:, :, :])

            # h/t5 pads: cols 0 and 257 are read (left/right taps) but
            # never written; zero once — rolling writes only touch [1,257).
            for _t in (h1, h2, h3):
                nc.vector.memset(_t[:, :, 0:1].bitcast(f32), 0.0)
                nc.vector.memset(_t[:, :, 257:258].bitcast(f32), 0.0)
            nc.vector.memset(t5[:, :, 0:1], 0.0)
            nc.vector.memset(t5[:, :, 257:258], 0.0)
            # t5s non-slab partitions are read by the selector matmul
            # (0 * garbage must not be NaN) — zero everything once
            nc.vector.memset(t5s[:, :, :], 0.0)

            def conv_chunk(ps, w_sb, src, r0, n):
                """Cross-stencil accumulation for output rows [r0, r0+n).
                Center tap first (covers the whole psum tile -> start=True
                zeroes it); up/down taps clamp to [0,H) and split at the
                circular wrap."""
                parts = []  # (tap, psum_off, rows, slot)
                for tap, dr in ((1, -1), (2, 1), (3, 0), (4, 0)):
                    lo = max(0, r0 + dr)
                    hi = min(H, r0 + n + dr)
                    for ra, rb in _runs(lo, hi):
                        parts.append((tap, ra - dr - r0, rb - ra, ra % S))
                cols = {0: (1, 1 + W), 1: (1, 1 + W), 2: (1, 1 + W),
                        3: (0, W), 4: (2, 2 + W)}
                nc.tensor.matmul(
                    ps, w_sb[:, 0, :], src[:, r0 % S : r0 % S + n, 1 : 1 + W],
                    start=True, stop=False,
                )
                for i, (tap, off, m, sl) in enumerate(parts):
                    c0, c1 = cols[tap]
                    nc.tensor.matmul(
                        ps[:, off : off + m, :],
                        w_sb[:, tap, :],
                        src[:, sl : sl + m, c0:c1],
                        start=False, stop=(i == len(parts) - 1),
                    )

            # slab parts / tap row-offset / col offsets (dst, src)
            GAT = ((0, -1, 1, 1), (6, 0, 1, 1), (32, 1, 1, 1),
                   (64, 0, 2, 1), (96, 0, 1, 2))

            def emit_gather(a5, d0, d1, split=False):
                """Gather t5 slab rows into pre-shifted t5s slots [d0,d1).
                With split=True alternate slabs between the HWDGE (sync) and
                SWDGE (gpsimd) paths so the two DMA front-ends overlap."""
                for gi, (p0, dr, cd, cs) in enumerate(GAT):
                    eng = nc.gpsimd if split and gi % 2 else nc.sync
                    lo = max(0, a5 + d0 + dr)
                    hi = min(H, a5 + d1 + dr)
                    for ra, rb in _runs(lo, hi):
                        dd = ra - dr - a5
                        m = rb - ra
                        eng.dma_start(
                            out=t5s[p0 : p0 + 6, dd : dd + m, cd : cd + W],
                            in_=t5[p0 : p0 + 6, ra % S : ra % S + m, cs : cs + W],
                        )

            def l4b_chunks(a5, b5, tail=False):
                """Deferred emitters: one K=128 selector matmul per chunk
                sums the 5 pre-shifted slabs of t5s; bias on DVE into a
                strip-wide staging tile; two half-strip DMAs out."""
                nr = b5 - a5
                chunks = list(_chunks(a5, b5))
                # split into flush groups, each with its own staging tile
                nparts = 4 if tail else 2
                halves = []
                prev_i = 0
                for j in range(1, nparts + 1):
                    tgt = nr * j // nparts
                    i = next((i for i, (rr, n) in enumerate(chunks)
                              if rr + n - a5 >= tgt), len(chunks) - 1)
                    if i + 1 > prev_i:
                        halves.append(chunks[prev_i : i + 1])
                        prev_i = i + 1
                out = []  # (needed t5s rows, emitter) pairs
                dt0 = b5 - DIRECT_TAIL if tail else b5
                for half in halves:
                    if not half:
                        continue
                    lo = half[0][0] - a5
                    hi = half[-1][0] + half[-1][1] - a5
                    yst = iopool.tile([OUT_C, hi - lo, W], f32, tag="yst")
                    for ei, (rr, n) in enumerate(half):

                        def emit(rr=rr, n=n, a5=a5, yst=yst, lo=lo, ei=ei,
                                 tail=tail):
                            d = rr - a5
                            ps = pmain.tile([OUT_C, n, W], f32, tag="ps")
                            if tail and rr >= a5 + (dt0 - a5):
                                # gather-free: per-slab selectors read t5
                                # directly with the slab's row/col shift
                                parts = []
                                for tp, dr, c0 in ((0, -1, 1), (2, 1, 1),
                                                   (3, 0, 0), (4, 0, 2)):
                                    pl = max(0, rr + dr)
                                    ph = min(H, rr + n + dr)
                                    for ra, rb in _runs(pl, ph):
                                        parts.append(
                                            (tp, ra - dr - rr, rb - ra,
                                             ra % S, c0))
                                nc.tensor.matmul(
                                    ps, sd_sb[:, 1, :],
                                    t5[:, rr % S : rr % S + n, 1 : 1 + W],
                                    start=True, stop=False,
                                )
                                for i, (tp, off, m, sl, c0) in enumerate(parts):
                                    nc.tensor.matmul(
                                        ps[:, off : off + m, :], sd_sb[:, tp, :],
                                        t5[:, sl : sl + m, c0 : c0 + W],
                                        start=False, stop=(i == len(parts) - 1),
                                    )
                            else:
                                nc.tensor.matmul(
                                    ps, s6_sb[:, :], t5s[:, d : d + n, 1 : 1 + W],
                                    start=True, stop=True,
                                )
                            if tail and ei % 2:
                                # ACT is idle at the drain; alternating
                                # halves the serialized add chain
                                nc.scalar.activation(
                                    yst[:, d - lo : d - lo + n, :], ps,
                                    Ident, bias=b4_sb)
                            else:
                                nc.vector.tensor_scalar_add(
                                    yst[:, d - lo : d - lo + n, :], ps, b4_sb)

                        need = (min(rr + n - a5, dt0 - a5) if tail
                                else rr + n - a5)
                        out.append((need, emit))

                    def flush(lo=lo, hi=hi, a5=a5, yst=yst):
                        nc.sync.dma_start(
                            out=y_d[:, a5 + lo : a5 + hi, :],
                            in_=yst[:, :, :])

                    out.append((hi, flush))
                return out

            pending = []  # tap-sum emitters from the previous strip
            mma_carry = []  # end-of-strip mmA chunks deferred into next L1
            gather_pend = None  # deferred gather + l4b creation
            for k, a in enumerate(strips):
                last = k == len(strips) - 1
                # skewed layer blocks for this strip
                a1, b1r = a, min(H, a + R)
                a2, b2r = max(0, a - 2), min(H, a + R - 2)
                a3, b3r = max(0, a - 4), min(H, a + R - 4)
                a4, b4r = max(0, a - 5), min(H, a + R - 5)
                a5, b5r = max(0, a - 7), min(H, a + R - 7)
                if last:
                    b2r = b3r = b4r = b5r = H

                # prefetch next strip's x into the other staging buffer
                if not last:
                    a_n = a + R
                    if a_n + R >= H:
                        # final strip: slots past the image tail keep stale
                        # data from two strips ago — zero them before the DMA
                        tail0 = min(H, a_n + R + 1) - a_n + 1
                        nc.gpsimd.memset(
                            xst[(k + 1) % 2][:, tail0:XS, :].bitcast(f32), 0.0)
                    x_dma(a_n, (k + 1) % 2)

                # --- L1: one K=30 matmul per chunk; copies alternate
                # DVE/ACT ---
                xb = xst[k % 2]
                ci = 0
                for rr, n in _chunks(a1, b1r):
                    ps = pmain.tile([HID_C, n, W], f32, tag="ps")
                    s0 = rr - a + 2
                    nc.tensor.matmul(
                        ps, w1_sb[:, :], xb[:, s0 : s0 + n, 1 : 1 + W],
                        start=True, stop=True,
                    )
                    sl = rr % S
                    if ci % 2 == 0:
                        nc.vector.tensor_scalar(
                            h1[:, sl : sl + n, 1 : 1 + W], ps, b1_sb, 0.0, Add, Max
                        )
                    else:
                        nc.scalar.activation(
                            h1[:, sl : sl + n, 1 : 1 + W], ps, Relu, bias=b1_sb
                        )
                    ci += 1
                    if mma_carry and ci >= 4:
                        mma_carry.pop(0)()
                while mma_carry:
                    mma_carry.pop(0)()
                if gather_pend is not None:
                    pending = gather_pend()
                    gather_pend = None

                # --- L2: reads h1; interleave prev strip's tap-sum ---
                for rr, n in _chunks(a2, b2r):
                    ps = pmain.tile([HID_C, n, W], f32, tag="ps")
                    conv_chunk(ps, w2_sb, h1, rr, n)
                    sl = rr % S
                    nc.scalar.activation(
                        h2[:, sl : sl + n, 1 : 1 + W], ps, Relu, bias=b2_sb
                    )
                    if pending:
                        pending.pop(0)[1]()
                while pending:
                    pending.pop(0)[1]()

                # --- L3: reads h2; mmA interleaved so t5 fills (and the
                # gather can start) as h3 rows land ---
                mma_q = list(_chunks(a4, b4r))

                def emit_mma(rr, n, ci):
                    ps = pmain.tile([HID_C, n, W], f32, tag="ps")
                    sl = rr % S
                    nc.tensor.matmul(
                        ps, w4a_sb[:, :], h3[:, sl : sl + n, 1 : 1 + W],
                        start=True, stop=True,
                    )
                    if ci % 2 == 0:
                        nc.vector.tensor_copy(t5[:, sl : sl + n, 1 : 1 + W], ps)
                    else:
                        nc.scalar.activation(
                            t5[:, sl : sl + n, 1 : 1 + W], ps, Ident)

                if k == 0:
                    # row 0 up-slab source is row -1: zero that dest
                    nc.gpsimd.memset(t5s[0:6, 0:1, :], 0.0)
                if last:
                    # row 255 down-slab source is row 256
                    nc.gpsimd.memset(t5s[32:38, b5r - 1 - a5 : b5r - a5, :], 0.0)

                if last:
                    # final strip: interleave gather / tap-sum into the L3
                    # drain so the tail doesn't serialize; y flushes (whose
                    # deps resolve last) go at the very end. Selector pops
                    # lag one gather batch so they never block the in-order
                    # PE queue while their gather DMAs are still in flight.
                    allp = l4b_chunks(a5, b5r, tail=True)
                    chunks_f = [p for p in allp if p[1].__name__ == "emit"]
                    flushes = [p for p in allp if p[1].__name__ == "flush"]
                    gathered = 0
                    gprev = 0
                    gage = 0  # tail_steps since the last gather batch
                    nrl = b5r - a5

                def tail_step(final=False):
                    nonlocal gathered, gprev, gage
                    nrg = nrl - DIRECT_TAIL
                    ready = (nrg if not mma_q else
                             max(0, min(nrg, mma_q[0][0] - 1 - a5)))
                    if ready - gathered >= 12 or (ready == nrg and ready > gathered):
                        gprev = gathered
                        emit_gather(a5, gathered, ready, split=True)
                        gathered = ready
                        gage = 0
                    gage += 1
                    # a gather's DMAs are safely in flight after ~2 more L3/
                    # mmA chunks of PE work; then its selectors won't block
                    # the in-order PE queue
                    lim = gathered if (final and gathered == nrl - DIRECT_TAIL) \
                        or gage > 5 else gprev
                    while chunks_f and chunks_f[0][0] <= lim:
                        chunks_f.pop(0)[1]()

                ci4 = 0
                for rr, n in _chunks(a3, b3r):
                    ps = pmain.tile([HID_C, n, W], f32, tag="ps")
                    conv_chunk(ps, w3_sb, h2, rr, n)
                    sl = rr % S
                    nc.scalar.activation(
                        h3[:, sl : sl + n, 1 : 1 + W], ps, Relu, bias=b3_sb
                    )
                    rr_next = rr + n
                    while mma_q and mma_q[0][0] + mma_q[0][1] + 2 <= rr_next:
                        m0, n0 = mma_q.pop(0)
                        emit_mma(m0, n0, ci4)
                        ci4 += 1
                    if last:
                        tail_step()

                while last and mma_q:
                    m0, n0 = mma_q.pop(0)
                    emit_mma(m0, n0, ci4)
                    ci4 += 1
                    tail_step()

                if not last:
                    # defer the mmA stragglers into the next strip's L1
                    # phase (their ACT/DVE copy deps get ~5us of slack) and
                    # the gather + tap-sum creation right after them
                    for m0, n0 in mma_q:
                        mma_carry.append(
                            lambda m0=m0, n0=n0, c=ci4: emit_mma(m0, n0, c))
                        ci4 += 1
                    mma_q.clear()

                    def gather_pend(a5=a5, b5=b5r):
                        nrg = b5 - a5
                        emit_gather(a5, 0, nrg // 2)
                        emit_gather(a5, nrg // 2, nrg)
                        return l4b_chunks(a5, b5)
                else:
                    # keep the PE clock hot through the gather wait: idle
                    # resets the p-state ramp and the post-gap selectors
                    # would run at half clock
                    while chunks_f or gathered < nrl - DIRECT_TAIL:
                        tail_step(final=True)
                    pending = flushes

            # flush any remaining tap-sum emitters
            while pending:
                pending.pop(0)[1]()

    nc.finalize()
    return nc


_NC_CACHE = {}


def _pack_inputs(x, w1, b1, w2, b2, w3, b3, w4, b4):
    x = np.ascontiguousarray(np.asarray(x, dtype=np.float32))
    w1 = np.asarray(w1, dtype=np.float32)
    w2 = np.asarray(w2, dtype=np.float32)
    w3 = np.asarray(w3, dtype=np.float32)
    w4 = np.asarray(w4, dtype=np.float32)
    # w4a slabs: up@0-5, center@6-11, down@32-37, left@64-69, right@96-101
    w4a = np.zeros((HID_C, HID_C), np.float32)
    w4a[:, 0:OUT_C] = w4[:, :, 1].T          # up
    w4a[:, 6 : 6 + OUT_C] = w4[:, :, 0].T    # center
    w4a[:, 32 : 32 + OUT_C] = w4[:, :, 2].T  # down
    w4a[:, 64 : 64 + OUT_C] = w4[:, :, 3].T  # left
    w4a[:, 96 : 96 + OUT_C] = w4[:, :, 4].T  # right
    s6 = np.zeros((HID_C, OUT_C), np.float32)
    for base in (0, 6, 32, 64, 96):
        s6[base + np.arange(OUT_C), np.arange(OUT_C)] = 1.0
    s6 = s6.astype(ml_dtypes.bfloat16)
    sd = np.zeros((HID_C, 5, OUT_C), np.float32)
    for t, base in enumerate((0, 6, 32, 64, 96)):  # up,cen,dn,lf,rt slabs
        sd[base + np.arange(OUT_C), t, np.arange(OUT_C)] = 1.0
    sd = sd.astype(ml_dtypes.bfloat16)
    bias = np.zeros((HID_C, 4), np.float32)
    bias[:, 0] = np.asarray(b1, np.float32)
    bias[:, 1] = np.asarray(b2, np.float32)
    bias[:, 2] = np.asarray(b3, np.float32)
    bias[:OUT_C, 3] = np.asarray(b4, np.float32)
    common = {
        # w1p[t*6+ic, oc] = w1[oc, ic, t]
        "w1p": np.ascontiguousarray(w1.transpose(2, 1, 0).reshape(5 * IN_C, HID_C)),
        # w23p[ic, t, oc] = w2[oc, ic, t] (t<5) / w3[oc, ic, t-5]
        "w23p": np.ascontiguousarray(np.concatenate(
            [w2.transpose(1, 2, 0), w3.transpose(1, 2, 0)], axis=1)),
        "w4a": w4a,
        "s6": s6,
        "sd": sd,
        "bias": bias,
    }
    return x, common


def kernel(x, w1, b1, w2, b2, w3, b3, w4, b4):
    x, common = _pack_inputs(x, w1, b1, w2, b2, w3, b3, w4, b4)
    if "nc" not in _NC_CACHE:
        _NC_CACHE["nc"] = _build()
    nc = _NC_CACHE["nc"]
    in_maps = [dict(common, x=x[i]) for i in range(N_CORES)]
    res = bass_utils.run_bass_kernel_spmd(nc, in_maps, core_ids=list(range(N_CORES)))
    out = np.stack([res.results[i]["y"] for i in range(N_CORES)], axis=0)
    return out


# revision 77
# speedup vs baseline: 1.2387x; 1.0012x over previous
"""Trainium2 Bass kernel for 4-layer cross-stencil CNN.

Per-core: one image [6,256,256] (batch dim sharded across 8 cores).
conv(cross-5-stencil) = 5 channel-matmuls with spatially shifted rhs APs,
accumulated in PSUM. Channels on partitions, spatial (rows x cols) on the
free dim. fp32r matmuls (full PE rate at N>=256).

Rolling-retention pipeline: intermediate buffers h1/h2/h3/t5 are circular
row-slot buffers (S=28 slots, slot = row % 28). Each layer computes every
image row exactly once — no halo recompute. Per strip of R=24 rows the
layer blocks are skewed (L1 rows [a,a+24), L2 [a-2,a+22), L3 [a-4,a+20),
mmA [a-5,a+19), L4b [a-7,a+17)) so all reads hit rows already produced
this strip or retained from the previous one. Reads that cross the
circular wrap are split into <=2 accumulating matmuls; chunk boundaries
are chosen so writes never wrap. Image-boundary taps are handled by
narrowing the tap matmul to valid rows (no zero-pad rows needed).

L1 packs the 5 taps into K=30 via a 5-group pre-shifted x staging buffer
(one matmul per chunk), double-buffered across strips. L4 computes all 5
taps as one M=128 matmul (slabs at partitions 0/6/32/64/96), gathers the
shifted slabs into t5s by SBUF->SBUF DMA, and sums them with one K=128
selector matmul; the strip's tap-sum drains interleaved into the next
strip's L2 phase.
"""

import sys

sys.path.insert(0, "/opt/trn_rl_repo")

import ml_dtypes
import numpy as np

import concourse.bacc as bacc
import concourse.mybir as mybir
from concourse.tile import TileContext
from concourse import bass_utils

IN_C, HID_C, OUT_C = 6, 128, 6
B, H, W = 8, 256, 256
WP = W + 2  # padded width
R = 24  # rows per strip
DIRECT_TAIL = 4  # final rows computed gather-free from t5
S = 28  # circular row slots in h1/h2/h3/t5 (live windows are <= 27 rows)
XS = 28  # row slots in the x staging buffers
N_CORES = 8

f32 = mybir.dt.float32
f32r = mybir.dt.float32r
bf16 = mybir.dt.bfloat16
Add = mybir.AluOpType.add
Max = mybir.AluOpType.max
Relu = mybir.ActivationFunctionType.Relu
Ident = mybir.ActivationFunctionType.Identity

# tap order matches reference: 0=center, 1=up(x[h-1]), 2=down(x[h+1]),
# 3=left(x[w-1]), 4=right(x[w+1])


def _chunks(r0, r1):
    """2-row chunks of [r0,r1), never straddling a multiple of S."""
    r = r0
    while r < r1:
        n = min(2, r1 - r, (r // S + 1) * S - r)
        yield r, n
        r += n


def _runs(r0, r1):
    """Split [r0,r1) into runs of consecutive slots (break at S multiples)."""
    r = r0
    while r < r1:
        e = min(r1, (r // S + 1) * S)
        yield r, e
        r = e


def _build():
    nc = bacc.Bacc("TRN2", target_bir_lowering=False)

    x_d = nc.dram_tensor("x", [IN_C, H, W], f32, kind="ExternalInput")
    w1_d = nc.dram_tensor("w1p", [5 * IN_C, HID_C], f32, kind="ExternalInput")
    w23_d = nc.dram_tensor("w23p", [HID_C, 10, HID_C], f32, kind="ExternalInput")
    # w4a: all 5 taps as M=128 slabs: up@0-5, center@6-11, down@32-37,
    # left@64-69, right@96-101; zero elsewhere
    w4a_d = nc.dram_tensor("w4a", [HID_C, HID_C], f32, kind="ExternalInput")
    # s6: bf16 selector summing the 5 (pre-shifted) slabs of t5s
    s6_d = nc.dram_tensor("s6", [HID_C, OUT_C], bf16, kind="ExternalInput")
    # sd: per-slab selectors for the direct (gather-free) tail chunks
    sd_d = nc.dram_tensor("sd", [HID_C, 5, OUT_C], bf16, kind="ExternalInput")
    bias_d = nc.dram_tensor("bias", [HID_C, 4], f32, kind="ExternalInput")
    y_d = nc.dram_tensor("y", [OUT_C, H, W], f32, kind="ExternalOutput")

    strips = list(range(0, H, R))  # block anchors a = 24k

    with TileContext(nc) as tc:
        with (
            tc.tile_pool(name="const", bufs=1) as cpool,
            tc.tile_pool(name="bufs", bufs=1) as bpool,
            tc.tile_pool(name="io", bufs=2) as iopool,
            tc.tile_pool(name="psmain", bufs=7, space="PSUM") as pmain,
        ):
            # --- weights / biases (resident). HWDGE costs ~625ns per DMA
            # instruction and is a single serial device, so: w1 + strip-0 x
            # first (these gate the first matmul), everything else after,
            # with biases packed into one [128,4] tensor and w2+w3 into one
            # [128,10,128] tensor to cut the instruction count ---
            w1_sb = cpool.tile([5 * IN_C, HID_C], f32r)

            # --- persistent buffers ---
            # xst: 2 staging buffers, 5 tap-groups x 6ch, pre-shifted by DMA
            # placement. Strip k window: x rows [24k-2, 24k+26) at slot
            # r - 24k + 2 (center group); reading slot(r) in group g yields
            # x row r + dr_g at col w + dc_g.
            xst = [
                bpool.tile([5 * IN_C, XS, WP], f32r, name=f"xst{i}")
                for i in range(2)
            ]
            h1 = bpool.tile([HID_C, S, WP], f32r)
            h2 = bpool.tile([HID_C, S, WP], f32r)
            h3 = bpool.tile([HID_C, S, WP], f32r)
            # t5: tap partials (slabs up@0,cen@6,dn@32,lf@64,rt@96), bf16
            t5 = bpool.tile([HID_C, S, WP], bf16)
            # t5s: gathered pre-shifted taps; slot d = output row a5+d
            t5s = bpool.tile([HID_C, R, WP], bf16)

            # x staging pads, narrowed to exactly the cells that are read
            # but never DMA'd (disjoint from every placement, so the strip-0
            # DMAs carry no memset dependency): col 1 for the left group
            # (placed at cols [2,258)), col 256 for the right group (cols
            # [0,256)), and the up-group's slot 2 (x row -1) in buffer 0.
            # Cols 0/257 and other slots are never read.
            # (engine ops need partition base 0/32/64/96, so the ranges
            # start at 0; the extra cells are DMA-overwritten every strip)
            for xb in xst:
                nc.gpsimd.memset(xb[0:24, :, 1:2].bitcast(f32), 0.0)
                nc.gpsimd.memset(xb[0:30, :, 256:257].bitcast(f32), 0.0)
            nc.gpsimd.memset(xst[0][0:12, 2:3, :].bitcast(f32), 0.0)

            def x_dma(a, buf, split=False):
                """Load x rows [a-1, a+R+1) into staging buffer (5 shifted
                placements); center group slot = r - a + 2."""
                lo, hi = max(0, a - 1), min(H, a + R + 1)
                src = x_d[:, lo:hi, :].bitcast(f32r)
                o = lo - a  # >= -1
                # (partitions, slot offset of x row m = m - a + ofs, cols)
                for gi, (p0, ofs, c0) in enumerate((
                    (0, 2, 1),   # center
                    (6, 3, 1),   # up: read slot(r) -> x row r-1
                    (12, 1, 1),  # down
                    (18, 2, 2),  # left: read col w+1 -> x col w-1
                    (24, 2, 0),  # right
                )):
                    # split across the two DMA front-ends: HWDGE (sync) and
                    # SWDGE (gpsimd) process their ~625/1050ns-per-DMA fixed
                    # costs in parallel
                    eng = nc.gpsimd if split and gi >= 3 else nc.sync
                    eng.dma_start(
                        out=xst[buf][p0 : p0 + 6, o + ofs : hi - a + ofs, c0 : c0 + W],
                        in_=src,
                    )

            x_dma(0, 0)

            # p-state priming: the PE idles ~6us while the strip-0 x
            # placements stage, and its clock ramps 0.65->1.2->2.4GHz over
            # the first 3us of busy time. Dependency-free matmuls on a
            # never-written scratch tile fill the idle window so the real
            # work starts at full clock. Their psum is never read.
            dummy = bpool.tile([HID_C, 2, W], f32r)
            nc.gpsimd.memset(dummy[:, 0:1, :].bitcast(f32), 0.0)
            # trigger the 1.3us activation-table load now (ACT is idle);
            # otherwise it lands right before the first L1 relu copy.
            # Output goes to its own scratch (the verifier forbids non-f32r
            # writes into tiles that f32r matmuls consume).
            actwarm = bpool.tile([HID_C, 1], f32)
            nc.scalar.activation(
                actwarm[:, 0:1], dummy[:, 0:1, 0:1].bitcast(f32), Relu)
            for _ in range(12):
                psd = pmain.tile([HID_C, 1, W], f32, tag="ps")
                nc.tensor.matmul(
                    psd, dummy[:, 0, 0:HID_C], dummy[:, 0:1, :],
                    start=True, stop=True,
                )

            # w1 after the strip-0 placements: its 43ns transfer hides in
            # the HWDGE pipeline while the placements' 427ns transfers
            # overlap later HWDGE processing
            nc.sync.dma_start(out=w1_sb, in_=w1_d[:, :].bitcast(f32r))

            bias_sb = cpool.tile([HID_C, 4], f32)
            nc.sync.dma_start(out=bias_sb, in_=bias_d[:, :])
            b1_sb = bias_sb[:, 0:1]
            b2_sb = bias_sb[:, 1:2]
            b3_sb = bias_sb[:, 2:3]
            b4_sb = bias_sb[0:OUT_C, 3:4]
            w23_sb = cpool.tile([HID_C, 10, HID_C], f32r)
            nc.sync.dma_start(out=w23_sb, in_=w23_d[:, :, :].bitcast(f32r))
            w2_sb = w23_sb[:, 0:5, :]
            w3_sb = w23_sb[:, 5:10, :]
            w4a_sb = cpool.tile([HID_C, HID_C], f32r)
            nc.sync.dma_start(out=w4a_sb, in_=w4a_d[:, :].bitcast(f32r))
            s6_sb = cpool.tile([HID_C, OUT_C], bf16)
            nc.sync.dma_start(out=s6_sb, in_=s6_d[:, :])
            sd_sb = cpool.tile([HID_C, 5, OUT_C], bf16)
            nc.sync.dma_start(out=sd_sb, in_=sd_d[:, :, :])

            # h/t5 pads: cols 0 and 257 are read (left/right taps) but
            # never written; zero once — rolling writes only touch [1,257).
            for _t in (h1, h2, h3):
                nc.vector.memset(_t[:, :, 0:1].bitcast(f32), 0.0)
                nc.vector.memset(_t[:, :, 257:258].bitcast(f32), 0.0)
            nc.vector.memset(t5[:, :, 0:1], 0.0)
            nc.vector.memset(t5[:, :, 257:258], 0.0)
            # t5s non-slab partitions are read by the selector matmul
            # (0 * garbage must not be NaN) — zero everything once
            nc.vector.memset(t5s[:, :, :], 0.0)

            def conv_chunk(ps, w_sb, src, r0, n):
                """Cross-stencil accumulation for output rows [r0, r0+n).
                Center tap first (covers the whole psum tile -> start=True
                zeroes it); up/down taps clamp to [0,H) and split at the
                circular wrap."""
                parts = []  # (tap, psum_off, rows, slot)
                for tap, dr in ((1, -1), (2, 1), (3, 0), (4, 0)):
                    lo = max(0, r0 + dr)
                    hi = min(H, r0 + n + dr)
                    for ra, rb in _runs(lo, hi):
                        parts.append((tap, ra - dr - r0, rb - ra, ra % S))
                cols = {0: (1, 1 + W), 1: (1, 1 + W), 2: (1, 1 + W),
                        3: (0, W), 4: (2, 2 + W)}
                nc.tensor.matmul(
                    ps, w_sb[:, 0, :], src[:, r0 % S : r0 % S + n, 1 : 1 + W],
                    start=True, stop=False,
                )
                for i, (tap, off, m, sl) in enumerate(parts):
                    c0, c1 = cols[tap]
                    nc.tensor.matmul(
                        ps[:, off : off + m, :],
                        w_sb[:, tap, :],
                        src[:, sl : sl + m, c0:c1],
                        start=False, stop=(i == len(parts) - 1),
                    )

            # slab parts / tap row-offset / col offsets (dst, src)
            GAT = ((0, -1, 1, 1), (6, 0, 1, 1), (32, 1, 1, 1),
                   (64, 0, 2, 1), (96, 0, 1, 2))

            def emit_gather(a5, d0, d1, split=False):
                """Gather t5 slab rows into pre-shifted t5s slots [d0,d1).
                With split=True alternate slabs between the HWDGE (sync) and
                SWDGE (gpsimd) paths so the two DMA front-ends overlap."""
                for gi, (p0, dr, cd, cs) in enumerate(GAT):
                    eng = nc.gpsimd if split and gi % 2 else nc.sync
                    lo = max(0, a5 + d0 + dr)
                    hi = min(H, a5 + d1 + dr)
                    for ra, rb in _runs(lo, hi):
                        dd = ra - dr - a5
                        m = rb - ra
                        eng.dma_start(
                            out=t5s[p0 : p0 + 6, dd : dd + m, cd : cd + W],
                            in_=t5[p0 : p0 + 6, ra % S : ra % S + m, cs : cs + W],
                        )

            def l4b_chunks(a5, b5, tail=False):
                """Deferred emitters: one K=128 selector matmul per chunk
                sums the 5 pre-shifted slabs of t5s; bias on DVE into a
                strip-wide staging tile; two half-strip DMAs out."""
                nr = b5 - a5
                chunks = list(_chunks(a5, b5))
                # split into flush groups, each with its own staging tile
                nparts = 4 if tail else 2
                halves = []
                prev_i = 0
                for j in range(1, nparts + 1):
                    tgt = nr * j // nparts
                    i = next((i for i, (rr, n) in enumerate(chunks)
                              if rr + n - a5 >= tgt), len(chunks) - 1)
                    if i + 1 > prev_i:
                        halves.append(chunks[prev_i : i + 1])
                        prev_i = i + 1
                out = []  # (needed t5s rows, emitter) pairs
                dt0 = b5 - DIRECT_TAIL if tail else b5
                for half in halves:
                    if not half:
                        continue
                    lo = half[0][0] - a5
                    hi = half[-1][0] + half[-1][1] - a5
                    yst = iopool.tile([OUT_C, hi - lo, W], f32, tag="yst")
                    for ei, (rr, n) in enumerate(half):

                        def emit(rr=rr, n=n, a5=a5, yst=yst, lo=lo, ei=ei,
                                 tail=tail):
                            d = rr - a5
                            ps = pmain.tile([OUT_C, n, W], f32, tag="ps")
                            if tail and rr >= a5 + (dt0 - a5):
                                # gather-free: per-slab selectors read t5
                                # directly with the slab's row/col shift
                                parts = []
                                for tp, dr, c0 in ((0, -1, 1), (2, 1, 1),
                                                   (3, 0, 0), (4, 0, 2)):
                                    pl = max(0, rr + dr)
                                    ph = min(H, rr + n + dr)
                                    for ra, rb in _runs(pl, ph):
                                        parts.append(
                                            (tp, ra - dr - rr, rb - ra,
                                             ra % S, c0))
                                nc.tensor.matmul(
                                    ps, sd_sb[:, 1, :],
                                    t5[:, rr % S : rr % S + n, 1 : 1 + W],
                                    start=True, stop=False,
                                )
                                for i, (tp, off, m, sl, c0) in enumerate(parts):
                                    nc.tensor.matmul(
                                        ps[:, off : off + m, :], sd_sb[:, tp, :],
                                        t5[:, sl : sl + m, c0 : c0 + W],
                                        start=False, stop=(i == len(parts) - 1),
                                    )
                            else:
                                nc.tensor.matmul(
                                    ps, s6_sb[:, :], t5s[:, d : d + n, 1 : 1 + W],
                                    start=True, stop=True,
                                )
                            if tail and ei % 2:
                                # ACT is idle at the drain; alternating
                                # halves the serialized add chain
                                nc.scalar.activation(
                                    yst[:, d - lo : d - lo + n, :], ps,
                                    Ident, bias=b4_sb)
                            else:
                                nc.vector.tensor_scalar_add(
                                    yst[:, d - lo : d - lo + n, :], ps, b4_sb)

                        need = (min(rr + n - a5, dt0 - a5) if tail
                                else rr + n - a5)
                        out.append((need, emit))

                    def flush(lo=lo, hi=hi, a5=a5, yst=yst):
                        nc.sync.dma_start(
                            out=y_d[:, a5 + lo : a5 + hi, :],
                            in_=yst[:, :, :])

                    out.append((hi, flush))
                return out

            pending = []  # tap-sum emitters from the previous strip
            mma_carry = []  # end-of-strip mmA chunks deferred into next L1
            gather_pend = None  # deferred gather + l4b creation
            for k, a in enumerate(strips):
                last = k == len(strips) - 1
                # skewed layer blocks for this strip
                a1, b1r = a, min(H, a + R)
                a2, b2r = max(0, a - 2), min(H, a + R - 2)
                a3, b3r = max(0, a - 4), min(H, a + R - 4)
                a4, b4r = max(0, a - 5), min(H, a + R - 5)
                a5, b5r = max(0, a - 7), min(H, a + R - 7)
                if last:
                    b2r = b3r = b4r = b5r = H

                # prefetch next strip's x into the other staging buffer
                if not last:
                    a_n = a + R
                    if a_n + R >= H:
                        # final strip: slots past the image tail keep stale
                        # data from two strips ago — zero them before the DMA
                        tail0 = min(H, a_n + R + 1) - a_n + 1
                        nc.gpsimd.memset(
                            xst[(k + 1) % 2][:, tail0:XS, :].bitcast(f32), 0.0)
                    x_dma(a_n, (k + 1) % 2)

                # --- L1: one K=30 matmul per chunk; copies alternate
                # DVE/ACT ---
                xb = xst[k % 2]
                ci = 0
                for rr, n in _chunks(a1, b1r):
                    ps = pmain.tile([HID_C, n, W], f32, tag="ps")
                    s0 = rr - a + 2
                    nc.tensor.matmul(
                        ps, w1_sb[:, :], xb[:, s0 : s0 + n, 1 : 1 + W],
                        start=True, stop=True,
                    )
                    sl = rr % S
                    if ci % 2 == 0:
                        nc.vector.tensor_scalar(
                            h1[:, sl : sl + n, 1 : 1 + W], ps, b1_sb, 0.0, Add, Max
                        )
                    else:
                        nc.scalar.activation(
                            h1[:, sl : sl + n, 1 : 1 + W], ps, Relu, bias=b1_sb
                        )
                    ci += 1
                    if mma_carry and ci >= 4:
                        mma_carry.pop(0)()
                while mma_carry:
                    mma_carry.pop(0)()
                if gather_pend is not None:
                    pending = gather_pend()
                    gather_pend = None

                # --- L2: reads h1; interleave prev strip's tap-sum ---
                for rr, n in _chunks(a2, b2r):
                    ps = pmain.tile([HID_C, n, W], f32, tag="ps")
                    conv_chunk(ps, w2_sb, h1, rr, n)
                    sl = rr % S
                    nc.scalar.activation(
                        h2[:, sl : sl + n, 1 : 1 + W], ps, Relu, bias=b2_sb
                    )
                    if pending:
                        pending.pop(0)[1]()
                while pending:
                    pending.pop(0)[1]()

                # --- L3: reads h2; mmA interleaved so t5 fills (and the
                # gather can start) as h3 rows land ---
                mma_q = list(_chunks(a4, b4r))

                def emit_mma(rr, n, ci):
                    ps = pmain.tile([HID_C, n, W], f32, tag="ps")
                    sl = rr % S
                    nc.tensor.matmul(
                        ps, w4a_sb[:, :], h3[:, sl : sl + n, 1 : 1 + W],
                        start=True, stop=True,
                    )
                    if ci % 2 == 0:
                        nc.vector.tensor_copy(t5[:, sl : sl + n, 1 : 1 + W], ps)
                    else:
                        nc.scalar.activation(
                            t5[:, sl : sl + n, 1 : 1 + W], ps, Ident)

                if k == 0:
                    # row 0 up-slab source is row -1: zero that dest
                    nc.gpsimd.memset(t5s[0:6, 0:1, :], 0.0)
                if last:
                    # row 255 down-slab source is row 256
                    nc.gpsimd.memset(t5s[32:38, b5r - 1 - a5 : b5r - a5, :], 0.0)

                if last:
                    # final strip: interleave gather / tap-sum into the L3
                    # drain so the tail doesn't serialize; y flushes (whose
                    # deps resolve last) go at the very end. Selector pops
                    # lag one gather batch so they never block the in-order
                    # PE queue while their gather DMAs are still in flight.
                    allp = l4b_chunks(a5, b5r, tail=True)
                    chunks_f = [p for p in allp if p[1].__name__ == "emit"]
                    flushes = [p for p in allp if p[1].__name__ == "flush"]
                    gathered = 0
                    gprev = 0
                    gage = 0  # tail_steps since the last gather batch
                    nrl = b5r - a5

                def tail_step(final=False):
                    nonlocal gathered, gprev, gage
                    nrg = nrl - DIRECT_TAIL
                    ready = (nrg if not mma_q else
                             max(0, min(nrg, mma_q[0][0] - 1 - a5)))
                    if ready - gathered >= 12 or (ready == nrg and ready > gathered):
                        gprev = gathered
                        emit_gather(a5, gathered, ready, split=True)
                        gathered = ready
                        gage = 0
                    gage += 1
                    # a gather's DMAs are safely in flight after ~2 more L3/
                    # mmA chunks of PE work; then its selectors won't block
                    # the in-order PE queue
                    lim = gathered if (final and gathered == nrl - DIRECT_TAIL) \
                        or gage > 5 else gprev
                    while chunks_f and chunks_f[0][0] <= lim:
                        chunks_f.pop(0)[1]()

                ci4 = 0
                for rr, n in _chunks(a3, b3r):
                    ps = pmain.tile([HID_C, n, W], f32, tag="ps")
                    conv_chunk(ps, w3_sb, h2, rr, n)
                    sl = rr % S
                    nc.scalar.activation(
                        h3[:, sl : sl + n, 1 : 1 + W], ps, Relu, bias=b3_sb
                    )
                    rr_next = rr + n
                    while mma_q and mma_q[0][0] + mma_q[0][1] + 2 <= rr_next:
                        m0, n0 = mma_q.pop(0)
                        emit_mma(m0, n0, ci4)
                        ci4 += 1
                    if last:
                        tail_step()

                while last and mma_q:
                    m0, n0 = mma_q.pop(0)
                    emit_mma(m0, n0, ci4)
                    ci4 += 1
                    tail_step()

                if not last:
                    # defer the mmA stragglers into the next strip's L1
                    # phase (their ACT/DVE copy deps get ~5us of slack) and
                    # the gather + tap-sum creation right after them
                    for m0, n0 in mma_q:
                        mma_carry.append(
                            lambda m0=m0, n0=n0, c=ci4: emit_mma(m0, n0, c))
                        ci4 += 1
                    mma_q.clear()

                    def gather_pend(a5=a5, b5=b5r):
                        nrg = b5 - a5
                        emit_gather(a5, 0, nrg // 2)
                        emit_gather(a5, nrg // 2, nrg)
                        return l4b_chunks(a5, b5)
                else:
                    # keep the PE clock hot through the gather wait: idle
                    # resets the p-state ramp and the post-gap selectors
                    # would run at half clock
                    while chunks_f or gathered < nrl - DIRECT_TAIL:
                        tail_step(final=True)
                    pending = flushes

            # flush any remaining tap-sum emitters
            while pending:
                pending.pop(0)[1]()

    nc.finalize()
    return nc


_NC_CACHE = {}


def _pack_inputs(x, w1, b1, w2, b2, w3, b3, w4, b4):
    x = np.ascontiguousarray(np.asarray(x, dtype=np.float32))
    w1 = np.asarray(w1, dtype=np.float32)
    w2 = np.asarray(w2, dtype=np.float32)
    w3 = np.asarray(w3, dtype=np.float32)
    w4 = np.asarray(w4, dtype=np.float32)
    # w4a slabs: up@0-5, center@6-11, down@32-37, left@64-69, right@96-101
    w4a = np.zeros((HID_C, HID_C), np.float32)
    w4a[:, 0:OUT_C] = w4[:, :, 1].T          # up
    w4a[:, 6 : 6 + OUT_C] = w4[:, :, 0].T    # center
    w4a[:, 32 : 32 + OUT_C] = w4[:, :, 2].T  # down
    w4a[:, 64 : 64 + OUT_C] = w4[:, :, 3].T  # left
    w4a[:, 96 : 96 + OUT_C] = w4[:, :, 4].T  # right
    s6 = np.zeros((HID_C, OUT_C), np.float32)
    for base in (0, 6, 32, 64, 96):
        s6[base + np.arange(OUT_C), np.arange(OUT_C)] = 1.0
    s6 = s6.astype(ml_dtypes.bfloat16)
    sd = np.zeros((HID_C, 5, OUT_C), np.float32)
    for t, base in enumerate((0, 6, 32, 64, 96)):  # up,cen,dn,lf,rt slabs
        sd[base + np.arange(OUT_C), t, np.arange(OUT_C)] = 1.0
    sd = sd.astype(ml_dtypes.bfloat16)
    bias = np.zeros((HID_C, 4), np.float32)
    bias[:, 0] = np.asarray(b1, np.float32)
    bias[:, 1] = np.asarray(b2, np.float32)
    bias[:, 2] = np.asarray(b3, np.float32)
    bias[:OUT_C, 3] = np.asarray(b4, np.float32)
    common = {
        # w1p[t*6+ic, oc] = w1[oc, ic, t]
        "w1p": np.ascontiguousarray(w1.transpose(2, 1, 0).reshape(5 * IN_C, HID_C)),
        # w23p[ic, t, oc] = w2[oc, ic, t] (t<5) / w3[oc, ic, t-5]
        "w23p": np.ascontiguousarray(np.concatenate(
            [w2.transpose(1, 2, 0), w3.transpose(1, 2, 0)], axis=1)),
        "w4a": w4a,
        "s6": s6,
        "sd": sd,
        "bias": bias,
    }
    return x, common


def kernel(x, w1, b1, w2, b2, w3, b3, w4, b4):
    x, common = _pack_inputs(x, w1, b1, w2, b2, w3, b3, w4, b4)
    if "nc" not in _NC_CACHE:
        _NC_CACHE["nc"] = _build()
    nc = _NC_CACHE["nc"]
    in_maps = [dict(common, x=x[i]) for i in range(N_CORES)]
    res = bass_utils.run_bass_kernel_spmd(nc, in_maps, core_ids=list(range(N_CORES)))
    out = np.stack([res.results[i]["y"] for i in range(N_CORES)], axis=0)
    return out


# revision 91
# speedup vs baseline: 1.2392x; 1.0005x over previous
"""Trainium2 Bass kernel for 4-layer cross-stencil CNN.

Per-core: one image [6,256,256] (batch dim sharded across 8 cores).
conv(cross-5-stencil) = 5 channel-matmuls with spatially shifted rhs APs,
accumulated in PSUM. Channels on partitions, spatial (rows x cols) on the
free dim. fp32r matmuls (full PE rate at N>=256).

Rolling-retention pipeline: intermediate buffers h1/h2/h3/t5 are circular
row-slot buffers (S=28 slots, slot = row % 28). Each layer computes every
image row exactly once — no halo recompute. Per strip of R=24 rows the
layer blocks are skewed (L1 rows [a,a+24), L2 [a-2,a+22), L3 [a-4,a+20),
mmA [a-5,a+19), L4b [a-7,a+17)) so all reads hit rows already produced
this strip or retained from the previous one. Reads that cross the
circular wrap are split into <=2 accumulating matmuls; chunk boundaries
are chosen so writes never wrap. Image-boundary taps are handled by
narrowing the tap matmul to valid rows (no zero-pad rows needed).

L1 packs the 5 taps into K=30 via a 5-group pre-shifted x staging buffer
(one matmul per chunk), double-buffered across strips. L4 computes all 5
taps as one M=128 matmul (slabs at partitions 0/6/32/64/96), gathers the
shifted slabs into t5s by SBUF->SBUF DMA, and sums them with one K=128
selector matmul; the strip's tap-sum drains interleaved into the next
strip's L2 phase.
"""

import sys

sys.path.insert(0, "/opt/trn_rl_repo")

import ml_dtypes
import numpy as np

import concourse.bacc as bacc
import concourse.mybir as mybir
from concourse.tile import TileContext
from concourse import bass_utils

IN_C, HID_C, OUT_C = 6, 128, 6
B, H, W = 8, 256, 256
WP = W + 2  # padded width
R = 24  # rows per strip
DIRECT_TAIL = 4  # final rows computed gather-free from t5
S = 28  # circular row slots in h1/h2/h3/t5 (live windows are <= 27 rows)
XS = 28  # row slots in the x staging buffers
N_CORES = 8

f32 = mybir.dt.float32
f32r = mybir.dt.float32r
bf16 = mybir.dt.bfloat16
Add = mybir.AluOpType.add
Max = mybir.AluOpType.max
Relu = mybir.ActivationFunctionType.Relu
Ident = mybir.ActivationFunctionType.Identity

# tap order matches reference: 0=center, 1=up(x[h-1]), 2=down(x[h+1]),
# 3=left(x[w-1]), 4=right(x[w+1])


def _chunks(r0, r1):
    """2-row chunks of [r0,r1), never straddling a multiple of S."""
    r = r0
    while r < r1:
        n = min(2, r1 - r, (r // S + 1) * S - r)
        yield r, n
        r += n


def _runs(r0, r1):
    """Split [r0,r1) into runs of consecutive slots (break at S multiples)."""
    r = r0
    while r < r1:
        e = min(r1, (r // S + 1) * S)
        yield r, e
        r = e


def _build():
    nc = bacc.Bacc("TRN2", target_bir_lowering=False)

    x_d = nc.dram_tensor("x", [IN_C, H, W], f32, kind="ExternalInput")
    w1_d = nc.dram_tensor("w1p", [5 * IN_C, HID_C], f32, kind="ExternalInput")
    w23_d = nc.dram_tensor("w23p", [HID_C, 10, HID_C], f32, kind="ExternalInput")
    # w4a: all 5 taps as M=128 slabs: up@0-5, center@6-11, down@32-37,
    # left@64-69, right@96-101; zero elsewhere
    w4a_d = nc.dram_tensor("w4a", [HID_C, HID_C], f32, kind="ExternalInput")
    # s6: bf16 selector summing the 5 (pre-shifted) slabs of t5s
    s6_d = nc.dram_tensor("s6", [HID_C, OUT_C], bf16, kind="ExternalInput")
    # sd: per-slab selectors for the direct (gather-free) tail chunks
    sd_d = nc.dram_tensor("sd", [HID_C, 5, OUT_C], bf16, kind="ExternalInput")
    bias_d = nc.dram_tensor("bias", [HID_C, 4], f32, kind="ExternalInput")
    y_d = nc.dram_tensor("y", [OUT_C, H, W], f32, kind="ExternalOutput")

    strips = list(range(0, H, R))  # block anchors a = 24k

    with TileContext(nc) as tc:
        with (
            tc.tile_pool(name="const", bufs=1) as cpool,
            tc.tile_pool(name="bufs", bufs=1) as bpool,
            tc.tile_pool(name="io", bufs=2) as iopool,
            tc.tile_pool(name="psmain", bufs=7, space="PSUM") as pmain,
        ):
            # --- weights / biases (resident). HWDGE costs ~625ns per DMA
            # instruction and is a single serial device, so: w1 + strip-0 x
            # first (these gate the first matmul), everything else after,
            # with biases packed into one [128,4] tensor and w2+w3 into one
            # [128,10,128] tensor to cut the instruction count ---
            w1_sb = cpool.tile([5 * IN_C, HID_C], f32r)

            # --- persistent buffers ---
            # xst: 2 staging buffers, 5 tap-groups x 6ch, pre-shifted by DMA
            # placement. Strip k window: x rows [24k-2, 24k+26) at slot
            # r - 24k + 2 (center group); reading slot(r) in group g yields
            # x row r + dr_g at col w + dc_g.
            xst = [
                bpool.tile([5 * IN_C, XS, WP], f32r, name=f"xst{i}")
                for i in range(2)
            ]
            h1 = bpool.tile([HID_C, S, WP], f32r)
            h2 = bpool.tile([HID_C, S, WP], f32r)
            h3 = bpool.tile([HID_C, S, WP], f32r)
            # t5: tap partials (slabs up@0,cen@6,dn@32,lf@64,rt@96), bf16
            t5 = bpool.tile([HID_C, S, WP], bf16)
            # t5s: gathered pre-shifted taps; slot d = output row a5+d
            t5s = bpool.tile([HID_C, R, WP], bf16)

            # x staging pads, narrowed to exactly the cells that are read
            # but never DMA'd (disjoint from every placement, so the strip-0
            # DMAs carry no memset dependency): col 1 for the left group
            # (placed at cols [2,258)), col 256 for the right group (cols
            # [0,256)), and the up-group's slot 2 (x row -1) in buffer 0.
            # Cols 0/257 and other slots are never read.
            # (engine ops need partition base 0/32/64/96, so the ranges
            # start at 0; the extra cells are DMA-overwritten every strip)
            for xb in xst:
                nc.gpsimd.memset(xb[0:24, :, 1:2].bitcast(f32), 0.0)
                nc.gpsimd.memset(xb[0:30, :, 256:257].bitcast(f32), 0.0)
            nc.gpsimd.memset(xst[0][0:12, 2:3, :].bitcast(f32), 0.0)

            def x_dma(a, buf, split=False):
                """Load x rows [a-1, a+R+1) into staging buffer (5 shifted
                placements); center group slot = r - a + 2."""
                lo, hi = max(0, a - 1), min(H, a + R + 1)
                src = x_d[:, lo:hi, :].bitcast(f32r)
                o = lo - a  # >= -1
                # (partitions, slot offset of x row m = m - a + ofs, cols)
                for gi, (p0, ofs, c0) in enumerate((
                    (0, 2, 1),   # center
                    (6, 3, 1),   # up: read slot(r) -> x row r-1
                    (12, 1, 1),  # down
                    (18, 2, 2),  # left: read col w+1 -> x col w-1
                    (24, 2, 0),  # right
                )):
                    # split across the two DMA front-ends: HWDGE (sync) and
                    # SWDGE (gpsimd) process their ~625/1050ns-per-DMA fixed
                    # costs in parallel
                    eng = nc.gpsimd if split and gi >= 3 else nc.sync
                    eng.dma_start(
                        out=xst[buf][p0 : p0 + 6, o + ofs : hi - a + ofs, c0 : c0 + W],
                        in_=src,
                    )

            x_dma(0, 0)

            # p-state priming: the PE idles ~6us while the strip-0 x
            # placements stage, and its clock ramps 0.65->1.2->2.4GHz over
            # the first 3us of busy time. Dependency-free matmuls on a
            # never-written scratch tile fill the idle window so the real
            # work starts at full clock. Their psum is never read.
            dummy = bpool.tile([HID_C, 2, W], f32r)
            nc.gpsimd.memset(dummy[:, 0:1, :].bitcast(f32), 0.0)
            # trigger the 1.3us activation-table load now (ACT is idle);
            # otherwise it lands right before the first L1 relu copy.
            # Output goes to its own scratch (the verifier forbids non-f32r
            # writes into tiles that f32r matmuls consume).
            actwarm = bpool.tile([HID_C, 1], f32)
            nc.scalar.activation(
                actwarm[:, 0:1], dummy[:, 0:1, 0:1].bitcast(f32), Relu)
            for _ in range(12):
                psd = pmain.tile([HID_C, 1, W], f32, tag="ps")
                nc.tensor.matmul(
                    psd, dummy[:, 0, 0:HID_C], dummy[:, 0:1, :],
                    start=True, stop=True,
                )

            # w1 after the strip-0 placements: its 43ns transfer hides in
            # the HWDGE pipeline while the placements' 427ns transfers
            # overlap later HWDGE processing
            nc.sync.dma_start(out=w1_sb, in_=w1_d[:, :].bitcast(f32r))

            bias_sb = cpool.tile([HID_C, 4], f32)
            nc.sync.dma_start(out=bias_sb, in_=bias_d[:, :])
            b1_sb = bias_sb[:, 0:1]
            b2_sb = bias_sb[:, 1:2]
            b3_sb = bias_sb[:, 2:3]
            b4_sb = bias_sb[0:OUT_C, 3:4]
            w23_sb = cpool.tile([HID_C, 10, HID_C], f32r)
            nc.sync.dma_start(out=w23_sb, in_=w23_d[:, :, :].bitcast(f32r))
            w2_sb = w23_sb[:, 0:5, :]
            w3_sb = w23_sb[:, 5:10, :]
            w4a_sb = cpool.tile([HID_C, HID_C], f32r)
            nc.sync.dma_start(out=w4a_sb, in_=w4a_d[:, :].bitcast(f32r))
            s6_sb = cpool.tile([HID_C, OUT_C], bf16)
            nc.sync.dma_start(out=s6_sb, in_=s6_d[:, :])
            sd_sb = cpool.tile([HID_C, 5, OUT_C], bf16)
            nc.sync.dma_start(out=sd_sb, in_=sd_d[:, :, :])

            # h/t5 pads: cols 0 and 257 are read (left/right taps) but
            # never written; zero once — rolling writes only touch [1,257).
            for _t in (h1, h2, h3):
                nc.vector.memset(_t[:, :, 0:1].bitcast(f32), 0.0)
                nc.vector.memset(_t[:, :, 257:258].bitcast(f32), 0.0)
            nc.vector.memset(t5[:, :, 0:1], 0.0)
            nc.vector.memset(t5[:, :, 257:258], 0.0)
            # t5s non-slab partitions are read by the selector matmul
            # (0 * garbage must not be NaN) — zero everything once
            nc.vector.memset(t5s[:, :, :], 0.0)

            def conv_chunk(ps, w_sb, src, r0, n):
                """Cross-stencil accumulation for output rows [r0, r0+n).
                Center tap first (covers the whole psum tile -> start=True
                zeroes it); up/down taps clamp to [0,H) and split at the
                circular wrap."""
                parts = []  # (tap, psum_off, rows, slot)
                for tap, dr in ((1, -1), (2, 1), (3, 0), (4, 0)):
                    lo = max(0, r0 + dr)
                    hi = min(H, r0 + n + dr)
                    for ra, rb in _runs(lo, hi):
                        parts.append((tap, ra - dr - r0, rb - ra, ra % S))
                cols = {0: (1, 1 + W), 1: (1, 1 + W), 2: (1, 1 + W),
                        3: (0, W), 4: (2, 2 + W)}
                nc.tensor.matmul(
                    ps, w_sb[:, 0, :], src[:, r0 % S : r0 % S + n, 1 : 1 + W],
                    start=True, stop=False,
                )
                for i, (tap, off, m, sl) in enumerate(parts):
                    c0, c1 = cols[tap]
                    nc.tensor.matmul(
                        ps[:, off : off + m, :],
                        w_sb[:, tap, :],
                        src[:, sl : sl + m, c0:c1],
                        start=False, stop=(i == len(parts) - 1),
                    )

            # slab parts / tap row-offset / col offsets (dst, src)
            GAT = ((0, -1, 1, 1), (6, 0, 1, 1), (32, 1, 1, 1),
                   (64, 0, 2, 1), (96, 0, 1, 2))

            def emit_gather(a5, d0, d1, split=False):
                """Gather t5 slab rows into pre-shifted t5s slots [d0,d1).
                With split=True alternate slabs between the HWDGE (sync) and
                SWDGE (gpsimd) paths so the two DMA front-ends overlap."""
                for gi, (p0, dr, cd, cs) in enumerate(GAT):
                    eng = nc.gpsimd if split and gi % 2 else nc.sync
                    lo = max(0, a5 + d0 + dr)
                    hi = min(H, a5 + d1 + dr)
                    for ra, rb in _runs(lo, hi):
                        dd = ra - dr - a5
                        m = rb - ra
                        eng.dma_start(
                            out=t5s[p0 : p0 + 6, dd : dd + m, cd : cd + W],
                            in_=t5[p0 : p0 + 6, ra % S : ra % S + m, cs : cs + W],
                        )

            def l4b_chunks(a5, b5, tail=False):
                """Deferred emitters: one K=128 selector matmul per chunk
                sums the 5 pre-shifted slabs of t5s; bias on DVE into a
                strip-wide staging tile; two half-strip DMAs out."""
                nr = b5 - a5
                chunks = list(_chunks(a5, b5))
                # split into flush groups, each with its own staging tile
                nparts = 4 if tail else 2
                halves = []
                prev_i = 0
                for j in range(1, nparts + 1):
                    tgt = nr * j // nparts
                    i = next((i for i, (rr, n) in enumerate(chunks)
                              if rr + n - a5 >= tgt), len(chunks) - 1)
                    if i + 1 > prev_i:
                        halves.append(chunks[prev_i : i + 1])
                        prev_i = i + 1
                out = []  # (needed t5s rows, emitter) pairs
                dt0 = b5 - DIRECT_TAIL if tail else b5
                for half in halves:
                    if not half:
                        continue
                    lo = half[0][0] - a5
                    hi = half[-1][0] + half[-1][1] - a5
                    yst = iopool.tile([OUT_C, hi - lo, W], f32, tag="yst")
                    for ei, (rr, n) in enumerate(half):

                        def emit(rr=rr, n=n, a5=a5, yst=yst, lo=lo, ei=ei,
                                 tail=tail):
                            d = rr - a5
                            ps = pmain.tile([OUT_C, n, W], f32, tag="ps")
                            if tail and rr >= a5 + (dt0 - a5):
                                # gather-free: per-slab selectors read t5
                                # directly with the slab's row/col shift
                                parts = []
                                for tp, dr, c0 in ((0, -1, 1), (2, 1, 1),
                                                   (3, 0, 0), (4, 0, 2)):
                                    pl = max(0, rr + dr)
                                    ph = min(H, rr + n + dr)
                                    for ra, rb in _runs(pl, ph):
                                        parts.append(
                                            (tp, ra - dr - rr, rb - ra,
                                             ra % S, c0))
                                nc.tensor.matmul(
                                    ps, sd_sb[:, 1, :],
                                    t5[:, rr % S : rr % S + n, 1 : 1 + W],
                                    start=True, stop=False,
                                )
                                for i, (tp, off, m, sl, c0) in enumerate(parts):
                                    nc.tensor.matmul(
                                        ps[:, off : off + m, :], sd_sb[:, tp, :],
                                        t5[:, sl : sl + m, c0 : c0 + W],
                                        start=False, stop=(i == len(parts) - 1),
                                    )
                            else:
                                nc.tensor.matmul(
                                    ps, s6_sb[:, :], t5s[:, d : d + n, 1 : 1 + W],
                                    start=True, stop=True,
                                )
                            if tail and ei % 2:
                                # ACT is idle at the drain; alternating
                                # halves the serialized add chain
                                nc.scalar.activation(
                                    yst[:, d - lo : d - lo + n, :], ps,
                                    Ident, bias=b4_sb)
                            else:
                                nc.vector.tensor_scalar_add(
                                    yst[:, d - lo : d - lo + n, :], ps, b4_sb)

                        need = (min(rr + n - a5, dt0 - a5) if tail
                                else rr + n - a5)
                        out.append((need, emit))

                    def flush(lo=lo, hi=hi, a5=a5, yst=yst):
                        nc.sync.dma_start(
                            out=y_d[:, a5 + lo : a5 + hi, :],
                            in_=yst[:, :, :])

                    out.append((hi, flush))
                return out

            pending = []  # tap-sum emitters from the previous strip
            mma_carry = []  # end-of-strip mmA chunks deferred into next L1
            gather_pend = None  # deferred gather + l4b creation
            for k, a in enumerate(strips):
                last = k == len(strips) - 1
                # skewed layer blocks for this strip
                a1, b1r = a, min(H, a + R)
                a2, b2r = max(0, a - 2), min(H, a + R - 2)
                a3, b3r = max(0, a - 4), min(H, a + R - 4)
                a4, b4r = max(0, a - 5), min(H, a + R - 5)
                a5, b5r = max(0, a - 7), min(H, a + R - 7)
                if last:
                    b2r = b3r = b4r = b5r = H

                # prefetch next strip's x into the other staging buffer
                if not last:
                    a_n = a + R
                    if a_n + R >= H:
                        # final strip: slots past the image tail keep stale
                        # data from two strips ago — zero them before the DMA
                        tail0 = min(H, a_n + R + 1) - a_n + 1
                        nc.gpsimd.memset(
                            xst[(k + 1) % 2][:, tail0:XS, :].bitcast(f32), 0.0)
                    x_dma(a_n, (k + 1) % 2)

                # --- L1: one K=30 matmul per chunk; copies alternate
                # DVE/ACT ---
                # --- L1 + L2 merged: the L1 phase is copy-chain-bound
                # (213ns of PE per chunk vs ~635ns of DVE/ACT copy), so
                # interleave L2 convs (copy-independent PE work, data-ready
                # at 2 chunks of lag) to absorb the psum-rotation waits ---
                def do_l2(rr, n):
                    ps = pmain.tile([HID_C, n, W], f32, tag="ps")
                    conv_chunk(ps, w2_sb, h1, rr, n)
                    sl = rr % S
                    nc.scalar.activation(
                        h2[:, sl : sl + n, 1 : 1 + W], ps, Relu, bias=b2_sb
                    )
                    if pending:
                        pending.pop(0)[1]()

                xb = xst[k % 2]
                l2c = list(_chunks(a2, b2r))
                ci = 0
                for rr, n in _chunks(a1, b1r):
                    ps = pmain.tile([HID_C, n, W], f32, tag="ps")
                    s0 = rr - a + 2
                    nc.tensor.matmul(
                        ps, w1_sb[:, :], xb[:, s0 : s0 + n, 1 : 1 + W],
                        start=True, stop=True,
                    )
                    sl = rr % S
                    if ci % 2 == 0:
                        nc.vector.tensor_scalar(
                            h1[:, sl : sl + n, 1 : 1 + W], ps, b1_sb, 0.0, Add, Max
                        )
                    else:
                        nc.scalar.activation(
                            h1[:, sl : sl + n, 1 : 1 + W], ps, Relu, bias=b1_sb
                        )
                    ci += 1
                    if mma_carry and ci >= 2:
                        mma_carry.pop(0)()
                    # gather(k-1) reads t5 rows written by the carried mmA
                    # chunks: it must not be emitted before they drain
                    if ci >= 3 and not mma_carry and gather_pend is not None:
                        pending = gather_pend()
                        gather_pend = None
                    if ci >= 3 and l2c:
                        do_l2(*l2c.pop(0))
                while mma_carry:
                    mma_carry.pop(0)()
                if gather_pend is not None:
                    pending = gather_pend()
                    gather_pend = None
                while l2c:
                    do_l2(*l2c.pop(0))
                while pending:
                    pending.pop(0)[1]()

                # --- L3: reads h2; mmA interleaved so t5 fills (and the
                # gather can start) as h3 rows land ---
                mma_q = list(_chunks(a4, b4r))

                def emit_mma(rr, n, ci):
                    ps = pmain.tile([HID_C, n, W], f32, tag="ps")
                    sl = rr % S
                    nc.tensor.matmul(
                        ps, w4a_sb[:, :], h3[:, sl : sl + n, 1 : 1 + W],
                        start=True, stop=True,
                    )
                    if ci % 2 == 0:
                        nc.vector.tensor_copy(t5[:, sl : sl + n, 1 : 1 + W], ps)
                    else:
                        nc.scalar.activation(
                            t5[:, sl : sl + n, 1 : 1 + W], ps, Ident)

                if k == 0:
                    # row 0 up-slab source is row -1: zero that dest
                    nc.gpsimd.memset(t5s[0:6, 0:1, :], 0.0)
                if last:
                    # row 255 down-slab source is row 256
                    nc.gpsimd.memset(t5s[32:38, b5r - 1 - a5 : b5r - a5, :], 0.0)

                if last:
                    # final strip: interleave gather / tap-sum into the L3
                    # drain so the tail doesn't serialize; y flushes (whose
                    # deps resolve last) go at the very end. Selector pops
                    # lag one gather batch so they never block the in-order
                    # PE queue while their gather DMAs are still in flight.
                    allp = l4b_chunks(a5, b5r, tail=True)
                    chunks_f = [p for p in allp if p[1].__name__ == "emit"]
                    flushes = [p for p in allp if p[1].__name__ == "flush"]
                    gathered = 0
                    gprev = 0
                    gage = 0  # tail_steps since the last gather batch
                    nrl = b5r - a5

                def tail_step(final=False):
                    nonlocal gathered, gprev, gage
                    nrg = nrl - DIRECT_TAIL
                    ready = (nrg if not mma_q else
                             max(0, min(nrg, mma_q[0][0] - 1 - a5)))
                    if ready - gathered >= 12 or (ready == nrg and ready > gathered):
                        gprev = gathered
                        emit_gather(a5, gathered, ready, split=True)
                        gathered = ready
                        gage = 0
                    gage += 1
                    # a gather's DMAs are safely in flight after ~2 more L3/
                    # mmA chunks of PE work; then its selectors won't block
                    # the in-order PE queue
                    lim = gathered if (final and gathered == nrl - DIRECT_TAIL) \
                        or gage > 5 else gprev
                    while chunks_f and chunks_f[0][0] <= lim:
                        chunks_f.pop(0)[1]()

                ci4 = 0
                for rr, n in _chunks(a3, b3r):
                    ps = pmain.tile([HID_C, n, W], f32, tag="ps")
                    conv_chunk(ps, w3_sb, h2, rr, n)
                    sl = rr % S
                    nc.scalar.activation(
                        h3[:, sl : sl + n, 1 : 1 + W], ps, Relu, bias=b3_sb
                    )
                    rr_next = rr + n
                    while mma_q and mma_q[0][0] + mma_q[0][1] + 2 <= rr_next:
                        m0, n0 = mma_q.pop(0)
                        emit_mma(m0, n0, ci4)
                        ci4 += 1
                    if last:
                        tail_step()

                while last and mma_q:
                    m0, n0 = mma_q.pop(0)
                    emit_mma(m0, n0, ci4)
                    ci4 += 1
                    tail_step()

                if not last:
                    # defer the mmA stragglers into the next strip's L1
                    # phase (their ACT/DVE copy deps get ~5us of slack) and
                    # the gather + tap-sum creation right after them
                    for m0, n0 in mma_q:
                        mma_carry.append(
                            lambda m0=m0, n0=n0, c=ci4: emit_mma(m0, n0, c))
                        ci4 += 1
                    mma_q.clear()

                    def gather_pend(a5=a5, b5=b5r):
                        nrg = b5 - a5
                        emit_gather(a5, 0, nrg // 2)
                        emit_gather(a5, nrg // 2, nrg)
                        return l4b_chunks(a5, b5)
                else:
                    # keep the PE clock hot through the gather wait: idle
                    # resets the p-state ramp and the post-gap selectors
                    # would run at half clock
                    while chunks_f or gathered < nrl - DIRECT_TAIL:
                        tail_step(final=True)
                    pending = flushes

            # flush any remaining tap-sum emitters
            while pending:
                pending.pop(0)[1]()

    nc.finalize()
    return nc


_NC_CACHE = {}


def _pack_inputs(x, w1, b1, w2, b2, w3, b3, w4, b4):
    x = np.ascontiguousarray(np.asarray(x, dtype=np.float32))
    w1 = np.asarray(w1, dtype=np.float32)
    w2 = np.asarray(w2, dtype=np.float32)
    w3 = np.asarray(w3, dtype=np.float32)
    w4 = np.asarray(w4, dtype=np.float32)
    # w4a slabs: up@0-5, center@6-11, down@32-37, left@64-69, right@96-101
    w4a = np.zeros((HID_C, HID_C), np.float32)
    w4a[:, 0:OUT_C] = w4[:, :, 1].T          # up
    w4a[:, 6 : 6 + OUT_C] = w4[:, :, 0].T    # center
    w4a[:, 32 : 32 + OUT_C] = w4[:, :, 2].T  # down
    w4a[:, 64 : 64 + OUT_C] = w4[:, :, 3].T  # left
    w4a[:, 96 : 96 + OUT_C] = w4[:, :, 4].T  # right
    s6 = np.zeros((HID_C, OUT_C), np.float32)
    for base in (0, 6, 32, 64, 96):
        s6[base + np.arange(OUT_C), np.arange(OUT_C)] = 1.0
    s6 = s6.astype(ml_dtypes.bfloat16)
    sd = np.zeros((HID_C, 5, OUT_C), np.float32)
    for t, base in enumerate((0, 6, 32, 64, 96)):  # up,cen,dn,lf,rt slabs
        sd[base + np.arange(OUT_C), t, np.arange(OUT_C)] = 1.0
    sd = sd.astype(ml_dtypes.bfloat16)
    bias = np.zeros((HID_C, 4), np.float32)
    bias[:, 0] = np.asarray(b1, np.float32)
    bias[:, 1] = np.asarray(b2, np.float32)
    bias[:, 2] = np.asarray(b3, np.float32)
    bias[:OUT_C, 3] = np.asarray(b4, np.float32)
    common = {
        # w1p[t*6+ic, oc] = w1[oc, ic, t]
        "w1p": np.ascontiguousarray(w1.transpose(2, 1, 0).reshape(5 * IN_C, HID_C)),
        # w23p[ic, t, oc] = w2[oc, ic, t] (t<5) / w3[oc, ic, t-5]
        "w23p": np.ascontiguousarray(np.concatenate(
            [w2.transpose(1, 2, 0), w3.transpose(1, 2, 0)], axis=1)),
        "w4a": w4a,
        "s6": s6,
        "sd": sd,
        "bias": bias,
    }
    return x, common


def kernel(x, w1, b1, w2, b2, w3, b3, w4, b4):
    x, common = _pack_inputs(x, w1, b1, w2, b2, w3, b3, w4, b4)
    if "nc" not in _NC_CACHE:
        _NC_CACHE["nc"] = _build()
    nc = _NC_CACHE["nc"]
    in_maps = [dict(common, x=x[i]) for i in range(N_CORES)]
    res = bass_utils.run_bass_kernel_spmd(nc, in_maps, core_ids=list(range(N_CORES)))
    out = np.stack([res.results[i]["y"] for i in range(N_CORES)], axis=0)
    return out


# revision 97
# speedup vs baseline: 1.2417x; 1.0020x over previous
"""Trainium2 Bass kernel for 4-layer cross-stencil CNN.

Per-core: one image [6,256,256] (batch dim sharded across 8 cores).
conv(cross-5-stencil) = 5 channel-matmuls with spatially shifted rhs APs,
accumulated in PSUM. Channels on partitions, spatial (rows x cols) on the
free dim. fp32r matmuls (full PE rate at N>=256).

Rolling-retention pipeline: intermediate buffers h1/h2/h3/t5 are circular
row-slot buffers (S=28 slots, slot = row % 28). Each layer computes every
image row exactly once — no halo recompute. Per strip of R=24 rows the
layer blocks are skewed (L1 rows [a,a+24), L2 [a-2,a+22), L3 [a-4,a+20),
mmA [a-5,a+19), L4b [a-7,a+17)) so all reads hit rows already produced
this strip or retained from the previous one. Reads that cross the
circular wrap are split into <=2 accumulating matmuls; chunk boundaries
are chosen so writes never wrap. Image-boundary taps are handled by
narrowing the tap matmul to valid rows (no zero-pad rows needed).

L1 packs the 5 taps into K=30 via a 5-group pre-shifted x staging buffer
(one matmul per chunk), double-buffered across strips. L4 computes all 5
taps as one M=128 matmul (slabs at partitions 0/6/32/64/96), gathers the
shifted slabs into t5s by SBUF->SBUF DMA, and sums them with one K=128
selector matmul; the strip's tap-sum drains interleaved into the next
strip's L2 phase.
"""

import sys

sys.path.insert(0, "/opt/trn_rl_repo")

import ml_dtypes
import numpy as np

import concourse.bacc as bacc
import concourse.mybir as mybir
from concourse.tile import TileContext
from concourse import bass_utils

IN_C, HID_C, OUT_C = 6, 128, 6
B, H, W = 8, 256, 256
WP = W + 2  # padded width
R = 24  # rows per strip
DIRECT_TAIL = 4  # final rows computed gather-free from t5
S = 28  # circular row slots in h1/h2/h3/t5 (live windows are <= 27 rows)
XS = 28  # row slots in the x staging buffers
N_CORES = 8

f32 = mybir.dt.float32
f32r = mybir.dt.float32r
bf16 = mybir.dt.bfloat16
Add = mybir.AluOpType.add
Max = mybir.AluOpType.max
Relu = mybir.ActivationFunctionType.Relu
Ident = mybir.ActivationFunctionType.Identity

# tap order matches reference: 0=center, 1=up(x[h-1]), 2=down(x[h+1]),
# 3=left(x[w-1]), 4=right(x[w+1])


def _chunks(r0, r1):
    """2-row chunks of [r0,r1), never straddling a multiple of S."""
    r = r0
    while r < r1:
        n = min(2, r1 - r, (r // S + 1) * S - r)
        yield r, n
        r += n


def _runs(r0, r1):
    """Split [r0,r1) into runs of consecutive slots (break at S multiples)."""
    r = r0
    while r < r1:
        e = min(r1, (r // S + 1) * S)
        yield r, e
        r = e


def _build():
    nc = bacc.Bacc("TRN2", target_bir_lowering=False)

    x_d = nc.dram_tensor("x", [IN_C, H, W], f32, kind="ExternalInput")
    w1_d = nc.dram_tensor("w1p", [5 * IN_C, HID_C], f32, kind="ExternalInput")
    w23_d = nc.dram_tensor("w23p", [HID_C, 10, HID_C], f32, kind="ExternalInput")
    # w4a: all 5 taps as M=128 slabs: up@0-5, center@6-11, down@32-37,
    # left@64-69, right@96-101; zero elsewhere
    w4a_d = nc.dram_tensor("w4a", [HID_C, HID_C], f32, kind="ExternalInput")
    # s6: bf16 selector summing the 5 (pre-shifted) slabs of t5s
    s6_d = nc.dram_tensor("s6", [HID_C, OUT_C], bf16, kind="ExternalInput")
    # sd: per-slab selectors for the direct (gather-free) tail chunks
    sd_d = nc.dram_tensor("sd", [HID_C, 5, OUT_C], bf16, kind="ExternalInput")
    bias_d = nc.dram_tensor("bias", [HID_C, 4], f32, kind="ExternalInput")
    y_d = nc.dram_tensor("y", [OUT_C, H, W], f32, kind="ExternalOutput")

    strips = list(range(0, H, R))  # block anchors a = 24k

    with TileContext(nc) as tc:
        with (
            tc.tile_pool(name="const", bufs=1) as cpool,
            tc.tile_pool(name="bufs", bufs=1) as bpool,
            tc.tile_pool(name="io", bufs=2) as iopool,
            tc.tile_pool(name="psmain", bufs=7, space="PSUM") as pmain,
        ):
            # --- weights / biases (resident). HWDGE costs ~625ns per DMA
            # instruction and is a single serial device, so: w1 + strip-0 x
            # first (these gate the first matmul), everything else after,
            # with biases packed into one [128,4] tensor and w2+w3 into one
            # [128,10,128] tensor to cut the instruction count ---
            w1_sb = cpool.tile([5 * IN_C, HID_C], f32r)

            # --- persistent buffers ---
            # xst: 2 staging buffers, 5 tap-groups x 6ch, pre-shifted by DMA
            # placement. Strip k window: x rows [24k-2, 24k+26) at slot
            # r - 24k + 2 (center group); reading slot(r) in group g yields
            # x row r + dr_g at col w + dc_g.
            xst = [
                bpool.tile([5 * IN_C, XS, WP], f32r, name=f"xst{i}")
                for i in range(2)
            ]
            h1 = bpool.tile([HID_C, S, WP], f32r)
            h2 = bpool.tile([HID_C, S, WP], f32r)
            h3 = bpool.tile([HID_C, S, WP], f32r)
            # t5: tap partials (slabs up@0,cen@6,dn@32,lf@64,rt@96), bf16
            t5 = bpool.tile([HID_C, S, WP], bf16)
            # t5s: gathered pre-shifted taps; slot d = output row a5+d
            t5s = bpool.tile([HID_C, R, WP], bf16)

            # x staging pads, narrowed to exactly the cells that are read
            # but never DMA'd (disjoint from every placement, so the strip-0
            # DMAs carry no memset dependency): col 1 for the left group
            # (placed at cols [2,258)), col 256 for the right group (cols
            # [0,256)), and the up-group's slot 2 (x row -1) in buffer 0.
            # Cols 0/257 and other slots are never read.
            # (engine ops need partition base 0/32/64/96, so the ranges
            # start at 0; the extra cells are DMA-overwritten every strip)
            for xb in xst:
                nc.gpsimd.memset(xb[0:24, :, 1:2].bitcast(f32), 0.0)
                nc.gpsimd.memset(xb[0:30, :, 256:257].bitcast(f32), 0.0)
            nc.gpsimd.memset(xst[0][0:12, 2:3, :].bitcast(f32), 0.0)

            def x_dma(a, buf, split=False):
                """Load x rows [a-1, a+R+1) into staging buffer (5 shifted
                placements); center group slot = r - a + 2."""
                lo, hi = max(0, a - 1), min(H, a + R + 1)
                src = x_d[:, lo:hi, :].bitcast(f32r)
                o = lo - a  # >= -1
                # (partitions, slot offset of x row m = m - a + ofs, cols)
                for gi, (p0, ofs, c0) in enumerate((
                    (0, 2, 1),   # center
                    (6, 3, 1),   # up: read slot(r) -> x row r-1
                    (12, 1, 1),  # down
                    (18, 2, 2),  # left: read col w+1 -> x col w-1
                    (24, 2, 0),  # right
                )):
                    # split across the two DMA front-ends: HWDGE (sync) and
                    # SWDGE (gpsimd) process their ~625/1050ns-per-DMA fixed
                    # costs in parallel
                    eng = nc.gpsimd if split and gi >= 3 else nc.sync
                    eng.dma_start(
                        out=xst[buf][p0 : p0 + 6, o + ofs : hi - a + ofs, c0 : c0 + W],
                        in_=src,
                    )

            x_dma(0, 0)

            # p-state priming: the PE idles ~6us while the strip-0 x
            # placements stage, and its clock ramps 0.65->1.2->2.4GHz over
            # the first 3us of busy time. Dependency-free matmuls on a
            # never-written scratch tile fill the idle window so the real
            # work starts at full clock. Their psum is never read.
            dummy = bpool.tile([HID_C, 2, W], f32r)
            nc.gpsimd.memset(dummy[:, 0:1, :].bitcast(f32), 0.0)
            # trigger the 1.3us activation-table load now (ACT is idle);
            # otherwise it lands right before the first L1 relu copy.
            # Output goes to its own scratch (the verifier forbids non-f32r
            # writes into tiles that f32r matmuls consume).
            actwarm = bpool.tile([HID_C, 1], f32)
            nc.scalar.activation(
                actwarm[:, 0:1], dummy[:, 0:1, 0:1].bitcast(f32), Relu)
            for _ in range(12):
                psd = pmain.tile([HID_C, 1, W], f32, tag="ps")
                nc.tensor.matmul(
                    psd, dummy[:, 0, 0:HID_C], dummy[:, 0:1, :],
                    start=True, stop=True,
                )

            # w1 after the strip-0 placements: its 43ns transfer hides in
            # the HWDGE pipeline while the placements' 427ns transfers
            # overlap later HWDGE processing
            nc.sync.dma_start(out=w1_sb, in_=w1_d[:, :].bitcast(f32r))

            bias_sb = cpool.tile([HID_C, 4], f32)
            nc.sync.dma_start(out=bias_sb, in_=bias_d[:, :])
            b1_sb = bias_sb[:, 0:1]
            b2_sb = bias_sb[:, 1:2]
            b3_sb = bias_sb[:, 2:3]
            b4_sb = bias_sb[0:OUT_C, 3:4]
            w23_sb = cpool.tile([HID_C, 10, HID_C], f32r)
            nc.sync.dma_start(out=w23_sb, in_=w23_d[:, :, :].bitcast(f32r))
            w2_sb = w23_sb[:, 0:5, :]
            w3_sb = w23_sb[:, 5:10, :]
            w4a_sb = cpool.tile([HID_C, HID_C], f32r)
            nc.sync.dma_start(out=w4a_sb, in_=w4a_d[:, :].bitcast(f32r))
            s6_sb = cpool.tile([HID_C, OUT_C], bf16)
            nc.sync.dma_start(out=s6_sb, in_=s6_d[:, :])
            sd_sb = cpool.tile([HID_C, 5, OUT_C], bf16)
            nc.sync.dma_start(out=sd_sb, in_=sd_d[:, :, :])

            # h/t5 pads: cols 0 and 257 are read (left/right taps) but
            # never written; zero once — rolling writes only touch [1,257).
            for _t in (h1, h2, h3):
                nc.vector.memset(_t[:, :, 0:1].bitcast(f32), 0.0)
                nc.vector.memset(_t[:, :, 257:258].bitcast(f32), 0.0)
            nc.vector.memset(t5[:, :, 0:1], 0.0)
            nc.vector.memset(t5[:, :, 257:258], 0.0)
            # t5s non-slab partitions are read by the selector matmul
            # (0 * garbage must not be NaN) — zero everything once
            nc.vector.memset(t5s[:, :, :], 0.0)

            def conv_chunk(ps, w_sb, src, r0, n):
                """Cross-stencil accumulation for output rows [r0, r0+n).
                Center tap first (covers the whole psum tile -> start=True
                zeroes it); up/down taps clamp to [0,H) and split at the
                circular wrap."""
                parts = []  # (tap, psum_off, rows, slot)
                for tap, dr in ((1, -1), (2, 1), (3, 0), (4, 0)):
                    lo = max(0, r0 + dr)
                    hi = min(H, r0 + n + dr)
                    for ra, rb in _runs(lo, hi):
                        parts.append((tap, ra - dr - r0, rb - ra, ra % S))
                cols = {0: (1, 1 + W), 1: (1, 1 + W), 2: (1, 1 + W),
                        3: (0, W), 4: (2, 2 + W)}
                nc.tensor.matmul(
                    ps, w_sb[:, 0, :], src[:, r0 % S : r0 % S + n, 1 : 1 + W],
                    start=True, stop=False,
                )
                for i, (tap, off, m, sl) in enumerate(parts):
                    c0, c1 = cols[tap]
                    nc.tensor.matmul(
                        ps[:, off : off + m, :],
                        w_sb[:, tap, :],
                        src[:, sl : sl + m, c0:c1],
                        start=False, stop=(i == len(parts) - 1),
                    )

            # slab parts / tap row-offset / col offsets (dst, src)
            GAT = ((0, -1, 1, 1), (6, 0, 1, 1), (32, 1, 1, 1),
                   (64, 0, 2, 1), (96, 0, 1, 2))

            def emit_gather(a5, d0, d1, split=False):
                """Gather t5 slab rows into pre-shifted t5s slots [d0,d1).
                With split=True alternate slabs between the HWDGE (sync) and
                SWDGE (gpsimd) paths so the two DMA front-ends overlap."""
                for gi, (p0, dr, cd, cs) in enumerate(GAT):
                    eng = nc.gpsimd if split and gi % 2 else nc.sync
                    lo = max(0, a5 + d0 + dr)
                    hi = min(H, a5 + d1 + dr)
                    for ra, rb in _runs(lo, hi):
                        dd = ra - dr - a5
                        m = rb - ra
                        eng.dma_start(
                            out=t5s[p0 : p0 + 6, dd : dd + m, cd : cd + W],
                            in_=t5[p0 : p0 + 6, ra % S : ra % S + m, cs : cs + W],
                        )

            def l4b_chunks(a5, b5, tail=False):
                """Deferred emitters: one K=128 selector matmul per chunk
                sums the 5 pre-shifted slabs of t5s; bias on DVE into a
                strip-wide staging tile; two half-strip DMAs out."""
                nr = b5 - a5
                chunks = list(_chunks(a5, b5))
                # split into flush groups, each with its own staging tile
                nparts = 4 if tail else 2
                halves = []
                prev_i = 0
                for j in range(1, nparts + 1):
                    tgt = nr * j // nparts
                    i = next((i for i, (rr, n) in enumerate(chunks)
                              if rr + n - a5 >= tgt), len(chunks) - 1)
                    if i + 1 > prev_i:
                        halves.append(chunks[prev_i : i + 1])
                        prev_i = i + 1
                out = []  # (needed t5s rows, emitter) pairs
                dt0 = b5 - DIRECT_TAIL if tail else b5
                for half in halves:
                    if not half:
                        continue
                    lo = half[0][0] - a5
                    hi = half[-1][0] + half[-1][1] - a5
                    yst = iopool.tile([OUT_C, hi - lo, W], f32, tag="yst")
                    for ei, (rr, n) in enumerate(half):

                        def emit(rr=rr, n=n, a5=a5, yst=yst, lo=lo, ei=ei,
                                 tail=tail):
                            d = rr - a5
                            ps = pmain.tile([OUT_C, n, W], f32, tag="ps")
                            if tail and rr >= a5 + (dt0 - a5):
                                # gather-free: per-slab selectors read t5
                                # directly with the slab's row/col shift
                                parts = []
                                for tp, dr, c0 in ((0, -1, 1), (2, 1, 1),
                                                   (3, 0, 0), (4, 0, 2)):
                                    pl = max(0, rr + dr)
                                    ph = min(H, rr + n + dr)
                                    for ra, rb in _runs(pl, ph):
                                        parts.append(
                                            (tp, ra - dr - rr, rb - ra,
                                             ra % S, c0))
                                nc.tensor.matmul(
                                    ps, sd_sb[:, 1, :],
                                    t5[:, rr % S : rr % S + n, 1 : 1 + W],
                                    start=True, stop=False,
                                )
                                for i, (tp, off, m, sl, c0) in enumerate(parts):
                                    nc.tensor.matmul(
                                        ps[:, off : off + m, :], sd_sb[:, tp, :],
                                        t5[:, sl : sl + m, c0 : c0 + W],
                                        start=False, stop=(i == len(parts) - 1),
                                    )
                            else:
                                nc.tensor.matmul(
                                    ps, s6_sb[:, :], t5s[:, d : d + n, 1 : 1 + W],
                                    start=True, stop=True,
                                )
                            if tail and ei % 2:
                                # ACT is idle at the drain; alternating
                                # halves the serialized add chain
                                nc.scalar.activation(
                                    yst[:, d - lo : d - lo + n, :], ps,
                                    Ident, bias=b4_sb)
                            else:
                                nc.vector.tensor_scalar_add(
                                    yst[:, d - lo : d - lo + n, :], ps, b4_sb)

                        need = (min(rr + n - a5, dt0 - a5) if tail
                                else rr + n - a5)
                        out.append((need, emit))

                    def flush(lo=lo, hi=hi, a5=a5, yst=yst):
                        nc.sync.dma_start(
                            out=y_d[:, a5 + lo : a5 + hi, :],
                            in_=yst[:, :, :])

                    out.append((hi, flush))
                return out

            pending = []  # tap-sum emitters from the previous strip
            mma_carry = []  # end-of-strip mmA chunks deferred into next L1
            gather_pend = None  # deferred gather + l4b creation
            for k, a in enumerate(strips):
                last = k == len(strips) - 1
                # skewed layer blocks for this strip
                a1, b1r = a, min(H, a + R)
                a2, b2r = max(0, a - 2), min(H, a + R - 2)
                a3, b3r = max(0, a - 4), min(H, a + R - 4)
                a4, b4r = max(0, a - 5), min(H, a + R - 5)
                a5, b5r = max(0, a - 7), min(H, a + R - 7)
                if last:
                    b2r = b3r = b4r = b5r = H

                # prefetch next strip's x into the other staging buffer
                if not last:
                    a_n = a + R
                    if a_n + R >= H:
                        # final strip: slots past the image tail keep stale
                        # data from two strips ago — zero them before the DMA
                        tail0 = min(H, a_n + R + 1) - a_n + 1
                        nc.gpsimd.memset(
                            xst[(k + 1) % 2][:, tail0:XS, :].bitcast(f32), 0.0)
                    x_dma(a_n, (k + 1) % 2)

                # --- L1: one K=30 matmul per chunk; copies alternate
                # DVE/ACT ---
                # --- L1 + L2 merged: the L1 phase is copy-chain-bound
                # (213ns of PE per chunk vs ~635ns of DVE/ACT copy), so
                # interleave L2 convs (copy-independent PE work, data-ready
                # at 2 chunks of lag) to absorb the psum-rotation waits ---
                def do_l2(rr, n):
                    ps = pmain.tile([HID_C, n, W], f32, tag="ps")
                    conv_chunk(ps, w2_sb, h1, rr, n)
                    sl = rr % S
                    nc.scalar.activation(
                        h2[:, sl : sl + n, 1 : 1 + W], ps, Relu, bias=b2_sb
                    )
                    if pending:
                        pending.pop(0)[1]()

                xb = xst[k % 2]
                l2c = list(_chunks(a2, b2r))
                ci = 0
                for rr, n in _chunks(a1, b1r):
                    ps = pmain.tile([HID_C, n, W], f32, tag="ps")
                    s0 = rr - a + 2
                    nc.tensor.matmul(
                        ps, w1_sb[:, :], xb[:, s0 : s0 + n, 1 : 1 + W],
                        start=True, stop=True,
                    )
                    sl = rr % S
                    if ci % 2 == 0:
                        nc.vector.tensor_scalar(
                            h1[:, sl : sl + n, 1 : 1 + W], ps, b1_sb, 0.0, Add, Max
                        )
                    else:
                        nc.scalar.activation(
                            h1[:, sl : sl + n, 1 : 1 + W], ps, Relu, bias=b1_sb
                        )
                    ci += 1
                    if mma_carry and ci >= 2:
                        mma_carry.pop(0)()
                    # gather(k-1) reads t5 rows written by the carried mmA
                    # chunks: it must not be emitted before they drain
                    if ci >= 3 and not mma_carry and gather_pend is not None:
                        pending = gather_pend()
                        gather_pend = None
                    if ci >= 6 and l2c:
                        do_l2(*l2c.pop(0))
                while mma_carry:
                    mma_carry.pop(0)()
                if gather_pend is not None:
                    pending = gather_pend()
                    gather_pend = None
                while l2c:
                    do_l2(*l2c.pop(0))
                while pending:
                    pending.pop(0)[1]()

                # --- L3: reads h2; mmA interleaved so t5 fills (and the
                # gather can start) as h3 rows land ---
                mma_q = list(_chunks(a4, b4r))

                def emit_mma(rr, n, ci):
                    ps = pmain.tile([HID_C, n, W], f32, tag="ps")
                    sl = rr % S
                    nc.tensor.matmul(
                        ps, w4a_sb[:, :], h3[:, sl : sl + n, 1 : 1 + W],
                        start=True, stop=True,
                    )
                    if ci % 2 == 0:
                        nc.vector.tensor_copy(t5[:, sl : sl + n, 1 : 1 + W], ps)
                    else:
                        nc.scalar.activation(
                            t5[:, sl : sl + n, 1 : 1 + W], ps, Ident)

                if k == 0:
                    # row 0 up-slab source is row -1: zero that dest
                    nc.gpsimd.memset(t5s[0:6, 0:1, :], 0.0)
                if last:
                    # row 255 down-slab source is row 256
                    nc.gpsimd.memset(t5s[32:38, b5r - 1 - a5 : b5r - a5, :], 0.0)

                if last:
                    # final strip: interleave gather / tap-sum into the L3
                    # drain so the tail doesn't serialize; y flushes (whose
                    # deps resolve last) go at the very end. Selector pops
                    # lag one gather batch so they never block the in-order
                    # PE queue while their gather DMAs are still in flight.
                    allp = l4b_chunks(a5, b5r, tail=True)
                    chunks_f = [p for p in allp if p[1].__name__ == "emit"]
                    flushes = [p for p in allp if p[1].__name__ == "flush"]
                    gathered = 0
                    gprev = 0
                    gage = 0  # tail_steps since the last gather batch
                    nrl = b5r - a5

                def tail_step(final=False):
                    nonlocal gathered, gprev, gage
                    nrg = nrl - DIRECT_TAIL
                    ready = (nrg if not mma_q else
                             max(0, min(nrg, mma_q[0][0] - 1 - a5)))
                    if ready - gathered >= 12 or (ready == nrg and ready > gathered):
                        gprev = gathered
                        emit_gather(a5, gathered, ready, split=True)
                        gathered = ready
                        gage = 0
                    gage += 1
                    # a gather's DMAs are safely in flight after ~2 more L3/
                    # mmA chunks of PE work; then its selectors won't block
                    # the in-order PE queue
                    lim = gathered if (final and gathered == nrl - DIRECT_TAIL) \
                        or gage > 5 else gprev
                    while chunks_f and chunks_f[0][0] <= lim:
                        chunks_f.pop(0)[1]()

                ci4 = 0
                for rr, n in _chunks(a3, b3r):
                    ps = pmain.tile([HID_C, n, W], f32, tag="ps")
                    conv_chunk(ps, w3_sb, h2, rr, n)
                    sl = rr % S
                    nc.scalar.activation(
                        h3[:, sl : sl + n, 1 : 1 + W], ps, Relu, bias=b3_sb
                    )
                    rr_next = rr + n
                    while mma_q and mma_q[0][0] + mma_q[0][1] + 2 <= rr_next:
                        m0, n0 = mma_q.pop(0)
                        emit_mma(m0, n0, ci4)
                        ci4 += 1
                    if last:
                        tail_step()

                while last and mma_q:
                    m0, n0 = mma_q.pop(0)
                    emit_mma(m0, n0, ci4)
                    ci4 += 1
                    tail_step()

                if not last:
                    # defer the mmA stragglers into the next strip's L1
                    # phase (their ACT/DVE copy deps get ~5us of slack) and
                    # the gather + tap-sum creation right after them
                    for m0, n0 in mma_q:
                        mma_carry.append(
                            lambda m0=m0, n0=n0, c=ci4: emit_mma(m0, n0, c))
                        ci4 += 1
                    mma_q.clear()

                    def gather_pend(a5=a5, b5=b5r):
                        nrg = b5 - a5
                        emit_gather(a5, 0, nrg // 2)
                        emit_gather(a5, nrg // 2, nrg)
                        return l4b_chunks(a5, b5)
                else:
                    # keep the PE clock hot through the gather wait: idle
                    # resets the p-state ramp and the post-gap selectors
                    # would run at half clock
                    while chunks_f or gathered < nrl - DIRECT_TAIL:
                        tail_step(final=True)
                    pending = flushes

            # flush any remaining tap-sum emitters
            while pending:
                pending.pop(0)[1]()

    nc.finalize()
    return nc


_NC_CACHE = {}


def _pack_inputs(x, w1, b1, w2, b2, w3, b3, w4, b4):
    x = np.ascontiguousarray(np.asarray(x, dtype=np.float32))
    w1 = np.asarray(w1, dtype=np.float32)
    w2 = np.asarray(w2, dtype=np.float32)
    w3 = np.asarray(w3, dtype=np.float32)
    w4 = np.asarray(w4, dtype=np.float32)
    # w4a slabs: up@0-5, center@6-11, down@32-37, left@64-69, right@96-101
    w4a = np.zeros((HID_C, HID_C), np.float32)
    w4a[:, 0:OUT_C] = w4[:, :, 1].T          # up
    w4a[:, 6 : 6 + OUT_C] = w4[:, :, 0].T    # center
    w4a[:, 32 : 32 + OUT_C] = w4[:, :, 2].T  # down
    w4a[:, 64 : 64 + OUT_C] = w4[:, :, 3].T  # left
    w4a[:, 96 : 96 + OUT_C] = w4[:, :, 4].T  # right
    s6 = np.zeros((HID_C, OUT_C), np.float32)
    for base in (0, 6, 32, 64, 96):
        s6[base + np.arange(OUT_C), np.arange(OUT_C)] = 1.0
    s6 = s6.astype(ml_dtypes.bfloat16)
    sd = np.zeros((HID_C, 5, OUT_C), np.float32)
    for t, base in enumerate((0, 6, 32, 64, 96)):  # up,cen,dn,lf,rt slabs
        sd[base + np.arange(OUT_C), t, np.arange(OUT_C)] = 1.0
    sd = sd.astype(ml_dtypes.bfloat16)
    bias = np.zeros((HID_C, 4), np.float32)
    bias[:, 0] = np.asarray(b1, np.float32)
    bias[:, 1] = np.asarray(b2, np.float32)
    bias[:, 2] = np.asarray(b3, np.float32)
    bias[:OUT_C, 3] = np.asarray(b4, np.float32)
    common = {
        # w1p[t*6+ic, oc] = w1[oc, ic, t]
        "w1p": np.ascontiguousarray(w1.transpose(2, 1, 0).reshape(5 * IN_C, HID_C)),
        # w23p[ic, t, oc] = w2[oc, ic, t] (t<5) / w3[oc, ic, t-5]
        "w23p": np.ascontiguousarray(np.concatenate(
            [w2.transpose(1, 2, 0), w3.transpose(1, 2, 0)], axis=1)),
        "w4a": w4a,
        "s6": s6,
        "sd": sd,
        "bias": bias,
    }
    return x, common


def kernel(x, w1, b1, w2, b2, w3, b3, w4, b4):
    x, common = _pack_inputs(x, w1, b1, w2, b2, w3, b3, w4, b4)
    if "nc" not in _NC_CACHE:
        _NC_CACHE["nc"] = _build()
    nc = _NC_CACHE["nc"]
    in_maps = [dict(common, x=x[i]) for i in range(N_CORES)]
    res = bass_utils.run_bass_kernel_spmd(nc, in_maps, core_ids=list(range(N_CORES)))
    out = np.stack([res.results[i]["y"] for i in range(N_CORES)], axis=0)
    return out


# revision 98
# speedup vs baseline: 1.2424x; 1.0006x over previous
"""Trainium2 Bass kernel for 4-layer cross-stencil CNN.

Per-core: one image [6,256,256] (batch dim sharded across 8 cores).
conv(cross-5-stencil) = 5 channel-matmuls with spatially shifted rhs APs,
accumulated in PSUM. Channels on partitions, spatial (rows x cols) on the
free dim. fp32r matmuls (full PE rate at N>=256).

Rolling-retention pipeline: intermediate buffers h1/h2/h3/t5 are circular
row-slot buffers (S=28 slots, slot = row % 28). Each layer computes every
image row exactly once — no halo recompute. Per strip of R=24 rows the
layer blocks are skewed (L1 rows [a,a+24), L2 [a-2,a+22), L3 [a-4,a+20),
mmA [a-5,a+19), L4b [a-7,a+17)) so all reads hit rows already produced
this strip or retained from the previous one. Reads that cross the
circular wrap are split into <=2 accumulating matmuls; chunk boundaries
are chosen so writes never wrap. Image-boundary taps are handled by
narrowing the tap matmul to valid rows (no zero-pad rows needed).

L1 packs the 5 taps into K=30 via a 5-group pre-shifted x staging buffer
(one matmul per chunk), double-buffered across strips. L4 computes all 5
taps as one M=128 matmul (slabs at partitions 0/6/32/64/96), gathers the
shifted slabs into t5s by SBUF->SBUF DMA, and sums them with one K=128
selector matmul; the strip's tap-sum drains interleaved into the next
strip's L2 phase.
"""

import sys

sys.path.insert(0, "/opt/trn_rl_repo")

import ml_dtypes
import numpy as np

import concourse.bacc as bacc
import concourse.mybir as mybir
from concourse.tile import TileContext
from concourse import bass_utils

IN_C, HID_C, OUT_C = 6, 128, 6
B, H, W = 8, 256, 256
WP = W + 2  # padded width
R = 24  # rows per strip
DIRECT_TAIL = 4  # final rows computed gather-free from t5
S = 28  # circular row slots in h1/h2/h3/t5 (live windows are <= 27 rows)
XS = 28  # row slots in the x staging buffers
N_CORES = 8

f32 = mybir.dt.float32
f32r = mybir.dt.float32r
bf16 = mybir.dt.bfloat16
Add = mybir.AluOpType.add
Max = mybir.AluOpType.max
Relu = mybir.ActivationFunctionType.Relu
Ident = mybir.ActivationFunctionType.Identity

# tap order matches reference: 0=center, 1=up(x[h-1]), 2=down(x[h+1]),
# 3=left(x[w-1]), 4=right(x[w+1])


def _chunks(r0, r1):
    """2-row chunks of [r0,r1), never straddling a multiple of S."""
    r = r0
    while r < r1:
        n = min(2, r1 - r, (r // S + 1) * S - r)
        yield r, n
        r += n


def _runs(r0, r1):
    """Split [r0,r1) into runs of consecutive slots (break at S multiples)."""
    r = r0
    while r < r1:
        e = min(r1, (r // S + 1) * S)
        yield r, e
        r = e


def _build():
    nc = bacc.Bacc("TRN2", target_bir_lowering=False)

    x_d = nc.dram_tensor("x", [IN_C, H, W], f32, kind="ExternalInput")
    w1_d = nc.dram_tensor("w1p", [5 * IN_C, HID_C], f32, kind="ExternalInput")
    w23_d = nc.dram_tensor("w23p", [HID_C, 10, HID_C], f32, kind="ExternalInput")
    # w4a: all 5 taps as M=128 slabs: up@0-5, center@6-11, down@32-37,
    # left@64-69, right@96-101; zero elsewhere
    w4a_d = nc.dram_tensor("w4a", [HID_C, HID_C], f32, kind="ExternalInput")
    # s6: bf16 selector summing the 5 (pre-shifted) slabs of t5s
    s6_d = nc.dram_tensor("s6", [HID_C, OUT_C], bf16, kind="ExternalInput")
    # sd: per-slab selectors for the direct (gather-free) tail chunks
    sd_d = nc.dram_tensor("sd", [HID_C, 5, OUT_C], bf16, kind="ExternalInput")
    bias_d = nc.dram_tensor("bias", [HID_C, 4], f32, kind="ExternalInput")
    y_d = nc.dram_tensor("y", [OUT_C, H, W], f32, kind="ExternalOutput")

    strips = list(range(0, H, R))  # block anchors a = 24k

    with TileContext(nc) as tc:
        with (
            tc.tile_pool(name="const", bufs=1) as cpool,
            tc.tile_pool(name="bufs", bufs=1) as bpool,
            tc.tile_pool(name="io", bufs=2) as iopool,
            tc.tile_pool(name="psmain", bufs=7, space="PSUM") as pmain,
        ):
            # --- weights / biases (resident). HWDGE costs ~625ns per DMA
            # instruction and is a single serial device, so: w1 + strip-0 x
            # first (these gate the first matmul), everything else after,
            # with biases packed into one [128,4] tensor and w2+w3 into one
            # [128,10,128] tensor to cut the instruction count ---
            w1_sb = cpool.tile([5 * IN_C, HID_C], f32r)

            # --- persistent buffers ---
            # xst: 2 staging buffers, 5 tap-groups x 6ch, pre-shifted by DMA
            # placement. Strip k window: x rows [24k-2, 24k+26) at slot
            # r - 24k + 2 (center group); reading slot(r) in group g yields
            # x row r + dr_g at col w + dc_g.
            xst = [
                bpool.tile([5 * IN_C, XS, WP], f32r, name=f"xst{i}")
                for i in range(2)
            ]
            h1 = bpool.tile([HID_C, S, WP], f32r)
            h2 = bpool.tile([HID_C, S, WP], f32r)
            h3 = bpool.tile([HID_C, S, WP], f32r)
            # t5: tap partials (slabs up@0,cen@6,dn@32,lf@64,rt@96), bf16
            t5 = bpool.tile([HID_C, S, WP], bf16)
            # t5s: gathered pre-shifted taps; slot d = output row a5+d
            t5s = bpool.tile([HID_C, R, WP], bf16)

            # x staging pads, narrowed to exactly the cells that are read
            # but never DMA'd (disjoint from every placement, so the strip-0
            # DMAs carry no memset dependency): col 1 for the left group
            # (placed at cols [2,258)), col 256 for the right group (cols
            # [0,256)), and the up-group's slot 2 (x row -1) in buffer 0.
            # Cols 0/257 and other slots are never read.
            # (engine ops need partition base 0/32/64/96, so the ranges
            # start at 0; the extra cells are DMA-overwritten every strip)
            for xb in xst:
                nc.gpsimd.memset(xb[0:24, :, 1:2].bitcast(f32), 0.0)
                nc.gpsimd.memset(xb[0:30, :, 256:257].bitcast(f32), 0.0)
            nc.gpsimd.memset(xst[0][0:12, 2:3, :].bitcast(f32), 0.0)

            def x_dma(a, buf, split=False):
                """Load x rows [a-1, a+R+1) into staging buffer (5 shifted
                placements); center group slot = r - a + 2."""
                lo, hi = max(0, a - 1), min(H, a + R + 1)
                src = x_d[:, lo:hi, :].bitcast(f32r)
                o = lo - a  # >= -1
                # (partitions, slot offset of x row m = m - a + ofs, cols)
                for gi, (p0, ofs, c0) in enumerate((
                    (0, 2, 1),   # center
                    (6, 3, 1),   # up: read slot(r) -> x row r-1
                    (12, 1, 1),  # down
                    (18, 2, 2),  # left: read col w+1 -> x col w-1
                    (24, 2, 0),  # right
                )):
                    # split across the two DMA front-ends: HWDGE (sync) and
                    # SWDGE (gpsimd) process their ~625/1050ns-per-DMA fixed
                    # costs in parallel
                    eng = nc.gpsimd if split and gi >= 3 else nc.sync
                    eng.dma_start(
                        out=xst[buf][p0 : p0 + 6, o + ofs : hi - a + ofs, c0 : c0 + W],
                        in_=src,
                    )

            x_dma(0, 0)

            # p-state priming: the PE idles ~6us while the strip-0 x
            # placements stage, and its clock ramps 0.65->1.2->2.4GHz over
            # the first 3us of busy time. Dependency-free matmuls on a
            # never-written scratch tile fill the idle window so the real
            # work starts at full clock. Their psum is never read.
            dummy = bpool.tile([HID_C, 2, W], f32r)
            nc.gpsimd.memset(dummy[:, 0:1, :].bitcast(f32), 0.0)
            # trigger the 1.3us activation-table load now (ACT is idle);
            # otherwise it lands right before the first L1 relu copy.
            # Output goes to its own scratch (the verifier forbids non-f32r
            # writes into tiles that f32r matmuls consume).
            actwarm = bpool.tile([HID_C, 1], f32)
            nc.scalar.activation(
                actwarm[:, 0:1], dummy[:, 0:1, 0:1].bitcast(f32), Relu)
            for _ in range(12):
                psd = pmain.tile([HID_C, 1, W], f32, tag="ps")
                nc.tensor.matmul(
                    psd, dummy[:, 0, 0:HID_C], dummy[:, 0:1, :],
                    start=True, stop=True,
                )

            # w1 after the strip-0 placements: its 43ns transfer hides in
            # the HWDGE pipeline while the placements' 427ns transfers
            # overlap later HWDGE processing
            nc.sync.dma_start(out=w1_sb, in_=w1_d[:, :].bitcast(f32r))

            bias_sb = cpool.tile([HID_C, 4], f32)
            nc.sync.dma_start(out=bias_sb, in_=bias_d[:, :])
            b1_sb = bias_sb[:, 0:1]
            b2_sb = bias_sb[:, 1:2]
            b3_sb = bias_sb[:, 2:3]
            b4_sb = bias_sb[0:OUT_C, 3:4]
            w23_sb = cpool.tile([HID_C, 10, HID_C], f32r)
            nc.sync.dma_start(out=w23_sb, in_=w23_d[:, :, :].bitcast(f32r))
            w2_sb = w23_sb[:, 0:5, :]
            w3_sb = w23_sb[:, 5:10, :]
            w4a_sb = cpool.tile([HID_C, HID_C], f32r)
            nc.sync.dma_start(out=w4a_sb, in_=w4a_d[:, :].bitcast(f32r))
            s6_sb = cpool.tile([HID_C, OUT_C], bf16)
            nc.sync.dma_start(out=s6_sb, in_=s6_d[:, :])
            sd_sb = cpool.tile([HID_C, 5, OUT_C], bf16)
            nc.sync.dma_start(out=sd_sb, in_=sd_d[:, :, :])

            # h/t5 pads: cols 0 and 257 are read (left/right taps) but
            # never written; zero once — rolling writes only touch [1,257).
            for _t in (h1, h2, h3):
                nc.vector.memset(_t[:, :, 0:1].bitcast(f32), 0.0)
                nc.vector.memset(_t[:, :, 257:258].bitcast(f32), 0.0)
            nc.vector.memset(t5[:, :, 0:1], 0.0)
            nc.vector.memset(t5[:, :, 257:258], 0.0)
            # t5s non-slab partitions are read by the selector matmul
            # (0 * garbage must not be NaN) — zero everything once
            nc.vector.memset(t5s[:, :, :], 0.0)

            def conv_chunk(ps, w_sb, src, r0, n):
                """Cross-stencil accumulation for output rows [r0, r0+n).
                Center tap first (covers the whole psum tile -> start=True
                zeroes it); up/down taps clamp to [0,H) and split at the
                circular wrap."""
                parts = []  # (tap, psum_off, rows, slot)
                for tap, dr in ((1, -1), (2, 1), (3, 0), (4, 0)):
                    lo = max(0, r0 + dr)
                    hi = min(H, r0 + n + dr)
                    for ra, rb in _runs(lo, hi):
                        parts.append((tap, ra - dr - r0, rb - ra, ra % S))
                cols = {0: (1, 1 + W), 1: (1, 1 + W), 2: (1, 1 + W),
                        3: (0, W), 4: (2, 2 + W)}
                nc.tensor.matmul(
                    ps, w_sb[:, 0, :], src[:, r0 % S : r0 % S + n, 1 : 1 + W],
                    start=True, stop=False,
                )
                for i, (tap, off, m, sl) in enumerate(parts):
                    c0, c1 = cols[tap]
                    nc.tensor.matmul(
                        ps[:, off : off + m, :],
                        w_sb[:, tap, :],
                        src[:, sl : sl + m, c0:c1],
                        start=False, stop=(i == len(parts) - 1),
                    )

            # slab parts / tap row-offset / col offsets (dst, src)
            GAT = ((0, -1, 1, 1), (6, 0, 1, 1), (32, 1, 1, 1),
                   (64, 0, 2, 1), (96, 0, 1, 2))

            def emit_gather(a5, d0, d1, split=False):
                """Gather t5 slab rows into pre-shifted t5s slots [d0,d1).
                With split=True alternate slabs between the HWDGE (sync) and
                SWDGE (gpsimd) paths so the two DMA front-ends overlap."""
                for gi, (p0, dr, cd, cs) in enumerate(GAT):
                    eng = nc.gpsimd if split and gi % 2 else nc.sync
                    lo = max(0, a5 + d0 + dr)
                    hi = min(H, a5 + d1 + dr)
                    for ra, rb in _runs(lo, hi):
                        dd = ra - dr - a5
                        m = rb - ra
                        eng.dma_start(
                            out=t5s[p0 : p0 + 6, dd : dd + m, cd : cd + W],
                            in_=t5[p0 : p0 + 6, ra % S : ra % S + m, cs : cs + W],
                        )

            def l4b_chunks(a5, b5, tail=False):
                """Deferred emitters: one K=128 selector matmul per chunk
                sums the 5 pre-shifted slabs of t5s; bias on DVE into a
                strip-wide staging tile; two half-strip DMAs out."""
                nr = b5 - a5
                chunks = list(_chunks(a5, b5))
                # split into flush groups, each with its own staging tile
                nparts = 4 if tail else 2
                halves = []
                prev_i = 0
                for j in range(1, nparts + 1):
                    tgt = nr * j // nparts
                    i = next((i for i, (rr, n) in enumerate(chunks)
                              if rr + n - a5 >= tgt), len(chunks) - 1)
                    if i + 1 > prev_i:
                        halves.append(chunks[prev_i : i + 1])
                        prev_i = i + 1
                out = []  # (needed t5s rows, emitter) pairs
                dt0 = b5 - DIRECT_TAIL if tail else b5
                for half in halves:
                    if not half:
                        continue
                    lo = half[0][0] - a5
                    hi = half[-1][0] + half[-1][1] - a5
                    yst = iopool.tile([OUT_C, hi - lo, W], f32, tag="yst")
                    for ei, (rr, n) in enumerate(half):

                        def emit(rr=rr, n=n, a5=a5, yst=yst, lo=lo, ei=ei,
                                 tail=tail):
                            d = rr - a5
                            ps = pmain.tile([OUT_C, n, W], f32, tag="ps")
                            if tail and rr >= a5 + (dt0 - a5):
                                # gather-free: per-slab selectors read t5
                                # directly with the slab's row/col shift
                                parts = []
                                for tp, dr, c0 in ((0, -1, 1), (2, 1, 1),
                                                   (3, 0, 0), (4, 0, 2)):
                                    pl = max(0, rr + dr)
                                    ph = min(H, rr + n + dr)
                                    for ra, rb in _runs(pl, ph):
                                        parts.append(
                                            (tp, ra - dr - rr, rb - ra,
                                             ra % S, c0))
                                nc.tensor.matmul(
                                    ps, sd_sb[:, 1, :],
                                    t5[:, rr % S : rr % S + n, 1 : 1 + W],
                                    start=True, stop=False,
                                )
                                for i, (tp, off, m, sl, c0) in enumerate(parts):
                                    nc.tensor.matmul(
                                        ps[:, off : off + m, :], sd_sb[:, tp, :],
                                        t5[:, sl : sl + m, c0 : c0 + W],
                                        start=False, stop=(i == len(parts) - 1),
                                    )
                            else:
                                nc.tensor.matmul(
                                    ps, s6_sb[:, :], t5s[:, d : d + n, 1 : 1 + W],
                                    start=True, stop=True,
                                )
                            if tail and ei % 2:
                                # ACT is idle at the drain; alternating
                                # halves the serialized add chain
                                nc.scalar.activation(
                                    yst[:, d - lo : d - lo + n, :], ps,
                                    Ident, bias=b4_sb)
                            else:
                                nc.vector.tensor_scalar_add(
                                    yst[:, d - lo : d - lo + n, :], ps, b4_sb)

                        need = (min(rr + n - a5, dt0 - a5) if tail
                                else rr + n - a5)
                        out.append((need, emit))

                    def flush(lo=lo, hi=hi, a5=a5, yst=yst):
                        nc.sync.dma_start(
                            out=y_d[:, a5 + lo : a5 + hi, :],
                            in_=yst[:, :, :])

                    out.append((hi, flush))
                return out

            pending = []  # tap-sum emitters from the previous strip
            mma_carry = []  # end-of-strip mmA chunks deferred into next L1
            gather_pend = None  # deferred gather + l4b creation
            for k, a in enumerate(strips):
                last = k == len(strips) - 1
                # skewed layer blocks for this strip
                a1, b1r = a, min(H, a + R)
                a2, b2r = max(0, a - 2), min(H, a + R - 2)
                a3, b3r = max(0, a - 4), min(H, a + R - 4)
                a4, b4r = max(0, a - 5), min(H, a + R - 5)
                a5, b5r = max(0, a - 7), min(H, a + R - 7)
                if last:
                    b2r = b3r = b4r = b5r = H

                # prefetch next strip's x into the other staging buffer
                if not last:
                    a_n = a + R
                    if a_n + R >= H:
                        # final strip: slots past the image tail keep stale
                        # data from two strips ago — zero them before the DMA
                        tail0 = min(H, a_n + R + 1) - a_n + 1
                        nc.gpsimd.memset(
                            xst[(k + 1) % 2][:, tail0:XS, :].bitcast(f32), 0.0)
                    x_dma(a_n, (k + 1) % 2)

                # --- L1: one K=30 matmul per chunk; copies alternate
                # DVE/ACT ---
                # --- L1 + L2 merged: the L1 phase is copy-chain-bound
                # (213ns of PE per chunk vs ~635ns of DVE/ACT copy), so
                # interleave L2 convs (copy-independent PE work, data-ready
                # at 2 chunks of lag) to absorb the psum-rotation waits ---
                def do_l2(rr, n):
                    ps = pmain.tile([HID_C, n, W], f32, tag="ps")
                    conv_chunk(ps, w2_sb, h1, rr, n)
                    sl = rr % S
                    nc.scalar.activation(
                        h2[:, sl : sl + n, 1 : 1 + W], ps, Relu, bias=b2_sb
                    )
                    if pending:
                        pending.pop(0)[1]()

                xb = xst[k % 2]
                l2c = list(_chunks(a2, b2r))
                ci = 0
                for rr, n in _chunks(a1, b1r):
                    ps = pmain.tile([HID_C, n, W], f32, tag="ps")
                    s0 = rr - a + 2
                    nc.tensor.matmul(
                        ps, w1_sb[:, :], xb[:, s0 : s0 + n, 1 : 1 + W],
                        start=True, stop=True,
                    )
                    sl = rr % S
                    if ci % 2 == 0:
                        nc.vector.tensor_scalar(
                            h1[:, sl : sl + n, 1 : 1 + W], ps, b1_sb, 0.0, Add, Max
                        )
                    else:
                        nc.scalar.activation(
                            h1[:, sl : sl + n, 1 : 1 + W], ps, Relu, bias=b1_sb
                        )
                    ci += 1
                    if mma_carry and ci >= 2:
                        mma_carry.pop(0)()
                    # gather(k-1) reads t5 rows written by the carried mmA
                    # chunks: it must not be emitted before they drain
                    if ci >= 3 and not mma_carry and gather_pend is not None:
                        pending = gather_pend()
                        gather_pend = None
                    if ci >= 7 and l2c:
                        do_l2(*l2c.pop(0))
                while mma_carry:
                    mma_carry.pop(0)()
                if gather_pend is not None:
                    pending = gather_pend()
                    gather_pend = None
                while l2c:
                    do_l2(*l2c.pop(0))
                while pending:
                    pending.pop(0)[1]()

                # --- L3: reads h2; mmA interleaved so t5 fills (and the
                # gather can start) as h3 rows land ---
                mma_q = list(_chunks(a4, b4r))

                def emit_mma(rr, n, ci):
                    ps = pmain.tile([HID_C, n, W], f32, tag="ps")
                    sl = rr % S
                    nc.tensor.matmul(
                        ps, w4a_sb[:, :], h3[:, sl : sl + n, 1 : 1 + W],
                        start=True, stop=True,
                    )
                    if ci % 2 == 0:
                        nc.vector.tensor_copy(t5[:, sl : sl + n, 1 : 1 + W], ps)
                    else:
                        nc.scalar.activation(
                            t5[:, sl : sl + n, 1 : 1 + W], ps, Ident)

                if k == 0:
                    # row 0 up-slab source is row -1: zero that dest
                    nc.gpsimd.memset(t5s[0:6, 0:1, :], 0.0)
                if last:
                    # row 255 down-slab source is row 256
                    nc.gpsimd.memset(t5s[32:38, b5r - 1 - a5 : b5r - a5, :], 0.0)

                if last:
                    # final strip: interleave gather / tap-sum into the L3
                    # drain so the tail doesn't serialize; y flushes (whose
                    # deps resolve last) go at the very end. Selector pops
                    # lag one gather batch so they never block the in-order
                    # PE queue while their gather DMAs are still in flight.
                    allp = l4b_chunks(a5, b5r, tail=True)
                    chunks_f = [p for p in allp if p[1].__name__ == "emit"]
                    flushes = [p for p in allp if p[1].__name__ == "flush"]
                    gathered = 0
                    gprev = 0
                    gage = 0  # tail_steps since the last gather batch
                    nrl = b5r - a5

                def tail_step(final=False):
                    nonlocal gathered, gprev, gage
                    nrg = nrl - DIRECT_TAIL
                    ready = (nrg if not mma_q else
                             max(0, min(nrg, mma_q[0][0] - 1 - a5)))
                    if ready - gathered >= 12 or (ready == nrg and ready > gathered):
                        gprev = gathered
                        emit_gather(a5, gathered, ready, split=True)
                        gathered = ready
                        gage = 0
                    gage += 1
                    # a gather's DMAs are safely in flight after ~2 more L3/
                    # mmA chunks of PE work; then its selectors won't block
                    # the in-order PE queue
                    lim = gathered if (final and gathered == nrl - DIRECT_TAIL) \
                        or gage > 5 else gprev
                    while chunks_f and chunks_f[0][0] <= lim:
                        chunks_f.pop(0)[1]()

                ci4 = 0
                for rr, n in _chunks(a3, b3r):
                    ps = pmain.tile([HID_C, n, W], f32, tag="ps")
                    conv_chunk(ps, w3_sb, h2, rr, n)
                    sl = rr % S
                    nc.scalar.activation(
                        h3[:, sl : sl + n, 1 : 1 + W], ps, Relu, bias=b3_sb
                    )
                    rr_next = rr + n
                    while mma_q and mma_q[0][0] + mma_q[0][1] + 2 <= rr_next:
                        m0, n0 = mma_q.pop(0)
                        emit_mma(m0, n0, ci4)
                        ci4 += 1
                    if last:
                        tail_step()

                while last and mma_q:
                    m0, n0 = mma_q.pop(0)
                    emit_mma(m0, n0, ci4)
                    ci4 += 1
                    tail_step()

                if not last:
                    # defer the mmA stragglers into the next strip's L1
                    # phase (their ACT/DVE copy deps get ~5us of slack) and
                    # the gather + tap-sum creation right after them
                    for m0, n0 in mma_q:
                        mma_carry.append(
                            lambda m0=m0, n0=n0, c=ci4: emit_mma(m0, n0, c))
                        ci4 += 1
                    mma_q.clear()

                    def gather_pend(a5=a5, b5=b5r):
                        nrg = b5 - a5
                        emit_gather(a5, 0, nrg // 2)
                        emit_gather(a5, nrg // 2, nrg)
                        return l4b_chunks(a5, b5)
                else:
                    # keep the PE clock hot through the gather wait: idle
                    # resets the p-state ramp and the post-gap selectors
                    # would run at half clock
                    while chunks_f or gathered < nrl - DIRECT_TAIL:
                        tail_step(final=True)
                    pending = flushes

            # flush any remaining tap-sum emitters
            while pending:
                pending.pop(0)[1]()

    nc.finalize()
    return nc


_NC_CACHE = {}


def _pack_inputs(x, w1, b1, w2, b2, w3, b3, w4, b4):
    x = np.ascontiguousarray(np.asarray(x, dtype=np.float32))
    w1 = np.asarray(w1, dtype=np.float32)
    w2 = np.asarray(w2, dtype=np.float32)
    w3 = np.asarray(w3, dtype=np.float32)
    w4 = np.asarray(w4, dtype=np.float32)
    # w4a slabs: up@0-5, center@6-11, down@32-37, left@64-69, right@96-101
    w4a = np.zeros((HID_C, HID_C), np.float32)
    w4a[:, 0:OUT_C] = w4[:, :, 1].T          # up
    w4a[:, 6 : 6 + OUT_C] = w4[:, :, 0].T    # center
    w4a[:, 32 : 32 + OUT_C] = w4[:, :, 2].T  # down
    w4a[:, 64 : 64 + OUT_C] = w4[:, :, 3].T  # left
    w4a[:, 96 : 96 + OUT_C] = w4[:, :, 4].T  # right
    s6 = np.zeros((HID_C, OUT_C), np.float32)
    for base in (0, 6, 32, 64, 96):
        s6[base + np.arange(OUT_C), np.arange(OUT_C)] = 1.0
    s6 = s6.astype(ml_dtypes.bfloat16)
    sd = np.zeros((HID_C, 5, OUT_C), np.float32)
    for t, base in enumerate((0, 6, 32, 64, 96)):  # up,cen,dn,lf,rt slabs
        sd[base + np.arange(OUT_C), t, np.arange(OUT_C)] = 1.0
    sd = sd.astype(ml_dtypes.bfloat16)
    bias = np.zeros((HID_C, 4), np.float32)
    bias[:, 0] = np.asarray(b1, np.float32)
    bias[:, 1] = np.asarray(b2, np.float32)
    bias[:, 2] = np.asarray(b3, np.float32)
    bias[:OUT_C, 3] = np.asarray(b4, np.float32)
    common = {
        # w1p[t*6+ic, oc] = w1[oc, ic, t]
        "w1p": np.ascontiguousarray(w1.transpose(2, 1, 0).reshape(5 * IN_C, HID_C)),
        # w23p[ic, t, oc] = w2[oc, ic, t] (t<5) / w3[oc, ic, t-5]
        "w23p": np.ascontiguousarray(np.concatenate(
            [w2.transpose(1, 2, 0), w3.transpose(1, 2, 0)], axis=1)),
        "w4a": w4a,
        "s6": s6,
        "sd": sd,
        "bias": bias,
    }
    return x, common


def kernel(x, w1, b1, w2, b2, w3, b3, w4, b4):
    x, common = _pack_inputs(x, w1, b1, w2, b2, w3, b3, w4, b4)
    if "nc" not in _NC_CACHE:
        _NC_CACHE["nc"] = _build()
    nc = _NC_CACHE["nc"]
    in_maps = [dict(common, x=x[i]) for i in range(N_CORES)]
    res = bass_utils.run_bass_kernel_spmd(nc, in_maps, core_ids=list(range(N_CORES)))
    out = np.stack([res.results[i]["y"] for i in range(N_CORES)], axis=0)
    return out
